# revision 45
# baseline (speedup 1.0000x reference)
"""Trainium2 Bass kernel for nn_MultiHeadAttention_37838661877847.

Full-input contract: kernel(**inputs) takes the complete tensors and returns
the complete output. Internally shards across 8 NeuronCores:
  core c -> batch b = c // 2, head-group g = c % 2 (8 heads, 512 dims each).
Each core computes Q/K/V projections for its (batch, head-group) slice
(column-parallel weights), attention for its 8 heads, and a partial output
projection (row-parallel Wo). Host sums core pairs and adds bo + bv @ Wo.T
(the V bias commutes through softmax-weighted averaging, so it is folded
into the output-projection bias on the host).

Engine-level design (per core), built as ONE interleaved instruction stream
so the scalar engine's softmax-exp (the 266us serial floor: 33.5M exps at
1 elem/lane/cycle) overlaps the tensor engine work (281us):

  - Q_T/K_T stored (dl, s) in bf16; scores come out (k, q) per 128-k tile.
  - exp groups of [128, 1024] PSUM (4 score blocks: 2 heads x ... see sc
    layout below) -> ets tiles in bf16.
  - attn@V is FLIPPED: out (q, dk+1) accumulating over k with the exp tile
    as the stationary operand -> 65-row bf16 matmuls, half the PE rows of
    the (dk+1, q) orientation. V is augmented with a ones column per head so
    the softmax denominator Z lands in column 64; normalization is then a
    per-partition reciprocal+scale on DVE.
  - normalized output (q, dl) is transposed back to (dl, q) via PE-transpose
    through spare score-PSUM space, then the output projection streams wo.
  - emission interleaves projections / attn@V / transposes / out-proj between
    score+exp groups so the scalar engine rarely starves.

mm dtypes: x and w_qk in f32r/bf16 keep projections+scores accurate; the
attention path (probs, V, attn-out, Wo) runs in bf16 (PSUM accumulation is
fp32 throughout).
"""

import sys

sys.path.insert(0, "/opt/trn_rl_repo")

from collections import deque
from contextlib import ExitStack

import numpy as np

import concourse.bass as bass  # noqa: F401
import concourse.tile as tile
from concourse import bacc, masks, mybir
from concourse.bass_utils import run_bass_kernel_spmd

P = 128
DK = 64  # head dim

_CACHE = {}


def build_nc(S=2048, D=1024, DL=512, mm_dtype="float32r", n_cores=8,
             repeats=1, phases="ABC"):
    """Build + compile the per-core Bass program (same program on all cores).

    repeats exists only for timing experiments; production uses the default.
    mm_dtype/phases are accepted for test-harness compatibility (the kernel
    uses a fixed mixed f32r/bf16 precision scheme).
    """
    f32 = mybir.dt.float32
    f32r = mybir.dt.float32r
    bf16 = mybir.dt.bfloat16
    Exp = mybir.ActivationFunctionType.Exp

    H = DL // DK          # 8 local heads
    HP = H // 2           # 4 head pairs (one pair per 128-row q/k tile)
    ET = D // P           # 8 contraction tiles for projections
    ST = S // P           # 16 k tiles (and q tiles)
    NDT = DL // P         # 4 dl tiles
    QC = 512              # projection s-chunk
    NQ = S // QC          # 4
    KG = 4                # k-tiles per exp group
    NKG = ST // KG        # 4
    VW = H * (DK + 1)     # 520: v tile width incl. ones columns

    nc = bacc.Bacc("TRN2", target_bir_lowering=False, num_devices=n_cores)

    xqT = nc.dram_tensor("xqT", [D, S], bf16, kind="ExternalInput")
    xkT = nc.dram_tensor("xkT", [D, S], bf16, kind="ExternalInput")
    xvT = nc.dram_tensor("xvT", [D, S], bf16, kind="ExternalInput")
    wqT = nc.dram_tensor("wqT", [D, DL], bf16, kind="ExternalInput")
    wkT = nc.dram_tensor("wkT", [D, DL], bf16, kind="ExternalInput")
    wvT = nc.dram_tensor("wvT", [D, DL], bf16, kind="ExternalInput")
    woT = nc.dram_tensor("woT", [DL, D], bf16, kind="ExternalInput")
    bqd = nc.dram_tensor("bq", [DL, 1], f32, kind="ExternalInput")
    bkd = nc.dram_tensor("bk", [DL, 1], f32, kind="ExternalInput")
    y = nc.dram_tensor("y", [S, D], f32, kind="ExternalOutput")

    def mm(out, lhsT, rhs, start, stop):
        nc.tensor.matmul(out, lhsT=lhsT, rhs=rhs, start=start, stop=stop)

    with tile.TileContext(nc) as tc, ExitStack() as top:
        top.enter_context(
            nc.allow_low_precision(
                reason="attention path in bf16; PSUM accumulation stays fp32"
            )
        )
        persist = top.enter_context(tc.tile_pool(name="persist", bufs=1))
        qt = [persist.tile([P, S], bf16, tag=f"qt{i}", name=f"qt{i}") for i in range(NDT)]
        kt = [persist.tile([P, S], bf16, tag=f"kt{i}", name=f"kt{i}") for i in range(NDT)]
        vt = [persist.tile([P, VW], bf16, tag=f"vt{i}", name=f"vt{i}") for i in range(ST)]
        oaT = [persist.tile([P, S], bf16, tag=f"oaT{i}", name=f"oaT{i}") for i in range(NDT)]
        ident = persist.tile([P, P], f32, tag="ident", name="ident")
        bq_t = persist.tile([P, NDT], f32, tag="bq", name="bq")
        bk_t = persist.tile([P, NDT], f32, tag="bk", name="bk")

        masks.make_identity(nc, ident[:])
        warm = persist.tile([P, P], bf16, tag="warm", name="warm")
        nc.vector.memset(warm[:], 0.0)
        for i in range(ST):
            # ones columns for the softmax denominator; data cols overwritten
            nc.vector.memset(vt[i][:], 1.0)

        # PSUM: scores/exp 2x[128,1024] (4 banks) + attn@V accum 2x[128,260]
        # (2 banks) + generic matmul 2x[128,512] (2 banks) = 8 banks.
        spool = top.enter_context(tc.tile_pool(name="spool", bufs=2, space="PSUM"))
        acpool = top.enter_context(tc.tile_pool(name="acpool", bufs=2, space="PSUM"))
        gpool = top.enter_context(tc.tile_pool(name="gpool", bufs=2, space="PSUM"))

        # weight/x pools for Q (live through all Q chunks); wide layouts:
        # w tiles hold all ET contraction blocks side by side (one DMA each).
        wqp = top.enter_context(tc.tile_pool(name="wqp", bufs=1))
        wq = wqp.tile([P, ET * DL], bf16, tag="wq", name="wq")
        xqp = top.enter_context(tc.tile_pool(name="xqp", bufs=1))

        # long-lived attention pools (opened before any scoped pool so that
        # mid-stream pool closes stay LIFO)
        etsp = top.enter_context(tc.tile_pool(name="etsp", bufs=2))
        oasp = top.enter_context(tc.tile_pool(name="oasp", bufs=4))
        yvp = top.enter_context(tc.tile_pool(name="yvp", bufs=2))
        rcp = top.enter_context(tc.tile_pool(name="rcp", bufs=4))

        for _rep in range(repeats):
            # ---------------- pools for K and Q chunk streams -------------
            vstate = {}
            s3 = ExitStack()
            s2 = ExitStack()
            vxa = s2.enter_context(tc.tile_pool(name="vxa", bufs=1))
            vstate["wv"] = vxa.tile([P, ET * DL], bf16, tag="wv", name="wv")
            vstate["xv0"] = vxa.tile([P, ET * (S // 2)], bf16, tag="xv0",
                                     name="xv0")
            s1 = ExitStack()
            kx = s1.enter_context(tc.tile_pool(name="kx", bufs=2))
            wkp = s1.enter_context(tc.tile_pool(name="wkp", bufs=1))
            wk = wkp.tile([P, ET * DL], bf16, tag="wk", name="wk")

            def load_xk(c, eng=None):
                xkc = kx.tile([P, ET * QC], bf16, tag="xk", name="xk")
                (eng or nc.sync).dma_start(
                    out=xkc[:].rearrange("p (e s) -> p e s", e=ET),
                    in_=xkT[:, c * QC : (c + 1) * QC].rearrange(
                        "(e p) s -> p e s", p=P),
                )
                vstate["xk"] = xkc

            def load_xq(c, eng):
                xqc = xqp.tile([P, ET * QC], bf16, tag="xq", name="xq")
                eng.dma_start(
                    out=xqc[:].rearrange("p (e s) -> p e s", e=ET),
                    in_=xqT[:, c * QC : (c + 1) * QC].rearrange(
                        "(e p) s -> p e s", p=P),
                )
                vstate["xq"] = xqc

            def proj_piece(c, dch, w, xkey, bias, out_tiles):
                """One (chunk, dl-tile) projection: out (dl 128, s 512) + bias."""
                x = vstate[xkey]
                gp = gpool.tile([P, QC], f32, tag="gp", name="gp")
                for e in range(ET):
                    mm(gp[:], w[:, e * DL + dch * P : e * DL + (dch + 1) * P],
                       x[:, e * QC : (e + 1) * QC], e == 0, e == ET - 1)
                nc.vector.tensor_scalar_add(
                    out_tiles[dch][:, c * QC : (c + 1) * QC], gp[:],
                    bias[:, dch : dch + 1]
                )

            # --- V pools: wv + the first s-half of xv preload alongside the
            # K pool (slot 0); the second s-half lands in the space the K pool
            # frees. V projection runs head-half-major so attn@V for heads 0-3
            # unblocks as early as possible.
            SH = S // 2

            def load_wv():
                nc.sync.dma_start(
                    out=vstate["wv"][:].rearrange("p (e d) -> p e d", e=ET),
                    in_=wvT[:].rearrange("(e p) d -> p e d", p=P),
                )

            def load_xv0():
                nc.sync.dma_start(
                    out=vstate["xv0"][:].rearrange("p (e s) -> p e s", e=ET),
                    in_=xvT[:, 0:SH].rearrange("(e p) s -> p e s", p=P),
                )

            def open_vx():
                vxb = s2.enter_context(tc.tile_pool(name="vxb", bufs=1))
                vstate["xv1"] = vxb.tile([P, ET * SH], bf16, tag="xv1", name="xv1")
                nc.sync.dma_start(
                    out=vstate["xv1"][:].rearrange("p (e s) -> p e s", e=ET),
                    in_=xvT[:, SH:S].rearrange("(e p) s -> p e s", p=P),
                )

            def vproj_piece(st, qtr):
                """V projection for (s-tile st, head pair qtr): 2 heads."""
                Q4 = DL // 4
                gp = gpool.tile([P, QC], f32, tag="gp", name="gp")
                wv = vstate["wv"]
                xv = vstate["xv0"] if st < ST // 2 else vstate["xv1"]
                stl = st % (ST // 2)
                for e in range(ET):
                    mm(gp[:, 0:Q4],
                       xv[:, e * SH + stl * P : e * SH + (stl + 1) * P],
                       wv[:, e * DL + qtr * Q4 : e * DL + (qtr + 1) * Q4],
                       e == 0, e == ET - 1)
                nc.vector.tensor_copy(
                    vt[st][:].rearrange("p (h c) -> p h c", h=H)
                    [:, qtr * 2 : (qtr + 1) * 2, 0:DK],
                    gp[:, 0:Q4].rearrange("p (h c) -> p h c", h=2),
                )

            state = {
                "ets": {},    # (qt_idx, hp, kg) -> tile  (live window)
                "ac": {},     # (qt_idx, hgrp) -> tile
                "oas": {},    # qt_idx -> tile
                "wo": None,
            }

            def sc_use(qi, hp, kg):
                """Scores + exp for (q-tile qi, head pair hp, k-group kg)."""
                sc = spool.tile([P, 2 * KG * P], f32, tag="sc", name="sc")
                for hloc in range(2):
                    h = 2 * hp + hloc
                    r0 = hloc * DK
                    for ktl in range(KG):
                        ki = kg * KG + ktl
                        mm(
                            sc[:, hloc * KG * P + ktl * P : hloc * KG * P + (ktl + 1) * P],
                            kt[hp][r0 : r0 + DK, ki * P : (ki + 1) * P],
                            qt[hp][r0 : r0 + DK, qi * P : (qi + 1) * P],
                            True,
                            True,
                        )
                et = etsp.tile([P, 2 * KG * P], bf16, tag=f"et{hp}_{kg}",
                               name=f"et{hp}_{kg}")
                nc.scalar.activation(et[:], sc[:], Exp)
                state["ets"][(qi, hp, kg)] = et

            def attnv_piece(qi, h, kg):
                """attn@V for (q-tile qi, head h, k-group kg): 4 x 65-row mms."""
                hgrp, hidx = divmod(h, 4)
                key = (qi, hgrp)
                if key not in state["ac"]:
                    # padded to a full 2KB bank; cols 0-259 used (4 heads x 65)
                    state["ac"][key] = acpool.tile([P, 512], f32, tag="ac", name="ac")
                ac = state["ac"][key]
                et = state["ets"][(qi, h // 2, kg)]
                hloc = h % 2
                for ktl in range(KG):
                    ki = kg * KG + ktl
                    mm(
                        ac[:, hidx * (DK + 1) : (hidx + 1) * (DK + 1)],
                        et[:, hloc * KG * P + ktl * P : hloc * KG * P + (ktl + 1) * P],
                        vt[ki][:, h * (DK + 1) : (h + 1) * (DK + 1)],
                        ki == 0,
                        ki == ST - 1,
                    )
                if hloc == 1:
                    del state["ets"][(qi, h // 2, kg)]

            def norm_piece(qi, hgrp):
                """Normalize 4 heads: oa_s[:, hgrp*256:+256] = num * (1/Z)."""
                if qi not in state["oas"]:
                    state["oas"][qi] = oasp.tile([P, DL], f32, tag="oas", name="oas")
                oas = state["oas"][qi]
                ac = state["ac"].pop((qi, hgrp))
                acr = ac[:, 0 : 4 * (DK + 1)].rearrange("p (h c) -> p h c", h=4)
                rc = rcp.tile([P, 4], f32, tag="rc", name="rc")
                nc.vector.reciprocal(rc[:], acr[:, :, DK])
                for hh in range(4):
                    nc.vector.tensor_scalar_mul(
                        oas[:, hgrp * 4 * DK + hh * DK : hgrp * 4 * DK + (hh + 1) * DK],
                        acr[:, hh, 0:DK],
                        rc[:, hh : hh + 1],
                    )

            def transp_qt(qi):
                """Transpose oa_s (q, dl) -> oaT (dl, q) for one q-tile."""
                sc = spool.tile([P, 2 * KG * P], f32, tag="sc", name="sc")
                oas = state["oas"].pop(qi)
                for dlb in range(NDT):
                    nc.tensor.transpose(
                        sc[:, dlb * P : (dlb + 1) * P],
                        oas[:, dlb * P : (dlb + 1) * P],
                        ident[:],
                    )
                for dlb in range(NDT):
                    nc.vector.tensor_copy(
                        oaT[dlb][:, qi * P : (qi + 1) * P], sc[:, dlb * P : (dlb + 1) * P]
                    )

            def load_wo():
                wop = s3.enter_context(tc.tile_pool(name="wop", bufs=1))
                wo = wop.tile([P, NDT * D], bf16, tag="wo", name="wo")
                nc.sync.dma_start(
                    out=wo[:].rearrange("p (i d) -> p i d", i=NDT),
                    in_=woT[:].rearrange("(i p) d -> p i d", p=P),
                )
                state["wo"] = wo

            def c_piece(st, fc):
                """Output projection for (s-tile st, f-chunk fc)."""
                wo = state["wo"]
                gp = gpool.tile([P, QC], f32, tag="gp", name="gp")
                for dl in range(NDT):
                    mm(gp[:], oaT[dl][:, st * P : (st + 1) * P],
                       wo[:, dl * D + fc * QC : dl * D + (fc + 1) * QC],
                       dl == 0, dl == NDT - 1)
                yv = yvp.tile([P, QC], f32, tag="yv", name="yv")
                nc.vector.tensor_copy(yv[:], gp[:])
                nc.sync.dma_start(
                    out=y[st * P : (st + 1) * P, fc * QC : (fc + 1) * QC], in_=yv[:]
                )

            # ------------- interleaved emission ---------------------------
            # One FIFO of side pieces per slot, drained between score+exp
            # groups under a PE-lead budget, force-drained at slot end (and at
            # the MID marker before the hp2/hp3 half). Estimated PE ns/piece.
            EXP_NS, SC_NS = 1040.0, 430.0
            COST = {}

            def piece_cost(p):
                fn = p[0]
                if fn == proj_piece:
                    return 1750.0
                if fn == vproj_piece:
                    return 450.0
                if fn == attnv_piece:
                    return 160.0
                if fn == transp_qt:
                    return 520.0
                if fn == c_piece:
                    return 900.0
                return 0.0

            def run_piece(p):
                if p[0] == "loadxk":
                    load_xk(p[1])
                elif p[0] == "loadxq":
                    load_xq(p[1], nc.sync)
                elif p[0] == "loadwo":
                    load_wo()
                elif p[0] == "closes1":
                    s1.close()
                elif p[0] == "openvx":
                    open_vx()
                elif p[0] == "loadwv":
                    load_wv()
                elif p[0] == "loadxv0":
                    load_xv0()
                elif p[0] == "closes2":
                    s2.close()
                elif p[0] == "HPM":
                    pass
                else:
                    p[0](*p[1:])

            def attnv_hp(qi, hp):
                out = []
                for h in (2 * hp, 2 * hp + 1):
                    for kg in range(NKG):
                        out.append((attnv_piece, qi, h, kg))
                return out

            slot_inter = [[] for _ in range(ST)]
            slot_markers = [set() for _ in range(ST)]
            slot_inter[0] += [("loadwv",), ("loadxv0",)]
            slot_inter[0] += [(vproj_piece, st, 0) for st in range(ST // 2)]
            slot_inter[1] += [(vproj_piece, st, 0) for st in range(ST // 2, ST)]
            slot_inter[1] += attnv_hp(0, 0)
            slot_inter[1] += [(vproj_piece, st, 1) for st in range(ST)]
            slot_inter[1] += attnv_hp(0, 1) + [(norm_piece, 0, 0)]
            slot_inter[2] += [(vproj_piece, st, 2) for st in range(ST)]
            slot_inter[2] += attnv_hp(0, 2) + [("HPM", 2)]
            slot_inter[2] += [(vproj_piece, st, 3) for st in range(ST)]
            slot_inter[2] += attnv_hp(0, 3) + [(norm_piece, 0, 1), ("HPM", 3)]
            slot_inter[2] += [("closes2",), ("loadwo",)]
            slot_inter[2] += (attnv_hp(1, 0) + attnv_hp(1, 1)
                              + [(norm_piece, 1, 0)]
                              + attnv_hp(1, 2) + attnv_hp(1, 3)
                              + [(norm_piece, 1, 1)])
            slot_markers[2] = {2, 3}
            slot_inter[3] += [(transp_qt, 0), (transp_qt, 1)]
            for j in range(3, ST):
                slot_inter[j] += (attnv_hp(j - 1, 0) + attnv_hp(j - 1, 1)
                                  + [(norm_piece, j - 1, 0)]
                                  + attnv_hp(j - 1, 2) + attnv_hp(j - 1, 3)
                                  + [(norm_piece, j - 1, 1)])
                if j - 1 >= 2:
                    slot_inter[j].append((transp_qt, j - 1))
            # out-projection: early s-tiles deferred to late slots (the early
            # slots carry the V/K/Q overload), the rest two slots after their
            # transpose.
            for st in range(0, 5):
                slot_inter[11 + st] += [(c_piece, st, 0), (c_piece, st, 1)]
            for st in range(5, 14):
                slot_inter[st + 2] += [(c_piece, st, 0), (c_piece, st, 1)]
            # Q chunk c: DMA early, project each dl-tile just before the
            # first score group of slot 4c that needs it.
            slot_hp = [[[] for _ in range(HP)] for _ in range(ST)]
            for c in (1, 2, 3):
                slot_inter[4 * c - 2].append(("loadxq", c))
                for dch in range(NDT):
                    slot_hp[4 * c][dch].append(
                        (proj_piece, c, dch, wq, "xq", bq_t, qt))

            # ---------------- head + slot loop ----------------------------
            nc.scalar.dma_start(
                out=wk[:].rearrange("p (e d) -> p e d", e=ET),
                in_=wkT[:].rearrange("(e p) d -> p e d", p=P))
            load_xk(0)
            load_xq(0, nc.scalar)
            nc.sync.dma_start(
                out=wq[:].rearrange("p (e d) -> p e d", e=ET),
                in_=wqT[:].rearrange("(e p) d -> p e d", p=P))
            nc.sync.dma_start(out=bk_t[:].rearrange("p i -> p i ()"),
                              in_=bkd[:].rearrange("(i p) o -> p i o", p=P))
            nc.sync.dma_start(out=bq_t[:].rearrange("p i -> p i ()"),
                              in_=bqd[:].rearrange("(i p) o -> p i o", p=P))
            # PE clock warm-up while the first loads are in flight: harmless
            # zero matmuls keep the PE busy so it reaches full p-state before
            # the first projection.
            for wu in range(26):
                wsc = spool.tile([P, 2 * KG * P], f32, tag="sc", name="sc")
                for j in range(4):
                    mm(wsc[:, j * P : (j + 1) * P], warm[:], warm[:], True, True)

            budget = [0.0]

            def drain(inter, force=False, to_marker=None):
                while inter:
                    if (not force and to_marker is None
                            and piece_cost(inter[0]) > budget[0]):
                        return
                    p = inter.popleft()
                    run_piece(p)
                    budget[0] -= piece_cost(p)
                    if to_marker is not None and p == ("HPM", to_marker):
                        return

            def do_sc(qi, hp, kg, inter):
                sc_use(qi, hp, kg)
                budget[0] = min(budget[0] + (EXP_NS - SC_NS), 4000.0)
                drain(inter)

            # slot 0: interleave K/Q chunk-0 projections with the first score
            # groups (head pair hp becomes ready as soon as dl-tile hp
            # projects), then kg 1-3 as K chunks 1-3 land.
            inter0 = deque(slot_inter[0])
            for dch in range(NDT):
                proj_piece(0, dch, wk, "xk", bk_t, kt)
                proj_piece(0, dch, wq, "xq", bq_t, qt)
                sc_use(0, dch, 0)
            xk_next = {}
            load_xk(1)
            xk_next[1] = vstate["xk"]
            for kg in range(1, NKG):
                vstate["xk"] = xk_next[kg]
                if kg + 1 < NKG:
                    load_xk(kg + 1)
                    xk_next[kg + 1] = vstate["xk"]
                    vstate["xk"] = xk_next[kg]
                for dch in range(NDT):
                    proj_piece(kg, dch, wk, "xk", bk_t, kt)
                for hp in range(HP):
                    do_sc(0, hp, kg, inter0)
            drain(inter0, force=True)
            s1.close()
            open_vx()

            for qi in range(1, ST):
                inter = deque(slot_inter[qi])
                for hp in range(HP):
                    if hp in slot_markers[qi]:
                        drain(inter, to_marker=hp)
                    for p in slot_hp[qi][hp]:
                        run_piece(p)
                    for kg in range(NKG):
                        do_sc(qi, hp, kg, inter)
                drain(inter, force=True)

            # ---------------- tail ----------------------------------------
            for piece in (attnv_hp(ST - 1, 0) + attnv_hp(ST - 1, 1)
                          + [(norm_piece, ST - 1, 0)]
                          + attnv_hp(ST - 1, 2) + attnv_hp(ST - 1, 3)
                          + [(norm_piece, ST - 1, 1)]):
                run_piece(piece)
            transp_qt(ST - 1)
            for st in range(ST - 2, ST):
                for fc in range(D // QC):
                    c_piece(st, fc)
            s3.close()

    nc.compile()
    return nc


def make_in_maps(query, key, value, Wq, bq, Wk, bk, Wv, bv, n_cores=8,
                 mm_dtype="float32r"):
    """Host-side sharding: slice weights Megatron-style, transpose activations."""
    import ml_dtypes

    bft = ml_dtypes.bfloat16
    q = np.asarray(query, dtype=np.float32)
    k = np.asarray(key, dtype=np.float32)
    v = np.asarray(value, dtype=np.float32)
    Wq = np.asarray(Wq, dtype=np.float32)
    Wk = np.asarray(Wk, dtype=np.float32)
    Wv = np.asarray(Wv, dtype=np.float32)
    bq = np.asarray(bq, dtype=np.float32)
    bk = np.asarray(bk, dtype=np.float32)
    D = Wq.shape[0]
    DL = D // (n_cores // q.shape[0])
    scale = 1.0 / np.sqrt(np.float32(DK))
    in_maps = []
    for c in range(n_cores):
        b, g = divmod(c, n_cores // q.shape[0])
        sl = slice(DL * g, DL * (g + 1))
        in_maps.append(
            {
                "xqT": np.ascontiguousarray(q[b].T).astype(bft),
                "xkT": np.ascontiguousarray(k[b].T).astype(bft),
                "xvT": np.ascontiguousarray(v[b].T).astype(bft),
                "wqT": (np.ascontiguousarray(Wq[sl].T) * scale).astype(bft),
                "wkT": np.ascontiguousarray(Wk[sl].T).astype(bft),
                "wvT": np.ascontiguousarray(Wv[sl].T).astype(bft),
                "bq": np.ascontiguousarray((bq[sl] * scale).reshape(DL, 1)),
                "bk": np.ascontiguousarray(bk[sl].reshape(DL, 1)),
            }
        )
    return in_maps


def add_wo_maps(in_maps, Wo, n_cores=8, n_batch=4, mm_dtype="float32r"):
    import ml_dtypes

    Wo = np.asarray(Wo, dtype=np.float32)
    D = Wo.shape[0]
    DL = D // (n_cores // n_batch)
    for c in range(n_cores):
        _, g = divmod(c, n_cores // n_batch)
        sl = slice(DL * g, DL * (g + 1))
        in_maps[c]["woT"] = np.ascontiguousarray(Wo[:, sl].T).astype(ml_dtypes.bfloat16)
    return in_maps


MM_DTYPE = "float32r"


def kernel(query, key, value, Wq, bq, Wk, bk, Wv, bv, Wo, bo):
    if "nc" not in _CACHE:
        _CACHE["nc"] = build_nc(mm_dtype=MM_DTYPE)
    nc = _CACHE["nc"]
    n_cores = 8
    in_maps = make_in_maps(
        query, key, value, Wq, bq, Wk, bk, Wv, bv, n_cores, MM_DTYPE
    )
    add_wo_maps(in_maps, Wo, n_cores, np.asarray(query).shape[0], MM_DTYPE)
    res = run_bass_kernel_spmd(nc, in_maps, list(range(n_cores)))
    ys = [res.results[c]["y"] for c in range(n_cores)]
    bo = np.asarray(bo, dtype=np.float32)
    bv = np.asarray(bv, dtype=np.float32)
    Wo = np.asarray(Wo, dtype=np.float32)
    const = bo + bv @ Wo.T
    out = np.stack([ys[2 * b] + ys[2 * b + 1] for b in range(4)]) + const[None, None, :]
    return out.astype(np.float32)


# revision 48
# speedup vs baseline: 1.0026x; 1.0026x over previous
"""Trainium2 Bass kernel for nn_MultiHeadAttention_37838661877847.

Full-input contract: kernel(**inputs) takes the complete tensors and returns
the complete output. Internally shards across 8 NeuronCores:
  core c -> batch b = c // 2, head-group g = c % 2 (8 heads, 512 dims each).
Each core computes Q/K/V projections for its (batch, head-group) slice
(column-parallel weights), attention for its 8 heads, and a partial output
projection (row-parallel Wo). Host sums core pairs and adds bo + bv @ Wo.T
(the V bias commutes through softmax-weighted averaging, so it is folded
into the output-projection bias on the host).

Engine-level design (per core), built as ONE interleaved instruction stream
so the scalar engine's softmax-exp (the 266us serial floor: 33.5M exps at
1 elem/lane/cycle) overlaps the tensor engine work (281us):

  - Q_T/K_T stored (dl, s) in bf16; scores come out (k, q) per 128-k tile.
  - exp groups of [128, 1024] PSUM (4 score blocks: 2 heads x ... see sc
    layout below) -> ets tiles in bf16.
  - attn@V is FLIPPED: out (q, dk+1) accumulating over k with the exp tile
    as the stationary operand -> 65-row bf16 matmuls, half the PE rows of
    the (dk+1, q) orientation. V is augmented with a ones column per head so
    the softmax denominator Z lands in column 64; normalization is then a
    per-partition reciprocal+scale on DVE.
  - normalized output (q, dl) is transposed back to (dl, q) via PE-transpose
    through spare score-PSUM space, then the output projection streams wo.
  - emission interleaves projections / attn@V / transposes / out-proj between
    score+exp groups so the scalar engine rarely starves.

mm dtypes: x and w_qk in f32r/bf16 keep projections+scores accurate; the
attention path (probs, V, attn-out, Wo) runs in bf16 (PSUM accumulation is
fp32 throughout).
"""

import sys

sys.path.insert(0, "/opt/trn_rl_repo")

from collections import deque
from contextlib import ExitStack

import numpy as np

import concourse.bass as bass  # noqa: F401
import concourse.tile as tile
from concourse import bacc, masks, mybir
from concourse.bass_utils import run_bass_kernel_spmd

P = 128
DK = 64  # head dim

_CACHE = {}


def build_nc(S=2048, D=1024, DL=512, mm_dtype="float32r", n_cores=8,
             repeats=1, phases="ABC"):
    """Build + compile the per-core Bass program (same program on all cores).

    repeats exists only for timing experiments; production uses the default.
    mm_dtype/phases are accepted for test-harness compatibility (the kernel
    uses a fixed mixed f32r/bf16 precision scheme).
    """
    f32 = mybir.dt.float32
    f32r = mybir.dt.float32r
    bf16 = mybir.dt.bfloat16
    Exp = mybir.ActivationFunctionType.Exp

    H = DL // DK          # 8 local heads
    HP = H // 2           # 4 head pairs (one pair per 128-row q/k tile)
    ET = D // P           # 8 contraction tiles for projections
    ST = S // P           # 16 k tiles (and q tiles)
    NDT = DL // P         # 4 dl tiles
    QC = 512              # projection s-chunk
    NQ = S // QC          # 4
    KG = 4                # k-tiles per exp group
    NKG = ST // KG        # 4
    VW = H * (DK + 1)     # 520: v tile width incl. ones columns

    nc = bacc.Bacc("TRN2", target_bir_lowering=False, num_devices=n_cores)

    xqT = nc.dram_tensor("xqT", [D, S], bf16, kind="ExternalInput")
    xkT = nc.dram_tensor("xkT", [D, S], bf16, kind="ExternalInput")
    xvT = nc.dram_tensor("xvT", [D, S], bf16, kind="ExternalInput")
    wqT = nc.dram_tensor("wqT", [D, DL], bf16, kind="ExternalInput")
    wkT = nc.dram_tensor("wkT", [D, DL], bf16, kind="ExternalInput")
    wvT = nc.dram_tensor("wvT", [D, DL], bf16, kind="ExternalInput")
    woT = nc.dram_tensor("woT", [DL, D], bf16, kind="ExternalInput")
    bqd = nc.dram_tensor("bq", [DL, 1], f32, kind="ExternalInput")
    bkd = nc.dram_tensor("bk", [DL, 1], f32, kind="ExternalInput")
    y = nc.dram_tensor("y", [S, D], f32, kind="ExternalOutput")

    def mm(out, lhsT, rhs, start, stop):
        nc.tensor.matmul(out, lhsT=lhsT, rhs=rhs, start=start, stop=stop)

    with tile.TileContext(nc) as tc, ExitStack() as top:
        top.enter_context(
            nc.allow_low_precision(
                reason="attention path in bf16; PSUM accumulation stays fp32"
            )
        )
        persist = top.enter_context(tc.tile_pool(name="persist", bufs=1))
        qt = [persist.tile([P, S], bf16, tag=f"qt{i}", name=f"qt{i}") for i in range(NDT)]
        kt = [persist.tile([P, S], bf16, tag=f"kt{i}", name=f"kt{i}") for i in range(NDT)]
        vt = [persist.tile([P, VW], bf16, tag=f"vt{i}", name=f"vt{i}") for i in range(ST)]
        oaT = [persist.tile([P, S], bf16, tag=f"oaT{i}", name=f"oaT{i}") for i in range(NDT)]
        ident = persist.tile([P, P], f32, tag="ident", name="ident")
        bq_t = persist.tile([P, NDT], f32, tag="bq", name="bq")
        bk_t = persist.tile([P, NDT], f32, tag="bk", name="bk")

        masks.make_identity(nc, ident[:])
        warm = persist.tile([P, P], bf16, tag="warm", name="warm")
        nc.vector.memset(warm[:], 0.0)
        for i in range(ST):
            # ones columns for the softmax denominator; data cols overwritten
            nc.vector.memset(vt[i][:], 1.0)

        # PSUM: scores/exp 2x[128,1024] (4 banks) + attn@V accum 2x[128,260]
        # (2 banks) + generic matmul 2x[128,512] (2 banks) = 8 banks.
        spool = top.enter_context(tc.tile_pool(name="spool", bufs=2, space="PSUM"))
        acpool = top.enter_context(tc.tile_pool(name="acpool", bufs=2, space="PSUM"))
        gpool = top.enter_context(tc.tile_pool(name="gpool", bufs=2, space="PSUM"))

        # weight/x pools for Q (live through all Q chunks); wide layouts:
        # w tiles hold all ET contraction blocks side by side (one DMA each).
        wqp = top.enter_context(tc.tile_pool(name="wqp", bufs=1))
        wq = wqp.tile([P, ET * DL], bf16, tag="wq", name="wq")
        xqp = top.enter_context(tc.tile_pool(name="xqp", bufs=1))

        # long-lived attention pools (opened before any scoped pool so that
        # mid-stream pool closes stay LIFO)
        etsp = top.enter_context(tc.tile_pool(name="etsp", bufs=2))
        oasp = top.enter_context(tc.tile_pool(name="oasp", bufs=4))
        yvp = top.enter_context(tc.tile_pool(name="yvp", bufs=2))
        rcp = top.enter_context(tc.tile_pool(name="rcp", bufs=4))

        for _rep in range(repeats):
            # ---------------- pools for K and Q chunk streams -------------
            vstate = {}
            s3 = ExitStack()
            s2 = ExitStack()
            vxa = s2.enter_context(tc.tile_pool(name="vxa", bufs=1))
            vstate["wv"] = vxa.tile([P, ET * DL], bf16, tag="wv", name="wv")
            vstate["xv0"] = vxa.tile([P, ET * (S // 2)], bf16, tag="xv0",
                                     name="xv0")
            s1 = ExitStack()
            kx = s1.enter_context(tc.tile_pool(name="kx", bufs=2))
            wkp = s1.enter_context(tc.tile_pool(name="wkp", bufs=1))
            wk = wkp.tile([P, ET * DL], bf16, tag="wk", name="wk")

            def load_xk(c, eng=None):
                xkc = kx.tile([P, ET * QC], bf16, tag="xk", name="xk")
                (eng or nc.sync).dma_start(
                    out=xkc[:].rearrange("p (e s) -> p e s", e=ET),
                    in_=xkT[:, c * QC : (c + 1) * QC].rearrange(
                        "(e p) s -> p e s", p=P),
                )
                vstate["xk"] = xkc

            def load_xq(c, eng):
                xqc = xqp.tile([P, ET * QC], bf16, tag="xq", name="xq")
                eng.dma_start(
                    out=xqc[:].rearrange("p (e s) -> p e s", e=ET),
                    in_=xqT[:, c * QC : (c + 1) * QC].rearrange(
                        "(e p) s -> p e s", p=P),
                )
                vstate["xq"] = xqc

            def proj_piece(c, dch, w, xkey, bias, out_tiles):
                """One (chunk, dl-tile) projection: out (dl 128, s 512) + bias."""
                x = vstate[xkey]
                gp = gpool.tile([P, QC], f32, tag="gp", name="gp")
                for e in range(ET):
                    mm(gp[:], w[:, e * DL + dch * P : e * DL + (dch + 1) * P],
                       x[:, e * QC : (e + 1) * QC], e == 0, e == ET - 1)
                nc.vector.tensor_scalar_add(
                    out_tiles[dch][:, c * QC : (c + 1) * QC], gp[:],
                    bias[:, dch : dch + 1]
                )

            # --- V pools: wv + the first s-half of xv preload alongside the
            # K pool (slot 0); the second s-half lands in the space the K pool
            # frees. V projection runs head-half-major so attn@V for heads 0-3
            # unblocks as early as possible.
            SH = S // 2

            def load_wv():
                nc.sync.dma_start(
                    out=vstate["wv"][:].rearrange("p (e d) -> p e d", e=ET),
                    in_=wvT[:].rearrange("(e p) d -> p e d", p=P),
                )

            def load_xv0():
                nc.sync.dma_start(
                    out=vstate["xv0"][:].rearrange("p (e s) -> p e s", e=ET),
                    in_=xvT[:, 0:SH].rearrange("(e p) s -> p e s", p=P),
                )

            def open_vx():
                vxb = s2.enter_context(tc.tile_pool(name="vxb", bufs=1))
                vstate["xv1"] = vxb.tile([P, ET * SH], bf16, tag="xv1", name="xv1")
                nc.sync.dma_start(
                    out=vstate["xv1"][:].rearrange("p (e s) -> p e s", e=ET),
                    in_=xvT[:, SH:S].rearrange("(e p) s -> p e s", p=P),
                )

            def vproj_piece(st, qtr):
                """V projection for (s-tile st, head pair qtr): 2 heads."""
                Q4 = DL // 4
                gp = gpool.tile([P, QC], f32, tag="gp", name="gp")
                wv = vstate["wv"]
                xv = vstate["xv0"] if st < ST // 2 else vstate["xv1"]
                stl = st % (ST // 2)
                for e in range(ET):
                    mm(gp[:, 0:Q4],
                       xv[:, e * SH + stl * P : e * SH + (stl + 1) * P],
                       wv[:, e * DL + qtr * Q4 : e * DL + (qtr + 1) * Q4],
                       e == 0, e == ET - 1)
                nc.vector.tensor_copy(
                    vt[st][:].rearrange("p (h c) -> p h c", h=H)
                    [:, qtr * 2 : (qtr + 1) * 2, 0:DK],
                    gp[:, 0:Q4].rearrange("p (h c) -> p h c", h=2),
                )

            state = {
                "ets": {},    # (qt_idx, hp, kg) -> tile  (live window)
                "ac": {},     # (qt_idx, hgrp) -> tile
                "oas": {},    # qt_idx -> tile
                "wo": None,
            }

            def sc_use(qi, hp, kg):
                """Scores + exp for (q-tile qi, head pair hp, k-group kg)."""
                sc = spool.tile([P, 2 * KG * P], f32, tag="sc", name="sc")
                for hloc in range(2):
                    h = 2 * hp + hloc
                    r0 = hloc * DK
                    for ktl in range(KG):
                        ki = kg * KG + ktl
                        mm(
                            sc[:, hloc * KG * P + ktl * P : hloc * KG * P + (ktl + 1) * P],
                            kt[hp][r0 : r0 + DK, ki * P : (ki + 1) * P],
                            qt[hp][r0 : r0 + DK, qi * P : (qi + 1) * P],
                            True,
                            True,
                        )
                et = etsp.tile([P, 2 * KG * P], bf16, tag=f"et{hp}_{kg}",
                               name=f"et{hp}_{kg}")
                nc.scalar.activation(et[:], sc[:], Exp)
                state["ets"][(qi, hp, kg)] = et

            def attnv_piece(qi, h, kg):
                """attn@V for (q-tile qi, head h, k-group kg): 4 x 65-row mms."""
                hgrp, hidx = divmod(h, 4)
                key = (qi, hgrp)
                if key not in state["ac"]:
                    # padded to a full 2KB bank; cols 0-259 used (4 heads x 65)
                    state["ac"][key] = acpool.tile([P, 512], f32, tag="ac", name="ac")
                ac = state["ac"][key]
                et = state["ets"][(qi, h // 2, kg)]
                hloc = h % 2
                for ktl in range(KG):
                    ki = kg * KG + ktl
                    mm(
                        ac[:, hidx * (DK + 1) : (hidx + 1) * (DK + 1)],
                        et[:, hloc * KG * P + ktl * P : hloc * KG * P + (ktl + 1) * P],
                        vt[ki][:, h * (DK + 1) : (h + 1) * (DK + 1)],
                        ki == 0,
                        ki == ST - 1,
                    )
                if hloc == 1:
                    del state["ets"][(qi, h // 2, kg)]

            def norm_piece(qi, hgrp):
                """Normalize 4 heads: oa_s[:, hgrp*256:+256] = num * (1/Z)."""
                if qi not in state["oas"]:
                    state["oas"][qi] = oasp.tile([P, DL], f32, tag="oas", name="oas")
                oas = state["oas"][qi]
                ac = state["ac"].pop((qi, hgrp))
                acr = ac[:, 0 : 4 * (DK + 1)].rearrange("p (h c) -> p h c", h=4)
                rc = rcp.tile([P, 4], f32, tag="rc", name="rc")
                nc.vector.reciprocal(rc[:], acr[:, :, DK])
                for hh in range(4):
                    nc.vector.tensor_scalar_mul(
                        oas[:, hgrp * 4 * DK + hh * DK : hgrp * 4 * DK + (hh + 1) * DK],
                        acr[:, hh, 0:DK],
                        rc[:, hh : hh + 1],
                    )

            def transp_qt(qi):
                """Transpose oa_s (q, dl) -> oaT (dl, q) for one q-tile."""
                sc = spool.tile([P, 2 * KG * P], f32, tag="sc", name="sc")
                oas = state["oas"].pop(qi)
                for dlb in range(NDT):
                    nc.tensor.transpose(
                        sc[:, dlb * P : (dlb + 1) * P],
                        oas[:, dlb * P : (dlb + 1) * P],
                        ident[:],
                    )
                for dlb in range(NDT):
                    nc.vector.tensor_copy(
                        oaT[dlb][:, qi * P : (qi + 1) * P], sc[:, dlb * P : (dlb + 1) * P]
                    )

            def load_wo():
                wop = s3.enter_context(tc.tile_pool(name="wop", bufs=1))
                wo = wop.tile([P, NDT * D], bf16, tag="wo", name="wo")
                nc.sync.dma_start(
                    out=wo[:].rearrange("p (i d) -> p i d", i=NDT),
                    in_=woT[:].rearrange("(i p) d -> p i d", p=P),
                )
                state["wo"] = wo

            def c_piece(st, fc):
                """Output projection for (s-tile st, f-chunk fc)."""
                wo = state["wo"]
                gp = gpool.tile([P, QC], f32, tag="gp", name="gp")
                for dl in range(NDT):
                    mm(gp[:], oaT[dl][:, st * P : (st + 1) * P],
                       wo[:, dl * D + fc * QC : dl * D + (fc + 1) * QC],
                       dl == 0, dl == NDT - 1)
                yv = yvp.tile([P, QC], f32, tag="yv", name="yv")
                nc.vector.tensor_copy(yv[:], gp[:])
                nc.sync.dma_start(
                    out=y[st * P : (st + 1) * P, fc * QC : (fc + 1) * QC], in_=yv[:]
                )

            # ------------- interleaved emission ---------------------------
            # One FIFO of side pieces per slot, drained between score+exp
            # groups under a PE-lead budget, force-drained at slot end (and at
            # the MID marker before the hp2/hp3 half). Estimated PE ns/piece.
            EXP_NS, SC_NS = 1040.0, 430.0
            COST = {}

            def piece_cost(p):
                fn = p[0]
                if fn == proj_piece:
                    return 1750.0
                if fn == vproj_piece:
                    return 450.0
                if fn == attnv_piece:
                    return 160.0
                if fn == transp_qt:
                    return 520.0
                if fn == c_piece:
                    return 900.0
                return 0.0

            def run_piece(p):
                if p[0] == "loadxk":
                    load_xk(p[1])
                elif p[0] == "loadxq":
                    load_xq(p[1], nc.sync)
                elif p[0] == "loadwo":
                    load_wo()
                elif p[0] == "closes1":
                    s1.close()
                elif p[0] == "openvx":
                    open_vx()
                elif p[0] == "loadwv":
                    load_wv()
                elif p[0] == "loadxv0":
                    load_xv0()
                elif p[0] == "closes2":
                    s2.close()
                elif p[0] == "HPM":
                    pass
                else:
                    p[0](*p[1:])

            def attnv_hp(qi, hp):
                out = []
                for h in (2 * hp, 2 * hp + 1):
                    for kg in range(NKG):
                        out.append((attnv_piece, qi, h, kg))
                return out

            slot_inter = [[] for _ in range(ST)]
            slot_markers = [set() for _ in range(ST)]
            slot_inter[0] += [("loadwv",), ("loadxv0",)]
            slot_inter[0] += [(vproj_piece, st, 0) for st in range(ST // 2)]
            slot_inter[1] += [(vproj_piece, st, 0) for st in range(ST // 2, ST)]
            slot_inter[1] += attnv_hp(0, 0)
            slot_inter[1] += [(vproj_piece, st, 1) for st in range(ST)]
            slot_inter[1] += attnv_hp(0, 1) + [(norm_piece, 0, 0)]
            slot_inter[2] += [(vproj_piece, st, 2) for st in range(ST)]
            slot_inter[2] += attnv_hp(0, 2) + [("HPM", 2)]
            slot_inter[2] += [(vproj_piece, st, 3) for st in range(ST)]
            slot_inter[2] += attnv_hp(0, 3) + [(norm_piece, 0, 1), ("HPM", 3)]
            slot_inter[2] += [("closes2",), ("loadwo",)]
            slot_markers[2] = {2, 3}
            slot_inter[3] += (attnv_hp(1, 0) + [("HPM", 0)]
                              + attnv_hp(1, 1) + [(norm_piece, 1, 0), ("HPM", 1)]
                              + attnv_hp(1, 2) + [("HPM", 2)]
                              + attnv_hp(1, 3) + [(norm_piece, 1, 1), ("HPM", 3)])
            slot_markers[3] = {0, 1, 2, 3}
            slot_inter[3] += [(transp_qt, 0), (transp_qt, 1)]
            for j in range(3, ST):
                slot_inter[j] += (attnv_hp(j - 1, 0) + attnv_hp(j - 1, 1)
                                  + [(norm_piece, j - 1, 0)]
                                  + attnv_hp(j - 1, 2) + attnv_hp(j - 1, 3)
                                  + [(norm_piece, j - 1, 1)])
                if j - 1 >= 2:
                    slot_inter[j].append((transp_qt, j - 1))
            # out-projection: early s-tiles deferred to late slots (the early
            # slots carry the V/K/Q overload), the rest two slots after their
            # transpose.
            for st in range(0, 5):
                slot_inter[11 + st] += [(c_piece, st, 0), (c_piece, st, 1)]
            for st in range(5, 14):
                slot_inter[st + 2] += [(c_piece, st, 0), (c_piece, st, 1)]
            # Q chunk c: DMA early, project each dl-tile just before the
            # first score group of slot 4c that needs it.
            slot_hp = [[[] for _ in range(HP)] for _ in range(ST)]
            for c in (1, 2, 3):
                slot_inter[4 * c - 2].append(("loadxq", c))
                for dch in range(NDT):
                    slot_hp[4 * c][dch].append(
                        (proj_piece, c, dch, wq, "xq", bq_t, qt))

            # ---------------- head + slot loop ----------------------------
            nc.scalar.dma_start(
                out=wk[:].rearrange("p (e d) -> p e d", e=ET),
                in_=wkT[:].rearrange("(e p) d -> p e d", p=P))
            load_xk(0)
            load_xq(0, nc.scalar)
            nc.sync.dma_start(
                out=wq[:].rearrange("p (e d) -> p e d", e=ET),
                in_=wqT[:].rearrange("(e p) d -> p e d", p=P))
            nc.sync.dma_start(out=bk_t[:].rearrange("p i -> p i ()"),
                              in_=bkd[:].rearrange("(i p) o -> p i o", p=P))
            nc.sync.dma_start(out=bq_t[:].rearrange("p i -> p i ()"),
                              in_=bqd[:].rearrange("(i p) o -> p i o", p=P))
            # PE clock warm-up while the first loads are in flight: harmless
            # zero matmuls keep the PE busy so it reaches full p-state before
            # the first projection.
            for wu in range(26):
                wsc = spool.tile([P, 2 * KG * P], f32, tag="sc", name="sc")
                for j in range(4):
                    mm(wsc[:, j * P : (j + 1) * P], warm[:], warm[:], True, True)

            budget = [0.0]

            def drain(inter, force=False, to_marker=None):
                while inter:
                    if (not force and to_marker is None
                            and piece_cost(inter[0]) > budget[0]):
                        return
                    p = inter.popleft()
                    run_piece(p)
                    budget[0] -= piece_cost(p)
                    if to_marker is not None and p == ("HPM", to_marker):
                        return

            def do_sc(qi, hp, kg, inter):
                sc_use(qi, hp, kg)
                budget[0] = min(budget[0] + (EXP_NS - SC_NS), 4000.0)
                drain(inter)

            # slot 0: interleave K/Q chunk-0 projections with the first score
            # groups (head pair hp becomes ready as soon as dl-tile hp
            # projects), then kg 1-3 as K chunks 1-3 land.
            inter0 = deque(slot_inter[0])
            for dch in range(NDT):
                proj_piece(0, dch, wk, "xk", bk_t, kt)
                proj_piece(0, dch, wq, "xq", bq_t, qt)
                sc_use(0, dch, 0)
            xk_next = {}
            load_xk(1)
            xk_next[1] = vstate["xk"]
            for kg in range(1, NKG):
                vstate["xk"] = xk_next[kg]
                if kg + 1 < NKG:
                    load_xk(kg + 1)
                    xk_next[kg + 1] = vstate["xk"]
                    vstate["xk"] = xk_next[kg]
                for dch in range(NDT):
                    proj_piece(kg, dch, wk, "xk", bk_t, kt)
                for hp in range(HP):
                    do_sc(0, hp, kg, inter0)
            drain(inter0, force=True)
            s1.close()
            open_vx()

            for qi in range(1, ST):
                inter = deque(slot_inter[qi])
                for hp in range(HP):
                    if hp in slot_markers[qi]:
                        drain(inter, to_marker=hp)
                    for p in slot_hp[qi][hp]:
                        run_piece(p)
                    for kg in range(NKG):
                        do_sc(qi, hp, kg, inter)
                drain(inter, force=True)

            # ---------------- tail ----------------------------------------
            for piece in (attnv_hp(ST - 1, 0) + attnv_hp(ST - 1, 1)
                          + [(norm_piece, ST - 1, 0)]
                          + attnv_hp(ST - 1, 2) + attnv_hp(ST - 1, 3)
                          + [(norm_piece, ST - 1, 1)]):
                run_piece(piece)
            transp_qt(ST - 1)
            for st in range(ST - 2, ST):
                for fc in range(D // QC):
                    c_piece(st, fc)
            s3.close()

    nc.compile()
    return nc


def make_in_maps(query, key, value, Wq, bq, Wk, bk, Wv, bv, n_cores=8,
                 mm_dtype="float32r"):
    """Host-side sharding: slice weights Megatron-style, transpose activations."""
    import ml_dtypes

    bft = ml_dtypes.bfloat16
    q = np.asarray(query, dtype=np.float32)
    k = np.asarray(key, dtype=np.float32)
    v = np.asarray(value, dtype=np.float32)
    Wq = np.asarray(Wq, dtype=np.float32)
    Wk = np.asarray(Wk, dtype=np.float32)
    Wv = np.asarray(Wv, dtype=np.float32)
    bq = np.asarray(bq, dtype=np.float32)
    bk = np.asarray(bk, dtype=np.float32)
    D = Wq.shape[0]
    DL = D // (n_cores // q.shape[0])
    scale = 1.0 / np.sqrt(np.float32(DK))
    in_maps = []
    for c in range(n_cores):
        b, g = divmod(c, n_cores // q.shape[0])
        sl = slice(DL * g, DL * (g + 1))
        in_maps.append(
            {
                "xqT": np.ascontiguousarray(q[b].T).astype(bft),
                "xkT": np.ascontiguousarray(k[b].T).astype(bft),
                "xvT": np.ascontiguousarray(v[b].T).astype(bft),
                "wqT": (np.ascontiguousarray(Wq[sl].T) * scale).astype(bft),
                "wkT": np.ascontiguousarray(Wk[sl].T).astype(bft),
                "wvT": np.ascontiguousarray(Wv[sl].T).astype(bft),
                "bq": np.ascontiguousarray((bq[sl] * scale).reshape(DL, 1)),
                "bk": np.ascontiguousarray(bk[sl].reshape(DL, 1)),
            }
        )
    return in_maps


def add_wo_maps(in_maps, Wo, n_cores=8, n_batch=4, mm_dtype="float32r"):
    import ml_dtypes

    Wo = np.asarray(Wo, dtype=np.float32)
    D = Wo.shape[0]
    DL = D // (n_cores // n_batch)
    for c in range(n_cores):
        _, g = divmod(c, n_cores // n_batch)
        sl = slice(DL * g, DL * (g + 1))
        in_maps[c]["woT"] = np.ascontiguousarray(Wo[:, sl].T).astype(ml_dtypes.bfloat16)
    return in_maps


MM_DTYPE = "float32r"


def kernel(query, key, value, Wq, bq, Wk, bk, Wv, bv, Wo, bo):
    if "nc" not in _CACHE:
        _CACHE["nc"] = build_nc(mm_dtype=MM_DTYPE)
    nc = _CACHE["nc"]
    n_cores = 8
    in_maps = make_in_maps(
        query, key, value, Wq, bq, Wk, bk, Wv, bv, n_cores, MM_DTYPE
    )
    add_wo_maps(in_maps, Wo, n_cores, np.asarray(query).shape[0], MM_DTYPE)
    res = run_bass_kernel_spmd(nc, in_maps, list(range(n_cores)))
    ys = [res.results[c]["y"] for c in range(n_cores)]
    bo = np.asarray(bo, dtype=np.float32)
    bv = np.asarray(bv, dtype=np.float32)
    Wo = np.asarray(Wo, dtype=np.float32)
    const = bo + bv @ Wo.T
    out = np.stack([ys[2 * b] + ys[2 * b + 1] for b in range(4)]) + const[None, None, :]
    return out.astype(np.float32)


# revision 55
# speedup vs baseline: 1.0055x; 1.0029x over previous
"""Trainium2 Bass kernel for nn_MultiHeadAttention_37838661877847.

Full-input contract: kernel(**inputs) takes the complete tensors and returns
the complete output. Internally shards across 8 NeuronCores:
  core c -> batch b = c // 2, head-group g = c % 2 (8 heads, 512 dims each).
Each core computes Q/K/V projections for its (batch, head-group) slice
(column-parallel weights), attention for its 8 heads, and a partial output
projection (row-parallel Wo). Host sums core pairs and adds bo + bv @ Wo.T
(the V bias commutes through softmax-weighted averaging, so it is folded
into the output-projection bias on the host).

Engine-level design (per core), built as ONE interleaved instruction stream
so the scalar engine's softmax-exp (the 266us serial floor: 33.5M exps at
1 elem/lane/cycle) overlaps the tensor engine work (281us):

  - Q_T/K_T stored (dl, s) in bf16; scores come out (k, q) per 128-k tile.
  - exp groups of [128, 1024] PSUM (4 score blocks: 2 heads x ... see sc
    layout below) -> ets tiles in bf16.
  - attn@V is FLIPPED: out (q, dk+1) accumulating over k with the exp tile
    as the stationary operand -> 65-row bf16 matmuls, half the PE rows of
    the (dk+1, q) orientation. V is augmented with a ones column per head so
    the softmax denominator Z lands in column 64; normalization is then a
    per-partition reciprocal+scale on DVE.
  - normalized output (q, dl) is transposed back to (dl, q) via PE-transpose
    through spare score-PSUM space, then the output projection streams wo.
  - emission interleaves projections / attn@V / transposes / out-proj between
    score+exp groups so the scalar engine rarely starves.

mm dtypes: x and w_qk in f32r/bf16 keep projections+scores accurate; the
attention path (probs, V, attn-out, Wo) runs in bf16 (PSUM accumulation is
fp32 throughout).
"""

import sys

sys.path.insert(0, "/opt/trn_rl_repo")

from collections import deque
from contextlib import ExitStack

import numpy as np

import concourse.bass as bass  # noqa: F401
import concourse.tile as tile
from concourse import bacc, masks, mybir
from concourse.bass_utils import run_bass_kernel_spmd

P = 128
DK = 64  # head dim

_CACHE = {}


def build_nc(S=2048, D=1024, DL=512, mm_dtype="float32r", n_cores=8,
             repeats=1, phases="ABC"):
    """Build + compile the per-core Bass program (same program on all cores).

    repeats exists only for timing experiments; production uses the default.
    mm_dtype/phases are accepted for test-harness compatibility (the kernel
    uses a fixed mixed f32r/bf16 precision scheme).
    """
    f32 = mybir.dt.float32
    f32r = mybir.dt.float32r
    bf16 = mybir.dt.bfloat16
    Exp = mybir.ActivationFunctionType.Exp

    H = DL // DK          # 8 local heads
    HP = H // 2           # 4 head pairs (one pair per 128-row q/k tile)
    ET = D // P           # 8 contraction tiles for projections
    ST = S // P           # 16 k tiles (and q tiles)
    NDT = DL // P         # 4 dl tiles
    QC = 512              # projection s-chunk
    NQ = S // QC          # 4
    KG = 4                # k-tiles per exp group
    NKG = ST // KG        # 4
    VW = H * (DK + 1)     # 520: v tile width incl. ones columns

    nc = bacc.Bacc("TRN2", target_bir_lowering=False, num_devices=n_cores)

    xqT = nc.dram_tensor("xqT", [D, S], bf16, kind="ExternalInput")
    xkT = nc.dram_tensor("xkT", [D, S], bf16, kind="ExternalInput")
    xvT = nc.dram_tensor("xvT", [D, S], bf16, kind="ExternalInput")
    wqT = nc.dram_tensor("wqT", [D, DL], bf16, kind="ExternalInput")
    wkT = nc.dram_tensor("wkT", [D, DL], bf16, kind="ExternalInput")
    wvT = nc.dram_tensor("wvT", [D, DL], bf16, kind="ExternalInput")
    woT = nc.dram_tensor("woT", [DL, D], bf16, kind="ExternalInput")
    bqd = nc.dram_tensor("bq", [DL, 1], f32, kind="ExternalInput")
    bkd = nc.dram_tensor("bk", [DL, 1], f32, kind="ExternalInput")
    y = nc.dram_tensor("y", [S, D], f32, kind="ExternalOutput")

    def mm(out, lhsT, rhs, start, stop):
        nc.tensor.matmul(out, lhsT=lhsT, rhs=rhs, start=start, stop=stop)

    with tile.TileContext(nc) as tc, ExitStack() as top:
        top.enter_context(
            nc.allow_low_precision(
                reason="attention path in bf16; PSUM accumulation stays fp32"
            )
        )
        persist = top.enter_context(tc.tile_pool(name="persist", bufs=1))
        qt = [persist.tile([P, S], bf16, tag=f"qt{i}", name=f"qt{i}") for i in range(NDT)]
        kt = [persist.tile([P, S], bf16, tag=f"kt{i}", name=f"kt{i}") for i in range(NDT)]
        vt = [persist.tile([P, VW], bf16, tag=f"vt{i}", name=f"vt{i}") for i in range(ST)]
        oaT = [persist.tile([P, S], bf16, tag=f"oaT{i}", name=f"oaT{i}") for i in range(NDT)]
        ident = persist.tile([P, P], f32, tag="ident", name="ident")
        bq_t = persist.tile([P, NDT], f32, tag="bq", name="bq")
        bk_t = persist.tile([P, NDT], f32, tag="bk", name="bk")

        masks.make_identity(nc, ident[:])
        warm = persist.tile([P, P], bf16, tag="warm", name="warm")
        nc.vector.memset(warm[:], 0.0)
        # vt ones-columns are memset inside slot 0 (below) so the head's
        # K/Q projection evacuations reach the DVE queue first.

        # PSUM: scores/exp 2x[128,1024] (4 banks) + attn@V accum 2x[128,260]
        # (2 banks) + generic matmul 2x[128,512] (2 banks) = 8 banks.
        spool = top.enter_context(tc.tile_pool(name="spool", bufs=2, space="PSUM"))
        acpool = top.enter_context(tc.tile_pool(name="acpool", bufs=2, space="PSUM"))
        gpool = top.enter_context(tc.tile_pool(name="gpool", bufs=2, space="PSUM"))

        # weight/x pools for Q (live through all Q chunks); wide layouts:
        # w tiles hold all ET contraction blocks side by side (one DMA each).
        wqp = top.enter_context(tc.tile_pool(name="wqp", bufs=1))
        wq = wqp.tile([P, ET * DL], bf16, tag="wq", name="wq")
        xqp = top.enter_context(tc.tile_pool(name="xqp", bufs=1))

        # long-lived attention pools (opened before any scoped pool so that
        # mid-stream pool closes stay LIFO)
        etsp = top.enter_context(tc.tile_pool(name="etsp", bufs=2))
        oasp = top.enter_context(tc.tile_pool(name="oasp", bufs=4))
        yvp = top.enter_context(tc.tile_pool(name="yvp", bufs=2))
        rcp = top.enter_context(tc.tile_pool(name="rcp", bufs=4))

        for _rep in range(repeats):
            # ---------------- pools for K and Q chunk streams -------------
            vstate = {}
            s3 = ExitStack()
            s2 = ExitStack()
            vxa = s2.enter_context(tc.tile_pool(name="vxa", bufs=1))
            vstate["wv"] = vxa.tile([P, ET * DL], bf16, tag="wv", name="wv")
            vstate["xv0"] = vxa.tile([P, ET * (S // 2)], bf16, tag="xv0",
                                     name="xv0")
            s1 = ExitStack()
            kx = s1.enter_context(tc.tile_pool(name="kx", bufs=2))
            wkp = s1.enter_context(tc.tile_pool(name="wkp", bufs=1))
            wk = wkp.tile([P, ET * DL], bf16, tag="wk", name="wk")

            def load_xk(c, eng=None):
                xkc = kx.tile([P, ET * QC], bf16, tag="xk", name="xk")
                (eng or nc.sync).dma_start(
                    out=xkc[:].rearrange("p (e s) -> p e s", e=ET),
                    in_=xkT[:, c * QC : (c + 1) * QC].rearrange(
                        "(e p) s -> p e s", p=P),
                )
                vstate["xk"] = xkc

            def load_xq(c, eng):
                xqc = xqp.tile([P, ET * QC], bf16, tag="xq", name="xq")
                eng.dma_start(
                    out=xqc[:].rearrange("p (e s) -> p e s", e=ET),
                    in_=xqT[:, c * QC : (c + 1) * QC].rearrange(
                        "(e p) s -> p e s", p=P),
                )
                vstate["xq"] = xqc

            def proj_piece(c, dch, w, xkey, bias, out_tiles):
                """One (chunk, dl-tile) projection: out (dl 128, s 512) + bias."""
                x = vstate[xkey]
                gp = gpool.tile([P, QC], f32, tag="gp", name="gp")
                for e in range(ET):
                    mm(gp[:], w[:, e * DL + dch * P : e * DL + (dch + 1) * P],
                       x[:, e * QC : (e + 1) * QC], e == 0, e == ET - 1)
                nc.vector.tensor_scalar_add(
                    out_tiles[dch][:, c * QC : (c + 1) * QC], gp[:],
                    bias[:, dch : dch + 1]
                )

            # --- V pools: wv + the first s-half of xv preload alongside the
            # K pool (slot 0); the second s-half lands in the space the K pool
            # frees. V projection runs head-half-major so attn@V for heads 0-3
            # unblocks as early as possible.
            SH = S // 2

            def load_wv():
                nc.sync.dma_start(
                    out=vstate["wv"][:].rearrange("p (e d) -> p e d", e=ET),
                    in_=wvT[:].rearrange("(e p) d -> p e d", p=P),
                )

            def load_xv0():
                nc.sync.dma_start(
                    out=vstate["xv0"][:].rearrange("p (e s) -> p e s", e=ET),
                    in_=xvT[:, 0:SH].rearrange("(e p) s -> p e s", p=P),
                )

            def open_vx():
                vxb = s2.enter_context(tc.tile_pool(name="vxb", bufs=1))
                vstate["xv1"] = vxb.tile([P, ET * SH], bf16, tag="xv1", name="xv1")
                nc.sync.dma_start(
                    out=vstate["xv1"][:].rearrange("p (e s) -> p e s", e=ET),
                    in_=xvT[:, SH:S].rearrange("(e p) s -> p e s", p=P),
                )

            def vproj_piece(st, qtr):
                """V projection for (s-tile st, head pair qtr): 2 heads."""
                Q4 = DL // 4
                gp = gpool.tile([P, QC], f32, tag="gp", name="gp")
                wv = vstate["wv"]
                xv = vstate["xv0"] if st < ST // 2 else vstate["xv1"]
                stl = st % (ST // 2)
                for e in range(ET):
                    mm(gp[:, 0:Q4],
                       xv[:, e * SH + stl * P : e * SH + (stl + 1) * P],
                       wv[:, e * DL + qtr * Q4 : e * DL + (qtr + 1) * Q4],
                       e == 0, e == ET - 1)
                nc.vector.tensor_copy(
                    vt[st][:].rearrange("p (h c) -> p h c", h=H)
                    [:, qtr * 2 : (qtr + 1) * 2, 0:DK],
                    gp[:, 0:Q4].rearrange("p (h c) -> p h c", h=2),
                )

            state = {
                "ets": {},    # (qt_idx, hp, kg) -> tile  (live window)
                "ac": {},     # (qt_idx, hgrp) -> tile
                "oas": {},    # qt_idx -> tile
                "wo": None,
            }

            def sc_use(qi, hp, kg):
                """Scores + exp for (q-tile qi, head pair hp, k-group kg)."""
                sc = spool.tile([P, 2 * KG * P], f32, tag="sc", name="sc")
                for hloc in range(2):
                    h = 2 * hp + hloc
                    r0 = hloc * DK
                    for ktl in range(KG):
                        ki = kg * KG + ktl
                        mm(
                            sc[:, hloc * KG * P + ktl * P : hloc * KG * P + (ktl + 1) * P],
                            kt[hp][r0 : r0 + DK, ki * P : (ki + 1) * P],
                            qt[hp][r0 : r0 + DK, qi * P : (qi + 1) * P],
                            True,
                            True,
                        )
                et = etsp.tile([P, 2 * KG * P], bf16, tag=f"et{hp}_{kg}",
                               name=f"et{hp}_{kg}")
                nc.scalar.activation(et[:], sc[:], Exp)
                state["ets"][(qi, hp, kg)] = et

            def attnv_piece(qi, h, kg):
                """attn@V for (q-tile qi, head h, k-group kg): 4 x 65-row mms."""
                hgrp, hidx = divmod(h, 4)
                key = (qi, hgrp)
                if key not in state["ac"]:
                    # padded to a full 2KB bank; cols 0-259 used (4 heads x 65)
                    state["ac"][key] = acpool.tile([P, 512], f32, tag="ac", name="ac")
                ac = state["ac"][key]
                et = state["ets"][(qi, h // 2, kg)]
                hloc = h % 2
                for ktl in range(KG):
                    ki = kg * KG + ktl
                    mm(
                        ac[:, hidx * (DK + 1) : (hidx + 1) * (DK + 1)],
                        et[:, hloc * KG * P + ktl * P : hloc * KG * P + (ktl + 1) * P],
                        vt[ki][:, h * (DK + 1) : (h + 1) * (DK + 1)],
                        ki == 0,
                        ki == ST - 1,
                    )
                if hloc == 1:
                    del state["ets"][(qi, h // 2, kg)]

            def norm_piece(qi, hgrp):
                """Normalize 4 heads: oa_s[:, hgrp*256:+256] = num * (1/Z)."""
                if qi not in state["oas"]:
                    state["oas"][qi] = oasp.tile([P, DL], f32, tag="oas", name="oas")
                oas = state["oas"][qi]
                ac = state["ac"].pop((qi, hgrp))
                acr = ac[:, 0 : 4 * (DK + 1)].rearrange("p (h c) -> p h c", h=4)
                rc = rcp.tile([P, 4], f32, tag="rc", name="rc")
                nc.vector.reciprocal(rc[:], acr[:, :, DK])
                for hh in range(4):
                    nc.vector.tensor_scalar_mul(
                        oas[:, hgrp * 4 * DK + hh * DK : hgrp * 4 * DK + (hh + 1) * DK],
                        acr[:, hh, 0:DK],
                        rc[:, hh : hh + 1],
                    )

            def transp_qt(qi):
                """Transpose oa_s (q, dl) -> oaT (dl, q) for one q-tile."""
                sc = spool.tile([P, 2 * KG * P], f32, tag="sc", name="sc")
                oas = state["oas"].pop(qi)
                for dlb in range(NDT):
                    nc.tensor.transpose(
                        sc[:, dlb * P : (dlb + 1) * P],
                        oas[:, dlb * P : (dlb + 1) * P],
                        ident[:],
                    )
                for dlb in range(NDT):
                    nc.vector.tensor_copy(
                        oaT[dlb][:, qi * P : (qi + 1) * P], sc[:, dlb * P : (dlb + 1) * P]
                    )

            def load_wo():
                wop = s3.enter_context(tc.tile_pool(name="wop", bufs=1))
                wo = wop.tile([P, NDT * D], bf16, tag="wo", name="wo")
                nc.sync.dma_start(
                    out=wo[:].rearrange("p (i d) -> p i d", i=NDT),
                    in_=woT[:].rearrange("(i p) d -> p i d", p=P),
                )
                state["wo"] = wo

            def c_piece(st, fc):
                """Output projection for (s-tile st, f-chunk fc)."""
                wo = state["wo"]
                gp = gpool.tile([P, QC], f32, tag="gp", name="gp")
                for dl in range(NDT):
                    mm(gp[:], oaT[dl][:, st * P : (st + 1) * P],
                       wo[:, dl * D + fc * QC : dl * D + (fc + 1) * QC],
                       dl == 0, dl == NDT - 1)
                yv = yvp.tile([P, QC], f32, tag="yv", name="yv")
                nc.vector.tensor_copy(yv[:], gp[:])
                nc.sync.dma_start(
                    out=y[st * P : (st + 1) * P, fc * QC : (fc + 1) * QC], in_=yv[:]
                )

            # ------------- interleaved emission ---------------------------
            # One FIFO of side pieces per slot, drained between score+exp
            # groups under a PE-lead budget, force-drained at slot end (and at
            # the MID marker before the hp2/hp3 half). Estimated PE ns/piece.
            EXP_NS, SC_NS = 1090.0, 430.0
            COST = {}

            def piece_cost(p):
                fn = p[0]
                if fn == proj_piece:
                    return 1750.0
                if fn == vproj_piece:
                    return 450.0
                if fn == attnv_piece:
                    return 160.0
                if fn == transp_qt:
                    return 520.0
                if fn == c_piece:
                    return 900.0
                return 0.0

            def run_piece(p):
                if p[0] == "loadxk":
                    load_xk(p[1])
                elif p[0] == "loadxq":
                    load_xq(p[1], nc.sync)
                elif p[0] == "loadwo":
                    load_wo()
                elif p[0] == "closes1":
                    s1.close()
                elif p[0] == "openvx":
                    open_vx()
                elif p[0] == "memset":
                    nc.vector.memset(vt[p[1]][:], 1.0)
                elif p[0] == "loadwv":
                    load_wv()
                elif p[0] == "loadxv0":
                    load_xv0()
                elif p[0] == "closes2":
                    s2.close()
                elif p[0] == "HPM":
                    pass
                else:
                    p[0](*p[1:])

            def attnv_hp(qi, hp):
                out = []
                for h in (2 * hp, 2 * hp + 1):
                    for kg in range(NKG):
                        out.append((attnv_piece, qi, h, kg))
                return out

            slot_inter = [[] for _ in range(ST)]
            slot_markers = [set() for _ in range(ST)]
            slot_inter[0] += [("memset", i) for i in range(ST)]
            slot_inter[0] += [("loadwv",), ("loadxv0",)]
            slot_inter[0] += [(vproj_piece, st, 0) for st in range(ST // 2)]
            slot_inter[1] += [(vproj_piece, st, 0) for st in range(ST // 2, ST)]
            slot_inter[1] += attnv_hp(0, 0)
            slot_inter[1] += [(vproj_piece, st, 1) for st in range(ST)]
            slot_inter[1] += attnv_hp(0, 1) + [(norm_piece, 0, 0)]
            slot_inter[2] += [(vproj_piece, st, 2) for st in range(ST)]
            slot_inter[2] += attnv_hp(0, 2) + [("HPM", 2)]
            slot_inter[2] += [(vproj_piece, st, 3) for st in range(ST)]
            slot_inter[2] += attnv_hp(0, 3) + [(norm_piece, 0, 1), ("HPM", 3)]
            slot_inter[2] += [("closes2",), ("loadwo",)]
            slot_markers[2] = {2, 3}
            slot_inter[3] += (attnv_hp(1, 0) + [("HPM", 0)]
                              + attnv_hp(1, 1) + [(norm_piece, 1, 0), ("HPM", 1)]
                              + attnv_hp(1, 2) + [("HPM", 2)]
                              + attnv_hp(1, 3) + [(norm_piece, 1, 1), ("HPM", 3)])
            slot_markers[3] = {0, 1, 2, 3}
            slot_inter[3] += [(transp_qt, 0), (transp_qt, 1)]
            for j in range(3, ST):
                slot_inter[j] += (attnv_hp(j - 1, 0) + attnv_hp(j - 1, 1)
                                  + [(norm_piece, j - 1, 0)]
                                  + attnv_hp(j - 1, 2) + attnv_hp(j - 1, 3)
                                  + [(norm_piece, j - 1, 1)])
                if j - 1 >= 2:
                    slot_inter[j].append((transp_qt, j - 1))
            # out-projection: early s-tiles deferred to late slots (the early
            # slots carry the V/K/Q overload), the rest two slots after their
            # transpose.
            for st in range(0, 5):
                slot_inter[11 + st] += [(c_piece, st, 0), (c_piece, st, 1)]
            for st in range(5, 14):
                slot_inter[st + 2] += [(c_piece, st, 0), (c_piece, st, 1)]
            # Q chunk c: DMA early, project each dl-tile just before the
            # first score group of slot 4c that needs it.
            slot_hp = [[[] for _ in range(HP)] for _ in range(ST)]
            for c in (1, 2, 3):
                slot_inter[4 * c - 2].append(("loadxq", c))
                for dch in range(NDT):
                    slot_hp[4 * c][dch].append(
                        (proj_piece, c, dch, wq, "xq", bq_t, qt))

            # ---------------- head + slot loop ----------------------------
            nc.scalar.dma_start(
                out=wk[:].rearrange("p (e d) -> p e d", e=ET),
                in_=wkT[:].rearrange("(e p) d -> p e d", p=P))
            load_xk(0)
            load_xq(0, nc.scalar)
            nc.sync.dma_start(
                out=wq[:].rearrange("p (e d) -> p e d", e=ET),
                in_=wqT[:].rearrange("(e p) d -> p e d", p=P))
            nc.sync.dma_start(out=bk_t[:].rearrange("p i -> p i ()"),
                              in_=bkd[:].rearrange("(i p) o -> p i o", p=P))
            nc.sync.dma_start(out=bq_t[:].rearrange("p i -> p i ()"),
                              in_=bqd[:].rearrange("(i p) o -> p i o", p=P))
            # PE clock warm-up while the first loads are in flight: harmless
            # zero matmuls keep the PE busy so it reaches full p-state before
            # the first projection.
            for wu in range(26):
                wsc = spool.tile([P, 2 * KG * P], f32, tag="sc", name="sc")
                for j in range(4):
                    mm(wsc[:, j * P : (j + 1) * P], warm[:], warm[:], True, True)

            budget = [0.0]

            def drain(inter, force=False, to_marker=None):
                while inter:
                    if (not force and to_marker is None
                            and piece_cost(inter[0]) > budget[0]):
                        return
                    p = inter.popleft()
                    run_piece(p)
                    budget[0] -= piece_cost(p)
                    if to_marker is not None and p == ("HPM", to_marker):
                        return

            def do_sc(qi, hp, kg, inter):
                sc_use(qi, hp, kg)
                budget[0] = min(budget[0] + (EXP_NS - SC_NS), 4000.0)
                drain(inter)

            # slot 0: interleave K/Q chunk-0 projections with the first score
            # groups (head pair hp becomes ready as soon as dl-tile hp
            # projects), then kg 1-3 as K chunks 1-3 land.
            inter0 = deque(slot_inter[0])
            for dch in range(NDT):
                proj_piece(0, dch, wk, "xk", bk_t, kt)
                proj_piece(0, dch, wq, "xq", bq_t, qt)
                sc_use(0, dch, 0)
            xk_next = {}
            load_xk(1)
            xk_next[1] = vstate["xk"]
            for kg in range(1, NKG):
                vstate["xk"] = xk_next[kg]
                if kg + 1 < NKG:
                    load_xk(kg + 1)
                    xk_next[kg + 1] = vstate["xk"]
                    vstate["xk"] = xk_next[kg]
                for dch in range(NDT):
                    proj_piece(kg, dch, wk, "xk", bk_t, kt)
                for hp in range(HP):
                    do_sc(0, hp, kg, inter0)
            drain(inter0, force=True)
            s1.close()
            open_vx()

            for qi in range(1, ST):
                inter = deque(slot_inter[qi])
                for hp in range(HP):
                    if hp in slot_markers[qi]:
                        drain(inter, to_marker=hp)
                    for p in slot_hp[qi][hp]:
                        run_piece(p)
                    for kg in range(NKG):
                        if hp == HP - 1 and kg == NKG - 1:
                            # flush leftovers while the previous exp groups
                            # still cover the scalar engine
                            drain(inter, force=True)
                        do_sc(qi, hp, kg, inter)
                drain(inter, force=True)

            # ---------------- tail ----------------------------------------
            for piece in (attnv_hp(ST - 1, 0) + attnv_hp(ST - 1, 1)
                          + [(norm_piece, ST - 1, 0)]
                          + attnv_hp(ST - 1, 2) + attnv_hp(ST - 1, 3)
                          + [(norm_piece, ST - 1, 1)]):
                run_piece(piece)
            transp_qt(ST - 1)
            for st in range(ST - 2, ST):
                for fc in range(D // QC):
                    c_piece(st, fc)
            s3.close()

    nc.compile()
    return nc


def make_in_maps(query, key, value, Wq, bq, Wk, bk, Wv, bv, n_cores=8,
                 mm_dtype="float32r"):
    """Host-side sharding: slice weights Megatron-style, transpose activations."""
    import ml_dtypes

    bft = ml_dtypes.bfloat16
    q = np.asarray(query, dtype=np.float32)
    k = np.asarray(key, dtype=np.float32)
    v = np.asarray(value, dtype=np.float32)
    Wq = np.asarray(Wq, dtype=np.float32)
    Wk = np.asarray(Wk, dtype=np.float32)
    Wv = np.asarray(Wv, dtype=np.float32)
    bq = np.asarray(bq, dtype=np.float32)
    bk = np.asarray(bk, dtype=np.float32)
    D = Wq.shape[0]
    DL = D // (n_cores // q.shape[0])
    scale = 1.0 / np.sqrt(np.float32(DK))
    in_maps = []
    for c in range(n_cores):
        b, g = divmod(c, n_cores // q.shape[0])
        sl = slice(DL * g, DL * (g + 1))
        in_maps.append(
            {
                "xqT": np.ascontiguousarray(q[b].T).astype(bft),
                "xkT": np.ascontiguousarray(k[b].T).astype(bft),
                "xvT": np.ascontiguousarray(v[b].T).astype(bft),
                "wqT": (np.ascontiguousarray(Wq[sl].T) * scale).astype(bft),
                "wkT": np.ascontiguousarray(Wk[sl].T).astype(bft),
                "wvT": np.ascontiguousarray(Wv[sl].T).astype(bft),
                "bq": np.ascontiguousarray((bq[sl] * scale).reshape(DL, 1)),
                "bk": np.ascontiguousarray(bk[sl].reshape(DL, 1)),
            }
        )
    return in_maps


def add_wo_maps(in_maps, Wo, n_cores=8, n_batch=4, mm_dtype="float32r"):
    import ml_dtypes

    Wo = np.asarray(Wo, dtype=np.float32)
    D = Wo.shape[0]
    DL = D // (n_cores // n_batch)
    for c in range(n_cores):
        _, g = divmod(c, n_cores // n_batch)
        sl = slice(DL * g, DL * (g + 1))
        in_maps[c]["woT"] = np.ascontiguousarray(Wo[:, sl].T).astype(ml_dtypes.bfloat16)
    return in_maps


MM_DTYPE = "float32r"


def kernel(query, key, value, Wq, bq, Wk, bk, Wv, bv, Wo, bo):
    if "nc" not in _CACHE:
        _CACHE["nc"] = build_nc(mm_dtype=MM_DTYPE)
    nc = _CACHE["nc"]
    n_cores = 8
    in_maps = make_in_maps(
        query, key, value, Wq, bq, Wk, bk, Wv, bv, n_cores, MM_DTYPE
    )
    add_wo_maps(in_maps, Wo, n_cores, np.asarray(query).shape[0], MM_DTYPE)
    res = run_bass_kernel_spmd(nc, in_maps, list(range(n_cores)))
    ys = [res.results[c]["y"] for c in range(n_cores)]
    bo = np.asarray(bo, dtype=np.float32)
    bv = np.asarray(bv, dtype=np.float32)
    Wo = np.asarray(Wo, dtype=np.float32)
    const = bo + bv @ Wo.T
    out = np.stack([ys[2 * b] + ys[2 * b + 1] for b in range(4)]) + const[None, None, :]
    return out.astype(np.float32)


# revision 63
# speedup vs baseline: 1.0066x; 1.0010x over previous
"""Trainium2 Bass kernel for nn_MultiHeadAttention_37838661877847.

Full-input contract: kernel(**inputs) takes the complete tensors and returns
the complete output. Internally shards across 8 NeuronCores:
  core c -> batch b = c // 2, head-group g = c % 2 (8 heads, 512 dims each).
Each core computes Q/K/V projections for its (batch, head-group) slice
(column-parallel weights), attention for its 8 heads, and a partial output
projection (row-parallel Wo). Host sums core pairs and adds bo + bv @ Wo.T
(the V bias commutes through softmax-weighted averaging, so it is folded
into the output-projection bias on the host).

Engine-level design (per core), built as ONE interleaved instruction stream
so the scalar engine's softmax-exp (the 266us serial floor: 33.5M exps at
1 elem/lane/cycle) overlaps the tensor engine work (281us):

  - Q_T/K_T stored (dl, s) in bf16; scores come out (k, q) per 128-k tile.
  - exp groups of [128, 1024] PSUM (4 score blocks: 2 heads x ... see sc
    layout below) -> ets tiles in bf16.
  - attn@V is FLIPPED: out (q, dk+1) accumulating over k with the exp tile
    as the stationary operand -> 65-row bf16 matmuls, half the PE rows of
    the (dk+1, q) orientation. V is augmented with a ones column per head so
    the softmax denominator Z lands in column 64; normalization is then a
    per-partition reciprocal+scale on DVE.
  - normalized output (q, dl) is transposed back to (dl, q) via PE-transpose
    through spare score-PSUM space, then the output projection streams wo.
  - emission interleaves projections / attn@V / transposes / out-proj between
    score+exp groups so the scalar engine rarely starves.

mm dtypes: x and w_qk in f32r/bf16 keep projections+scores accurate; the
attention path (probs, V, attn-out, Wo) runs in bf16 (PSUM accumulation is
fp32 throughout).
"""

import sys

sys.path.insert(0, "/opt/trn_rl_repo")

from collections import deque
from contextlib import ExitStack

import numpy as np

import concourse.bass as bass  # noqa: F401
import concourse.tile as tile
from concourse import bacc, masks, mybir
from concourse.bass_utils import run_bass_kernel_spmd

P = 128
DK = 64  # head dim

_CACHE = {}


def build_nc(S=2048, D=1024, DL=512, mm_dtype="float32r", n_cores=8,
             repeats=1, phases="ABC"):
    """Build + compile the per-core Bass program (same program on all cores).

    repeats exists only for timing experiments; production uses the default.
    mm_dtype/phases are accepted for test-harness compatibility (the kernel
    uses a fixed mixed f32r/bf16 precision scheme).
    """
    f32 = mybir.dt.float32
    f32r = mybir.dt.float32r
    bf16 = mybir.dt.bfloat16
    Exp = mybir.ActivationFunctionType.Exp

    H = DL // DK          # 8 local heads
    HP = H // 2           # 4 head pairs (one pair per 128-row q/k tile)
    ET = D // P           # 8 contraction tiles for projections
    ST = S // P           # 16 k tiles (and q tiles)
    NDT = DL // P         # 4 dl tiles
    QC = 512              # projection s-chunk
    NQ = S // QC          # 4
    KG = 4                # k-tiles per exp group
    NKG = ST // KG        # 4
    VW = H * (DK + 1)     # 520: v tile width incl. ones columns

    nc = bacc.Bacc("TRN2", target_bir_lowering=False, num_devices=n_cores)

    xqT = nc.dram_tensor("xqT", [D, S], bf16, kind="ExternalInput")
    xkT = nc.dram_tensor("xkT", [D, S], bf16, kind="ExternalInput")
    xvT = nc.dram_tensor("xvT", [D, S], bf16, kind="ExternalInput")
    wqT = nc.dram_tensor("wqT", [D, DL], bf16, kind="ExternalInput")
    wkT = nc.dram_tensor("wkT", [D, DL], bf16, kind="ExternalInput")
    wvT = nc.dram_tensor("wvT", [D, DL], bf16, kind="ExternalInput")
    woT = nc.dram_tensor("woT", [DL, D], bf16, kind="ExternalInput")
    bqd = nc.dram_tensor("bq", [DL, 1], f32, kind="ExternalInput")
    bkd = nc.dram_tensor("bk", [DL, 1], f32, kind="ExternalInput")
    y = nc.dram_tensor("y", [S, D], f32, kind="ExternalOutput")

    def mm(out, lhsT, rhs, start, stop):
        nc.tensor.matmul(out, lhsT=lhsT, rhs=rhs, start=start, stop=stop)

    with tile.TileContext(nc) as tc, ExitStack() as top:
        top.enter_context(
            nc.allow_low_precision(
                reason="attention path in bf16; PSUM accumulation stays fp32"
            )
        )
        persist = top.enter_context(tc.tile_pool(name="persist", bufs=1))
        qt = [persist.tile([P, S], bf16, tag=f"qt{i}", name=f"qt{i}") for i in range(NDT)]
        kt = [persist.tile([P, S], bf16, tag=f"kt{i}", name=f"kt{i}") for i in range(NDT)]
        vt = [persist.tile([P, VW], bf16, tag=f"vt{i}", name=f"vt{i}") for i in range(ST)]
        oaT = [persist.tile([P, S], bf16, tag=f"oaT{i}", name=f"oaT{i}") for i in range(NDT)]
        ident = persist.tile([P, P], f32, tag="ident", name="ident")
        bq_t = persist.tile([P, NDT], f32, tag="bq", name="bq")
        bk_t = persist.tile([P, NDT], f32, tag="bk", name="bk")

        masks.make_identity(nc, ident[:])
        warm = persist.tile([P, P], bf16, tag="warm", name="warm")
        nc.vector.memset(warm[:], 0.0)
        # vt ones-columns are memset inside slot 0 (below) so the head's
        # K/Q projection evacuations reach the DVE queue first.

        # PSUM: scores/exp 2x[128,1024] (4 banks) + attn@V accum 2x[128,260]
        # (2 banks) + generic matmul 2x[128,512] (2 banks) = 8 banks.
        spool = top.enter_context(tc.tile_pool(name="spool", bufs=2, space="PSUM"))
        acpool = top.enter_context(tc.tile_pool(name="acpool", bufs=2, space="PSUM"))
        gpool = top.enter_context(tc.tile_pool(name="gpool", bufs=2, space="PSUM"))

        # weight/x pools for Q (live through all Q chunks); wide layouts:
        # w tiles hold all ET contraction blocks side by side (one DMA each).
        wqp = top.enter_context(tc.tile_pool(name="wqp", bufs=1))
        wq = wqp.tile([P, ET * DL], bf16, tag="wq", name="wq")
        xqp = top.enter_context(tc.tile_pool(name="xqp", bufs=1))

        # long-lived attention pools (opened before any scoped pool so that
        # mid-stream pool closes stay LIFO)
        etsp = top.enter_context(tc.tile_pool(name="etsp", bufs=2))
        oasp = top.enter_context(tc.tile_pool(name="oasp", bufs=4))
        yvp = top.enter_context(tc.tile_pool(name="yvp", bufs=2))
        rcp = top.enter_context(tc.tile_pool(name="rcp", bufs=4))

        for _rep in range(repeats):
            # ---------------- pools for K and Q chunk streams -------------
            vstate = {}
            s3 = ExitStack()
            s2 = ExitStack()
            vxa = s2.enter_context(tc.tile_pool(name="vxa", bufs=1))
            vstate["wv"] = vxa.tile([P, ET * DL], bf16, tag="wv", name="wv")
            vstate["xv0"] = vxa.tile([P, ET * (S // 2)], bf16, tag="xv0",
                                     name="xv0")
            s1 = ExitStack()
            kx = s1.enter_context(tc.tile_pool(name="kx", bufs=2))
            wkp = s1.enter_context(tc.tile_pool(name="wkp", bufs=1))
            wk = wkp.tile([P, ET * DL], bf16, tag="wk", name="wk")

            def load_xk(c, eng=None):
                xkc = kx.tile([P, ET * QC], bf16, tag="xk", name="xk")
                (eng or nc.sync).dma_start(
                    out=xkc[:].rearrange("p (e s) -> p e s", e=ET),
                    in_=xkT[:, c * QC : (c + 1) * QC].rearrange(
                        "(e p) s -> p e s", p=P),
                )
                vstate["xk"] = xkc

            def load_xq(c, eng):
                xqc = xqp.tile([P, ET * QC], bf16, tag="xq", name="xq")
                eng.dma_start(
                    out=xqc[:].rearrange("p (e s) -> p e s", e=ET),
                    in_=xqT[:, c * QC : (c + 1) * QC].rearrange(
                        "(e p) s -> p e s", p=P),
                )
                vstate["xq"] = xqc

            def proj_piece(c, dch, w, xkey, bias, out_tiles):
                """One (chunk, dl-tile) projection: out (dl 128, s 512) + bias."""
                x = vstate[xkey]
                gp = gpool.tile([P, QC], f32, tag="gp", name="gp")
                for e in range(ET):
                    mm(gp[:], w[:, e * DL + dch * P : e * DL + (dch + 1) * P],
                       x[:, e * QC : (e + 1) * QC], e == 0, e == ET - 1)
                nc.vector.tensor_scalar_add(
                    out_tiles[dch][:, c * QC : (c + 1) * QC], gp[:],
                    bias[:, dch : dch + 1]
                )

            # --- V pools: wv + the first s-half of xv preload alongside the
            # K pool (slot 0); the second s-half lands in the space the K pool
            # frees. V projection runs head-half-major so attn@V for heads 0-3
            # unblocks as early as possible.
            SH = S // 2

            def load_wv():
                nc.sync.dma_start(
                    out=vstate["wv"][:].rearrange("p (e d) -> p e d", e=ET),
                    in_=wvT[:].rearrange("(e p) d -> p e d", p=P),
                )

            def load_xv0():
                nc.sync.dma_start(
                    out=vstate["xv0"][:].rearrange("p (e s) -> p e s", e=ET),
                    in_=xvT[:, 0:SH].rearrange("(e p) s -> p e s", p=P),
                )

            def open_vx():
                vxb = s2.enter_context(tc.tile_pool(name="vxb", bufs=1))
                vstate["xv1"] = vxb.tile([P, ET * SH], bf16, tag="xv1", name="xv1")
                nc.sync.dma_start(
                    out=vstate["xv1"][:].rearrange("p (e s) -> p e s", e=ET),
                    in_=xvT[:, SH:S].rearrange("(e p) s -> p e s", p=P),
                )

            def vproj_piece(st, qtr):
                """V projection for (s-tile st, head pair qtr): 2 heads."""
                Q4 = DL // 4
                gp = gpool.tile([P, QC], f32, tag="gp", name="gp")
                wv = vstate["wv"]
                xv = vstate["xv0"] if st < ST // 2 else vstate["xv1"]
                stl = st % (ST // 2)
                for e in range(ET):
                    mm(gp[:, 0:Q4],
                       xv[:, e * SH + stl * P : e * SH + (stl + 1) * P],
                       wv[:, e * DL + qtr * Q4 : e * DL + (qtr + 1) * Q4],
                       e == 0, e == ET - 1)
                nc.vector.tensor_copy(
                    vt[st][:].rearrange("p (h c) -> p h c", h=H)
                    [:, qtr * 2 : (qtr + 1) * 2, 0:DK],
                    gp[:, 0:Q4].rearrange("p (h c) -> p h c", h=2),
                )

            state = {
                "ets": {},    # (qt_idx, hp, kg) -> tile  (live window)
                "ac": {},     # (qt_idx, hgrp) -> tile
                "oas": {},    # qt_idx -> tile
                "wo": None,
            }

            def sc_use(qi, hp, kg):
                """Scores + exp for (q-tile qi, head pair hp, k-group kg)."""
                sc = spool.tile([P, 2 * KG * P], f32, tag="sc", name="sc")
                for hloc in range(2):
                    h = 2 * hp + hloc
                    r0 = hloc * DK
                    for ktl in range(KG):
                        ki = kg * KG + ktl
                        mm(
                            sc[:, hloc * KG * P + ktl * P : hloc * KG * P + (ktl + 1) * P],
                            kt[hp][r0 : r0 + DK, ki * P : (ki + 1) * P],
                            qt[hp][r0 : r0 + DK, qi * P : (qi + 1) * P],
                            True,
                            True,
                        )
                et = etsp.tile([P, 2 * KG * P], bf16, tag=f"et{hp}_{kg}",
                               name=f"et{hp}_{kg}")
                nc.scalar.activation(et[:], sc[:], Exp)
                state["ets"][(qi, hp, kg)] = et

            def attnv_piece(qi, h, kg):
                """attn@V for (q-tile qi, head h, k-group kg): 4 x 65-row mms."""
                hgrp, hidx = divmod(h, 4)
                key = (qi, hgrp)
                if key not in state["ac"]:
                    # padded to a full 2KB bank; cols 0-259 used (4 heads x 65)
                    state["ac"][key] = acpool.tile([P, 512], f32, tag="ac", name="ac")
                ac = state["ac"][key]
                et = state["ets"][(qi, h // 2, kg)]
                hloc = h % 2
                for ktl in range(KG):
                    ki = kg * KG + ktl
                    mm(
                        ac[:, hidx * (DK + 1) : (hidx + 1) * (DK + 1)],
                        et[:, hloc * KG * P + ktl * P : hloc * KG * P + (ktl + 1) * P],
                        vt[ki][:, h * (DK + 1) : (h + 1) * (DK + 1)],
                        ki == 0,
                        ki == ST - 1,
                    )
                if hloc == 1:
                    del state["ets"][(qi, h // 2, kg)]

            def norm_piece(qi, hgrp):
                """Normalize 4 heads: oa_s[:, hgrp*256:+256] = num * (1/Z)."""
                if qi not in state["oas"]:
                    state["oas"][qi] = oasp.tile([P, DL], f32, tag="oas", name="oas")
                oas = state["oas"][qi]
                ac = state["ac"].pop((qi, hgrp))
                acr = ac[:, 0 : 4 * (DK + 1)].rearrange("p (h c) -> p h c", h=4)
                rc = rcp.tile([P, 4], f32, tag="rc", name="rc")
                nc.vector.reciprocal(rc[:], acr[:, :, DK])
                for hh in range(4):
                    nc.vector.tensor_scalar_mul(
                        oas[:, hgrp * 4 * DK + hh * DK : hgrp * 4 * DK + (hh + 1) * DK],
                        acr[:, hh, 0:DK],
                        rc[:, hh : hh + 1],
                    )

            def transp_qt(qi):
                """Transpose oa_s (q, dl) -> oaT (dl, q) for one q-tile."""
                sc = spool.tile([P, 2 * KG * P], f32, tag="sc", name="sc")
                oas = state["oas"].pop(qi)
                for dlb in range(NDT):
                    nc.tensor.transpose(
                        sc[:, dlb * P : (dlb + 1) * P],
                        oas[:, dlb * P : (dlb + 1) * P],
                        ident[:],
                    )
                for dlb in range(NDT):
                    nc.vector.tensor_copy(
                        oaT[dlb][:, qi * P : (qi + 1) * P], sc[:, dlb * P : (dlb + 1) * P]
                    )

            def load_wo():
                wop = s3.enter_context(tc.tile_pool(name="wop", bufs=1))
                wo = wop.tile([P, NDT * D], bf16, tag="wo", name="wo")
                nc.sync.dma_start(
                    out=wo[:].rearrange("p (i d) -> p i d", i=NDT),
                    in_=woT[:].rearrange("(i p) d -> p i d", p=P),
                )
                state["wo"] = wo

            def c_piece(st, fc):
                """Output projection for (s-tile st, f-chunk fc)."""
                wo = state["wo"]
                gp = gpool.tile([P, QC], f32, tag="gp", name="gp")
                for dl in range(NDT):
                    mm(gp[:], oaT[dl][:, st * P : (st + 1) * P],
                       wo[:, dl * D + fc * QC : dl * D + (fc + 1) * QC],
                       dl == 0, dl == NDT - 1)
                yv = yvp.tile([P, QC], f32, tag="yv", name="yv")
                nc.vector.tensor_copy(yv[:], gp[:])
                nc.sync.dma_start(
                    out=y[st * P : (st + 1) * P, fc * QC : (fc + 1) * QC], in_=yv[:]
                )

            # ------------- interleaved emission ---------------------------
            # One FIFO of side pieces per slot, drained between score+exp
            # groups under a PE-lead budget, force-drained at slot end (and at
            # the MID marker before the hp2/hp3 half). Estimated PE ns/piece.
            EXP_NS, SC_NS = 1090.0, 430.0
            COST = {}

            def piece_cost(p):
                fn = p[0]
                if fn == proj_piece:
                    return 1750.0
                if fn == qproj_rest:
                    return 1350.0
                if fn == vproj_piece:
                    return 450.0
                if fn == attnv_piece:
                    return 160.0
                if fn == transp_qt:
                    return 520.0
                if fn == c_piece:
                    return 900.0
                return 0.0

            def run_piece(p):
                if p[0] == "loadxk":
                    load_xk(p[1])
                elif p[0] == "loadxq":
                    load_xq(p[1], nc.sync)
                elif p[0] == "loadwo":
                    load_wo()
                elif p[0] == "closes1":
                    s1.close()
                elif p[0] == "openvx":
                    open_vx()
                elif p[0] == "memset":
                    nc.vector.memset(vt[p[1]][:], 1.0)
                elif p[0] == "loadwv":
                    load_wv()
                elif p[0] == "loadxv0":
                    load_xv0()
                elif p[0] == "closes2":
                    s2.close()
                elif p[0] == "HPM":
                    pass
                else:
                    p[0](*p[1:])

            def attnv_hp(qi, hp):
                out = []
                for h in (2 * hp, 2 * hp + 1):
                    for kg in range(NKG):
                        out.append((attnv_piece, qi, h, kg))
                return out

            slot_inter = [[] for _ in range(ST)]
            slot_markers = [set() for _ in range(ST)]
            slot_inter[0] += [("memset", i) for i in range(ST)]
            slot_inter[0] += [("loadwv",), ("loadxv0",)]
            slot_inter[0] += [(vproj_piece, st, 0) for st in range(ST // 2)]
            slot_inter[1] += [(vproj_piece, st, 0) for st in range(ST // 2, ST)]
            slot_inter[1] += attnv_hp(0, 0)
            slot_inter[1] += [(vproj_piece, st, 1) for st in range(ST)]
            slot_inter[1] += attnv_hp(0, 1) + [(norm_piece, 0, 0)]
            slot_inter[2] += [(vproj_piece, st, 2) for st in range(ST)]
            slot_inter[2] += attnv_hp(0, 2) + [("HPM", 2)]
            slot_inter[2] += [(vproj_piece, st, 3) for st in range(ST)]
            slot_inter[2] += attnv_hp(0, 3) + [(norm_piece, 0, 1), ("HPM", 3)]
            slot_inter[2] += [("closes2",), ("loadwo",)]
            slot_markers[2] = {2, 3}
            slot_inter[3] += (attnv_hp(1, 0) + [("HPM", 0)]
                              + attnv_hp(1, 1) + [(norm_piece, 1, 0), ("HPM", 1)]
                              + attnv_hp(1, 2) + [("HPM", 2)]
                              + attnv_hp(1, 3) + [(norm_piece, 1, 1), ("HPM", 3)])
            slot_markers[3] = {0, 1, 2, 3}
            slot_inter[3] += [(transp_qt, 0), (transp_qt, 1)]
            for j in range(3, ST):
                slot_inter[j] += (attnv_hp(j - 1, 0) + attnv_hp(j - 1, 1)
                                  + [(norm_piece, j - 1, 0)]
                                  + attnv_hp(j - 1, 2) + attnv_hp(j - 1, 3)
                                  + [(norm_piece, j - 1, 1)])
                if j - 1 >= 2:
                    slot_inter[j].append((transp_qt, j - 1))
            # out-projection: early s-tiles deferred to late slots (the early
            # slots carry the V/K/Q overload), the rest two slots after their
            # transpose.
            for st in range(0, 5):
                slot_inter[11 + st] += [(c_piece, st, 0), (c_piece, st, 1)]
            for st in range(5, 14):
                slot_inter[st + 2] += [(c_piece, st, 0), (c_piece, st, 1)]
            # Q chunk c: DMA early, project each dl-tile just before the
            # first score group of slot 4c that needs it.
            def qproj_mini(c, dch):
                """Q projection for q-tile 4c only (slot 4c's own columns)."""
                x = vstate["xq"]
                gp = gpool.tile([P, QC], f32, tag="gp", name="gp")
                for e in range(ET):
                    mm(gp[:, 0:P],
                       wq[:, e * DL + dch * P : e * DL + (dch + 1) * P],
                       x[:, e * QC : e * QC + P], e == 0, e == ET - 1)
                nc.vector.tensor_scalar_add(
                    qt[dch][:, 4 * c * P : (4 * c + 1) * P], gp[:, 0:P],
                    bq_t[:, dch : dch + 1])

            def qproj_rest(c, dch):
                """Q projection for q-tiles 4c+1..4c+3 (needed next slot)."""
                x = vstate["xq"]
                gp = gpool.tile([P, QC], f32, tag="gp", name="gp")
                for e in range(ET):
                    mm(gp[:, 0 : 3 * P],
                       wq[:, e * DL + dch * P : e * DL + (dch + 1) * P],
                       x[:, e * QC + P : (e + 1) * QC], e == 0, e == ET - 1)
                nc.vector.tensor_scalar_add(
                    qt[dch][:, (4 * c + 1) * P : (4 * c + 4) * P],
                    gp[:, 0 : 3 * P], bq_t[:, dch : dch + 1])

            slot_hp = [[[] for _ in range(HP)] for _ in range(ST)]
            for c in (1, 2, 3):
                slot_inter[4 * c - 2].append(("loadxq", c))
                for dch in range(NDT):
                    slot_hp[4 * c][dch].append((qproj_mini, c, dch))
                slot_inter[4 * c] += [(qproj_rest, c, dch)
                                      for dch in range(NDT)]

            # ---------------- head + slot loop ----------------------------
            nc.scalar.dma_start(
                out=wk[:].rearrange("p (e d) -> p e d", e=ET),
                in_=wkT[:].rearrange("(e p) d -> p e d", p=P))
            load_xk(0)
            load_xq(0, nc.scalar)
            nc.sync.dma_start(
                out=wq[:].rearrange("p (e d) -> p e d", e=ET),
                in_=wqT[:].rearrange("(e p) d -> p e d", p=P))
            nc.sync.dma_start(out=bk_t[:].rearrange("p i -> p i ()"),
                              in_=bkd[:].rearrange("(i p) o -> p i o", p=P))
            nc.sync.dma_start(out=bq_t[:].rearrange("p i -> p i ()"),
                              in_=bqd[:].rearrange("(i p) o -> p i o", p=P))
            # PE clock warm-up while the first loads are in flight: harmless
            # zero matmuls keep the PE busy so it reaches full p-state before
            # the first projection.
            for wu in range(26):
                wsc = spool.tile([P, 2 * KG * P], f32, tag="sc", name="sc")
                for j in range(4):
                    mm(wsc[:, j * P : (j + 1) * P], warm[:], warm[:], True, True)

            budget = [0.0]

            def drain(inter, force=False, to_marker=None):
                while inter:
                    if (not force and to_marker is None
                            and piece_cost(inter[0]) > budget[0]):
                        return
                    p = inter.popleft()
                    run_piece(p)
                    budget[0] -= piece_cost(p)
                    if to_marker is not None and p == ("HPM", to_marker):
                        return

            def do_sc(qi, hp, kg, inter):
                sc_use(qi, hp, kg)
                budget[0] = min(budget[0] + (EXP_NS - SC_NS), 4000.0)
                drain(inter)

            # slot 0: interleave K/Q chunk-0 projections with the first score
            # groups (head pair hp becomes ready as soon as dl-tile hp
            # projects), then kg 1-3 as K chunks 1-3 land.
            inter0 = deque(slot_inter[0])
            for dch in range(NDT):
                proj_piece(0, dch, wk, "xk", bk_t, kt)
                proj_piece(0, dch, wq, "xq", bq_t, qt)
                sc_use(0, dch, 0)
            xk_next = {}
            load_xk(1)
            xk_next[1] = vstate["xk"]
            for kg in range(1, NKG):
                vstate["xk"] = xk_next[kg]
                if kg + 1 < NKG:
                    load_xk(kg + 1)
                    xk_next[kg + 1] = vstate["xk"]
                    vstate["xk"] = xk_next[kg]
                for dch in range(NDT):
                    proj_piece(kg, dch, wk, "xk", bk_t, kt)
                for hp in range(HP):
                    do_sc(0, hp, kg, inter0)
            drain(inter0, force=True)
            s1.close()
            open_vx()

            for qi in range(1, ST):
                inter = deque(slot_inter[qi])
                for hp in range(HP):
                    if hp in slot_markers[qi]:
                        drain(inter, to_marker=hp)
                    for p in slot_hp[qi][hp]:
                        run_piece(p)
                    for kg in range(NKG):
                        if hp == HP - 2 and kg == 0:
                            # flush leftovers while the previous exp groups
                            # still cover the scalar engine
                            drain(inter, force=True)
                        do_sc(qi, hp, kg, inter)
                drain(inter, force=True)

            # ---------------- tail ----------------------------------------
            for piece in (attnv_hp(ST - 1, 0) + attnv_hp(ST - 1, 1)
                          + [(norm_piece, ST - 1, 0)]
                          + attnv_hp(ST - 1, 2) + attnv_hp(ST - 1, 3)
                          + [(norm_piece, ST - 1, 1)]):
                run_piece(piece)
            transp_qt(ST - 1)
            for st in range(ST - 2, ST):
                for fc in range(D // QC):
                    c_piece(st, fc)
            s3.close()

    nc.compile()
    return nc


def make_in_maps(query, key, value, Wq, bq, Wk, bk, Wv, bv, n_cores=8,
                 mm_dtype="float32r"):
    """Host-side sharding: slice weights Megatron-style, transpose activations."""
    import ml_dtypes

    bft = ml_dtypes.bfloat16
    q = np.asarray(query, dtype=np.float32)
    k = np.asarray(key, dtype=np.float32)
    v = np.asarray(value, dtype=np.float32)
    Wq = np.asarray(Wq, dtype=np.float32)
    Wk = np.asarray(Wk, dtype=np.float32)
    Wv = np.asarray(Wv, dtype=np.float32)
    bq = np.asarray(bq, dtype=np.float32)
    bk = np.asarray(bk, dtype=np.float32)
    D = Wq.shape[0]
    DL = D // (n_cores // q.shape[0])
    scale = 1.0 / np.sqrt(np.float32(DK))
    in_maps = []
    for c in range(n_cores):
        b, g = divmod(c, n_cores // q.shape[0])
        sl = slice(DL * g, DL * (g + 1))
        in_maps.append(
            {
                "xqT": np.ascontiguousarray(q[b].T).astype(bft),
                "xkT": np.ascontiguousarray(k[b].T).astype(bft),
                "xvT": np.ascontiguousarray(v[b].T).astype(bft),
                "wqT": (np.ascontiguousarray(Wq[sl].T) * scale).astype(bft),
                "wkT": np.ascontiguousarray(Wk[sl].T).astype(bft),
                "wvT": np.ascontiguousarray(Wv[sl].T).astype(bft),
                "bq": np.ascontiguousarray((bq[sl] * scale).reshape(DL, 1)),
                "bk": np.ascontiguousarray(bk[sl].reshape(DL, 1)),
            }
        )
    return in_maps


def add_wo_maps(in_maps, Wo, n_cores=8, n_batch=4, mm_dtype="float32r"):
    import ml_dtypes

    Wo = np.asarray(Wo, dtype=np.float32)
    D = Wo.shape[0]
    DL = D // (n_cores // n_batch)
    for c in range(n_cores):
        _, g = divmod(c, n_cores // n_batch)
        sl = slice(DL * g, DL * (g + 1))
        in_maps[c]["woT"] = np.ascontiguousarray(Wo[:, sl].T).astype(ml_dtypes.bfloat16)
    return in_maps


MM_DTYPE = "float32r"


def kernel(query, key, value, Wq, bq, Wk, bk, Wv, bv, Wo, bo):
    if "nc" not in _CACHE:
        _CACHE["nc"] = build_nc(mm_dtype=MM_DTYPE)
    nc = _CACHE["nc"]
    n_cores = 8
    in_maps = make_in_maps(
        query, key, value, Wq, bq, Wk, bk, Wv, bv, n_cores, MM_DTYPE
    )
    add_wo_maps(in_maps, Wo, n_cores, np.asarray(query).shape[0], MM_DTYPE)
    res = run_bass_kernel_spmd(nc, in_maps, list(range(n_cores)))
    ys = [res.results[c]["y"] for c in range(n_cores)]
    bo = np.asarray(bo, dtype=np.float32)
    bv = np.asarray(bv, dtype=np.float32)
    Wo = np.asarray(Wo, dtype=np.float32)
    const = bo + bv @ Wo.T
    out = np.stack([ys[2 * b] + ys[2 * b + 1] for b in range(4)]) + const[None, None, :]
    return out.astype(np.float32)


# revision 64
# speedup vs baseline: 1.0085x; 1.0020x over previous
"""Trainium2 Bass kernel for nn_MultiHeadAttention_37838661877847.

Full-input contract: kernel(**inputs) takes the complete tensors and returns
the complete output. Internally shards across 8 NeuronCores:
  core c -> batch b = c // 2, head-group g = c % 2 (8 heads, 512 dims each).
Each core computes Q/K/V projections for its (batch, head-group) slice
(column-parallel weights), attention for its 8 heads, and a partial output
projection (row-parallel Wo). Host sums core pairs and adds bo + bv @ Wo.T
(the V bias commutes through softmax-weighted averaging, so it is folded
into the output-projection bias on the host).

Engine-level design (per core), built as ONE interleaved instruction stream
so the scalar engine's softmax-exp (the 266us serial floor: 33.5M exps at
1 elem/lane/cycle) overlaps the tensor engine work (281us):

  - Q_T/K_T stored (dl, s) in bf16; scores come out (k, q) per 128-k tile.
  - exp groups of [128, 1024] PSUM (4 score blocks: 2 heads x ... see sc
    layout below) -> ets tiles in bf16.
  - attn@V is FLIPPED: out (q, dk+1) accumulating over k with the exp tile
    as the stationary operand -> 65-row bf16 matmuls, half the PE rows of
    the (dk+1, q) orientation. V is augmented with a ones column per head so
    the softmax denominator Z lands in column 64; normalization is then a
    per-partition reciprocal+scale on DVE.
  - normalized output (q, dl) is transposed back to (dl, q) via PE-transpose
    through spare score-PSUM space, then the output projection streams wo.
  - emission interleaves projections / attn@V / transposes / out-proj between
    score+exp groups so the scalar engine rarely starves.

mm dtypes: x and w_qk in f32r/bf16 keep projections+scores accurate; the
attention path (probs, V, attn-out, Wo) runs in bf16 (PSUM accumulation is
fp32 throughout).
"""

import sys

sys.path.insert(0, "/opt/trn_rl_repo")

from collections import deque
from contextlib import ExitStack

import numpy as np

import concourse.bass as bass  # noqa: F401
import concourse.tile as tile
from concourse import bacc, masks, mybir
from concourse.bass_utils import run_bass_kernel_spmd

P = 128
DK = 64  # head dim

_CACHE = {}


def build_nc(S=2048, D=1024, DL=512, mm_dtype="float32r", n_cores=8,
             repeats=1, phases="ABC"):
    """Build + compile the per-core Bass program (same program on all cores).

    repeats exists only for timing experiments; production uses the default.
    mm_dtype/phases are accepted for test-harness compatibility (the kernel
    uses a fixed mixed f32r/bf16 precision scheme).
    """
    f32 = mybir.dt.float32
    f32r = mybir.dt.float32r
    bf16 = mybir.dt.bfloat16
    Exp = mybir.ActivationFunctionType.Exp

    H = DL // DK          # 8 local heads
    HP = H // 2           # 4 head pairs (one pair per 128-row q/k tile)
    ET = D // P           # 8 contraction tiles for projections
    ST = S // P           # 16 k tiles (and q tiles)
    NDT = DL // P         # 4 dl tiles
    QC = 512              # projection s-chunk
    NQ = S // QC          # 4
    KG = 4                # k-tiles per exp group
    NKG = ST // KG        # 4
    VW = H * (DK + 1)     # 520: v tile width incl. ones columns

    nc = bacc.Bacc("TRN2", target_bir_lowering=False, num_devices=n_cores)

    xqT = nc.dram_tensor("xqT", [D, S], bf16, kind="ExternalInput")
    xkT = nc.dram_tensor("xkT", [D, S], bf16, kind="ExternalInput")
    xvT = nc.dram_tensor("xvT", [D, S], bf16, kind="ExternalInput")
    wqT = nc.dram_tensor("wqT", [D, DL], bf16, kind="ExternalInput")
    wkT = nc.dram_tensor("wkT", [D, DL], bf16, kind="ExternalInput")
    wvT = nc.dram_tensor("wvT", [D, DL], bf16, kind="ExternalInput")
    woT = nc.dram_tensor("woT", [DL, D], bf16, kind="ExternalInput")
    bqd = nc.dram_tensor("bq", [DL, 1], f32, kind="ExternalInput")
    bkd = nc.dram_tensor("bk", [DL, 1], f32, kind="ExternalInput")
    y = nc.dram_tensor("y", [S, D], f32, kind="ExternalOutput")

    def mm(out, lhsT, rhs, start, stop):
        nc.tensor.matmul(out, lhsT=lhsT, rhs=rhs, start=start, stop=stop)

    with tile.TileContext(nc) as tc, ExitStack() as top:
        top.enter_context(
            nc.allow_low_precision(
                reason="attention path in bf16; PSUM accumulation stays fp32"
            )
        )
        persist = top.enter_context(tc.tile_pool(name="persist", bufs=1))
        qt = [persist.tile([P, S], bf16, tag=f"qt{i}", name=f"qt{i}") for i in range(NDT)]
        kt = [persist.tile([P, S], bf16, tag=f"kt{i}", name=f"kt{i}") for i in range(NDT)]
        vt = [persist.tile([P, VW], bf16, tag=f"vt{i}", name=f"vt{i}") for i in range(ST)]
        oaT = [persist.tile([P, S], bf16, tag=f"oaT{i}", name=f"oaT{i}") for i in range(NDT)]
        ident = persist.tile([P, P], f32, tag="ident", name="ident")
        bq_t = persist.tile([P, NDT], f32, tag="bq", name="bq")
        bk_t = persist.tile([P, NDT], f32, tag="bk", name="bk")

        masks.make_identity(nc, ident[:])
        warm = persist.tile([P, P], bf16, tag="warm", name="warm")
        nc.vector.memset(warm[:], 0.0)
        # vt ones-columns are memset inside slot 0 (below) so the head's
        # K/Q projection evacuations reach the DVE queue first.

        # PSUM: scores/exp 2x[128,1024] (4 banks) + attn@V accum 2x[128,260]
        # (2 banks) + generic matmul 2x[128,512] (2 banks) = 8 banks.
        spool = top.enter_context(tc.tile_pool(name="spool", bufs=2, space="PSUM"))
        acpool = top.enter_context(tc.tile_pool(name="acpool", bufs=2, space="PSUM"))
        gpool = top.enter_context(tc.tile_pool(name="gpool", bufs=2, space="PSUM"))

        # weight/x pools for Q (live through all Q chunks); wide layouts:
        # w tiles hold all ET contraction blocks side by side (one DMA each).
        wqp = top.enter_context(tc.tile_pool(name="wqp", bufs=1))
        wq = wqp.tile([P, ET * DL], bf16, tag="wq", name="wq")
        xqp = top.enter_context(tc.tile_pool(name="xqp", bufs=1))

        # long-lived attention pools (opened before any scoped pool so that
        # mid-stream pool closes stay LIFO)
        etsp = top.enter_context(tc.tile_pool(name="etsp", bufs=2))
        oasp = top.enter_context(tc.tile_pool(name="oasp", bufs=4))
        yvp = top.enter_context(tc.tile_pool(name="yvp", bufs=2))
        rcp = top.enter_context(tc.tile_pool(name="rcp", bufs=4))

        for _rep in range(repeats):
            # ---------------- pools for K and Q chunk streams -------------
            vstate = {}
            s3 = ExitStack()
            s2 = ExitStack()
            vxa = s2.enter_context(tc.tile_pool(name="vxa", bufs=1))
            vstate["wv"] = vxa.tile([P, ET * DL], bf16, tag="wv", name="wv")
            vstate["xv0"] = vxa.tile([P, ET * (S // 2)], bf16, tag="xv0",
                                     name="xv0")
            s1 = ExitStack()
            kx = s1.enter_context(tc.tile_pool(name="kx", bufs=2))
            wkp = s1.enter_context(tc.tile_pool(name="wkp", bufs=1))
            wk = wkp.tile([P, ET * DL], bf16, tag="wk", name="wk")

            def load_xk(c, eng=None):
                xkc = kx.tile([P, ET * QC], bf16, tag="xk", name="xk")
                (eng or nc.sync).dma_start(
                    out=xkc[:].rearrange("p (e s) -> p e s", e=ET),
                    in_=xkT[:, c * QC : (c + 1) * QC].rearrange(
                        "(e p) s -> p e s", p=P),
                )
                vstate["xk"] = xkc

            def load_xq(c, eng):
                xqc = xqp.tile([P, ET * QC], bf16, tag="xq", name="xq")
                eng.dma_start(
                    out=xqc[:].rearrange("p (e s) -> p e s", e=ET),
                    in_=xqT[:, c * QC : (c + 1) * QC].rearrange(
                        "(e p) s -> p e s", p=P),
                )
                vstate["xq"] = xqc

            def proj_piece(c, dch, w, xkey, bias, out_tiles):
                """One (chunk, dl-tile) projection: out (dl 128, s 512) + bias."""
                x = vstate[xkey]
                gp = gpool.tile([P, QC], f32, tag="gp", name="gp")
                for e in range(ET):
                    mm(gp[:], w[:, e * DL + dch * P : e * DL + (dch + 1) * P],
                       x[:, e * QC : (e + 1) * QC], e == 0, e == ET - 1)
                nc.vector.tensor_scalar_add(
                    out_tiles[dch][:, c * QC : (c + 1) * QC], gp[:],
                    bias[:, dch : dch + 1]
                )

            # --- V pools: wv + the first s-half of xv preload alongside the
            # K pool (slot 0); the second s-half lands in the space the K pool
            # frees. V projection runs head-half-major so attn@V for heads 0-3
            # unblocks as early as possible.
            SH = S // 2

            def load_wv():
                nc.sync.dma_start(
                    out=vstate["wv"][:].rearrange("p (e d) -> p e d", e=ET),
                    in_=wvT[:].rearrange("(e p) d -> p e d", p=P),
                )

            def load_xv0():
                nc.sync.dma_start(
                    out=vstate["xv0"][:].rearrange("p (e s) -> p e s", e=ET),
                    in_=xvT[:, 0:SH].rearrange("(e p) s -> p e s", p=P),
                )

            def open_vx():
                vxb = s2.enter_context(tc.tile_pool(name="vxb", bufs=1))
                vstate["xv1"] = vxb.tile([P, ET * SH], bf16, tag="xv1", name="xv1")
                nc.sync.dma_start(
                    out=vstate["xv1"][:].rearrange("p (e s) -> p e s", e=ET),
                    in_=xvT[:, SH:S].rearrange("(e p) s -> p e s", p=P),
                )

            def vproj_piece(st, qtr):
                """V projection for (s-tile st, head pair qtr): 2 heads."""
                Q4 = DL // 4
                gp = gpool.tile([P, QC], f32, tag="gp", name="gp")
                wv = vstate["wv"]
                xv = vstate["xv0"] if st < ST // 2 else vstate["xv1"]
                stl = st % (ST // 2)
                for e in range(ET):
                    mm(gp[:, 0:Q4],
                       xv[:, e * SH + stl * P : e * SH + (stl + 1) * P],
                       wv[:, e * DL + qtr * Q4 : e * DL + (qtr + 1) * Q4],
                       e == 0, e == ET - 1)
                nc.vector.tensor_copy(
                    vt[st][:].rearrange("p (h c) -> p h c", h=H)
                    [:, qtr * 2 : (qtr + 1) * 2, 0:DK],
                    gp[:, 0:Q4].rearrange("p (h c) -> p h c", h=2),
                )

            state = {
                "ets": {},    # (qt_idx, hp, kg) -> tile  (live window)
                "ac": {},     # (qt_idx, hgrp) -> tile
                "oas": {},    # qt_idx -> tile
                "wo": None,
            }

            def sc_use(qi, hp, kg):
                """Scores + exp for (q-tile qi, head pair hp, k-group kg)."""
                sc = spool.tile([P, 2 * KG * P], f32, tag="sc", name="sc")
                for hloc in range(2):
                    h = 2 * hp + hloc
                    r0 = hloc * DK
                    for ktl in range(KG):
                        ki = kg * KG + ktl
                        mm(
                            sc[:, hloc * KG * P + ktl * P : hloc * KG * P + (ktl + 1) * P],
                            kt[hp][r0 : r0 + DK, ki * P : (ki + 1) * P],
                            qt[hp][r0 : r0 + DK, qi * P : (qi + 1) * P],
                            True,
                            True,
                        )
                et = etsp.tile([P, 2 * KG * P], bf16, tag=f"et{hp}_{kg}",
                               name=f"et{hp}_{kg}")
                nc.scalar.activation(et[:], sc[:], Exp)
                state["ets"][(qi, hp, kg)] = et

            def attnv_piece(qi, h, kg):
                """attn@V for (q-tile qi, head h, k-group kg): 4 x 65-row mms."""
                hgrp, hidx = divmod(h, 4)
                key = (qi, hgrp)
                if key not in state["ac"]:
                    # padded to a full 2KB bank; cols 0-259 used (4 heads x 65)
                    state["ac"][key] = acpool.tile([P, 512], f32, tag="ac", name="ac")
                ac = state["ac"][key]
                et = state["ets"][(qi, h // 2, kg)]
                hloc = h % 2
                for ktl in range(KG):
                    ki = kg * KG + ktl
                    mm(
                        ac[:, hidx * (DK + 1) : (hidx + 1) * (DK + 1)],
                        et[:, hloc * KG * P + ktl * P : hloc * KG * P + (ktl + 1) * P],
                        vt[ki][:, h * (DK + 1) : (h + 1) * (DK + 1)],
                        ki == 0,
                        ki == ST - 1,
                    )
                if hloc == 1:
                    del state["ets"][(qi, h // 2, kg)]

            def norm_piece(qi, hgrp):
                """Normalize 4 heads: oa_s[:, hgrp*256:+256] = num * (1/Z)."""
                if qi not in state["oas"]:
                    state["oas"][qi] = oasp.tile([P, DL], f32, tag="oas", name="oas")
                oas = state["oas"][qi]
                ac = state["ac"].pop((qi, hgrp))
                acr = ac[:, 0 : 4 * (DK + 1)].rearrange("p (h c) -> p h c", h=4)
                rc = rcp.tile([P, 4], f32, tag="rc", name="rc")
                nc.vector.reciprocal(rc[:], acr[:, :, DK])
                for hh in range(4):
                    nc.vector.tensor_scalar_mul(
                        oas[:, hgrp * 4 * DK + hh * DK : hgrp * 4 * DK + (hh + 1) * DK],
                        acr[:, hh, 0:DK],
                        rc[:, hh : hh + 1],
                    )

            def transp_qt(qi):
                """Transpose oa_s (q, dl) -> oaT (dl, q) for one q-tile."""
                sc = spool.tile([P, 2 * KG * P], f32, tag="sc", name="sc")
                oas = state["oas"].pop(qi)
                for dlb in range(NDT):
                    nc.tensor.transpose(
                        sc[:, dlb * P : (dlb + 1) * P],
                        oas[:, dlb * P : (dlb + 1) * P],
                        ident[:],
                    )
                for dlb in range(NDT):
                    nc.vector.tensor_copy(
                        oaT[dlb][:, qi * P : (qi + 1) * P], sc[:, dlb * P : (dlb + 1) * P]
                    )

            def load_wo():
                wop = s3.enter_context(tc.tile_pool(name="wop", bufs=1))
                wo = wop.tile([P, NDT * D], bf16, tag="wo", name="wo")
                nc.sync.dma_start(
                    out=wo[:].rearrange("p (i d) -> p i d", i=NDT),
                    in_=woT[:].rearrange("(i p) d -> p i d", p=P),
                )
                state["wo"] = wo

            def c_piece(st, fc):
                """Output projection for (s-tile st, f-chunk fc)."""
                wo = state["wo"]
                gp = gpool.tile([P, QC], f32, tag="gp", name="gp")
                for dl in range(NDT):
                    mm(gp[:], oaT[dl][:, st * P : (st + 1) * P],
                       wo[:, dl * D + fc * QC : dl * D + (fc + 1) * QC],
                       dl == 0, dl == NDT - 1)
                yv = yvp.tile([P, QC], f32, tag="yv", name="yv")
                nc.vector.tensor_copy(yv[:], gp[:])
                nc.sync.dma_start(
                    out=y[st * P : (st + 1) * P, fc * QC : (fc + 1) * QC], in_=yv[:]
                )

            # ------------- interleaved emission ---------------------------
            # One FIFO of side pieces per slot, drained between score+exp
            # groups under a PE-lead budget, force-drained at slot end (and at
            # the MID marker before the hp2/hp3 half). Estimated PE ns/piece.
            EXP_NS, SC_NS = 1090.0, 430.0
            COST = {}

            def piece_cost(p):
                fn = p[0]
                if fn == proj_piece:
                    return 1750.0
                if fn == qproj_rest:
                    return 1350.0
                if fn == vproj_piece:
                    return 450.0
                if fn == attnv_piece:
                    return 160.0
                if fn == transp_qt:
                    return 520.0
                if fn == c_piece:
                    return 900.0
                return 0.0

            def run_piece(p):
                if p[0] == "loadxk":
                    load_xk(p[1])
                elif p[0] == "loadxq":
                    load_xq(p[1], nc.sync)
                elif p[0] == "loadwo":
                    load_wo()
                elif p[0] == "closes1":
                    s1.close()
                elif p[0] == "openvx":
                    open_vx()
                elif p[0] == "memset":
                    nc.vector.memset(vt[p[1]][:], 1.0)
                elif p[0] == "loadwv":
                    load_wv()
                elif p[0] == "loadxv0":
                    load_xv0()
                elif p[0] == "closes2":
                    s2.close()
                elif p[0] == "HPM":
                    pass
                else:
                    p[0](*p[1:])

            def attnv_hp(qi, hp):
                out = []
                for h in (2 * hp, 2 * hp + 1):
                    for kg in range(NKG):
                        out.append((attnv_piece, qi, h, kg))
                return out

            slot_inter = [[] for _ in range(ST)]
            slot_markers = [set() for _ in range(ST)]
            slot_inter[0] += [("memset", i) for i in range(ST)]
            slot_inter[0] += [("loadwv",), ("loadxv0",)]
            slot_inter[0] += [(vproj_piece, st, 0) for st in range(ST // 2)]
            slot_inter[1] += [(vproj_piece, st, 0) for st in range(ST // 2, ST)]
            slot_inter[1] += attnv_hp(0, 0)
            slot_inter[1] += [(vproj_piece, st, 1) for st in range(ST)]
            slot_inter[1] += attnv_hp(0, 1) + [(norm_piece, 0, 0)]
            slot_inter[2] += [(vproj_piece, st, 2) for st in range(ST)]
            slot_inter[2] += attnv_hp(0, 2) + [("HPM", 2)]
            slot_inter[2] += [(vproj_piece, st, 3) for st in range(ST)]
            slot_inter[2] += attnv_hp(0, 3) + [(norm_piece, 0, 1), ("HPM", 3)]
            slot_inter[2] += [("closes2",), ("loadwo",)]
            slot_markers[2] = {2, 3}
            slot_inter[3] += (attnv_hp(1, 0) + [("HPM", 0)]
                              + attnv_hp(1, 1) + [(norm_piece, 1, 0), ("HPM", 1)]
                              + attnv_hp(1, 2) + [("HPM", 2)]
                              + attnv_hp(1, 3) + [(norm_piece, 1, 1), ("HPM", 3)])
            slot_markers[3] = {0, 1, 2, 3}
            slot_inter[3] += [(transp_qt, 0), (transp_qt, 1)]
            for j in range(3, ST):
                slot_inter[j] += (attnv_hp(j - 1, 0) + attnv_hp(j - 1, 1)
                                  + [(norm_piece, j - 1, 0)]
                                  + attnv_hp(j - 1, 2) + attnv_hp(j - 1, 3)
                                  + [(norm_piece, j - 1, 1)])
                if j - 1 >= 2:
                    slot_inter[j].append((transp_qt, j - 1))
            # out-projection: early s-tiles deferred to late slots (the early
            # slots carry the V/K/Q overload), the rest two slots after their
            # transpose.
            for st in range(0, 5):
                slot_inter[11 + st] += [(c_piece, st, 0), (c_piece, st, 1)]
            for st in range(5, 14):
                slot_inter[st + 2] += [(c_piece, st, 0), (c_piece, st, 1)]
            # Q chunk c: DMA early, project each dl-tile just before the
            # first score group of slot 4c that needs it.
            def qproj_mini(c, dch):
                """Q projection for q-tile 4c only (slot 4c's own columns)."""
                x = vstate["xq"]
                gp = gpool.tile([P, QC], f32, tag="gp", name="gp")
                for e in range(ET):
                    mm(gp[:, 0:P],
                       wq[:, e * DL + dch * P : e * DL + (dch + 1) * P],
                       x[:, e * QC : e * QC + P], e == 0, e == ET - 1)
                nc.vector.tensor_scalar_add(
                    qt[dch][:, 4 * c * P : (4 * c + 1) * P], gp[:, 0:P],
                    bq_t[:, dch : dch + 1])

            def qproj_rest(c, dch):
                """Q projection for q-tiles 4c+1..4c+3 (needed next slot)."""
                x = vstate["xq"]
                gp = gpool.tile([P, QC], f32, tag="gp", name="gp")
                for e in range(ET):
                    mm(gp[:, 0 : 3 * P],
                       wq[:, e * DL + dch * P : e * DL + (dch + 1) * P],
                       x[:, e * QC + P : (e + 1) * QC], e == 0, e == ET - 1)
                nc.vector.tensor_scalar_add(
                    qt[dch][:, (4 * c + 1) * P : (4 * c + 4) * P],
                    gp[:, 0 : 3 * P], bq_t[:, dch : dch + 1])

            slot_hp = [[[] for _ in range(HP)] for _ in range(ST)]
            for c in (1, 2, 3):
                slot_inter[4 * c - 2].append(("loadxq", c))
                for dch in range(NDT):
                    slot_hp[4 * c][dch].append((qproj_mini, c, dch))
                slot_inter[4 * c] += [(qproj_rest, c, dch)
                                      for dch in range(NDT)]

            # ---------------- head + slot loop ----------------------------
            nc.scalar.dma_start(
                out=wk[:].rearrange("p (e d) -> p e d", e=ET),
                in_=wkT[:].rearrange("(e p) d -> p e d", p=P))
            load_xk(0)
            load_xq(0, nc.scalar)
            nc.sync.dma_start(
                out=wq[:].rearrange("p (e d) -> p e d", e=ET),
                in_=wqT[:].rearrange("(e p) d -> p e d", p=P))
            nc.sync.dma_start(out=bk_t[:].rearrange("p i -> p i ()"),
                              in_=bkd[:].rearrange("(i p) o -> p i o", p=P))
            nc.sync.dma_start(out=bq_t[:].rearrange("p i -> p i ()"),
                              in_=bqd[:].rearrange("(i p) o -> p i o", p=P))
            # PE clock warm-up while the first loads are in flight: harmless
            # zero matmuls keep the PE busy so it reaches full p-state before
            # the first projection.
            for wu in range(26):
                wsc = spool.tile([P, 2 * KG * P], f32, tag="sc", name="sc")
                for j in range(4):
                    mm(wsc[:, j * P : (j + 1) * P], warm[:], warm[:], True, True)

            budget = [0.0]

            def drain(inter, force=False, to_marker=None):
                while inter:
                    if (not force and to_marker is None
                            and piece_cost(inter[0]) > budget[0]):
                        return
                    p = inter.popleft()
                    run_piece(p)
                    budget[0] -= piece_cost(p)
                    if to_marker is not None and p == ("HPM", to_marker):
                        return

            def do_sc(qi, hp, kg, inter):
                sc_use(qi, hp, kg)
                budget[0] = min(budget[0] + (EXP_NS - SC_NS), 4000.0)
                drain(inter)

            # slot 0: interleave K/Q chunk-0 projections with the first score
            # groups (head pair hp becomes ready as soon as dl-tile hp
            # projects), then kg 1-3 as K chunks 1-3 land.
            inter0 = deque(slot_inter[0])
            for dch in range(NDT):
                proj_piece(0, dch, wk, "xk", bk_t, kt)
                proj_piece(0, dch, wq, "xq", bq_t, qt)
                sc_use(0, dch, 0)
            xk_next = {}
            load_xk(1)
            xk_next[1] = vstate["xk"]
            for kg in range(1, NKG):
                vstate["xk"] = xk_next[kg]
                if kg + 1 < NKG:
                    load_xk(kg + 1)
                    xk_next[kg + 1] = vstate["xk"]
                    vstate["xk"] = xk_next[kg]
                for dch in range(NDT):
                    proj_piece(kg, dch, wk, "xk", bk_t, kt)
                for hp in range(HP):
                    do_sc(0, hp, kg, inter0)
            drain(inter0, force=True)
            s1.close()
            open_vx()

            for qi in range(1, ST):
                inter = deque(slot_inter[qi])
                for hp in range(HP):
                    if hp in slot_markers[qi]:
                        drain(inter, to_marker=hp)
                    for p in slot_hp[qi][hp]:
                        run_piece(p)
                    for kg in range(NKG):
                        if hp == HP - 1 and kg == 0:
                            # flush leftovers while the previous exp groups
                            # still cover the scalar engine
                            drain(inter, force=True)
                        do_sc(qi, hp, kg, inter)
                drain(inter, force=True)

            # ---------------- tail ----------------------------------------
            for piece in (attnv_hp(ST - 1, 0) + attnv_hp(ST - 1, 1)
                          + [(norm_piece, ST - 1, 0)]
                          + attnv_hp(ST - 1, 2) + attnv_hp(ST - 1, 3)
                          + [(norm_piece, ST - 1, 1)]):
                run_piece(piece)
            transp_qt(ST - 1)
            for st in range(ST - 2, ST):
                for fc in range(D // QC):
                    c_piece(st, fc)
            s3.close()

    nc.compile()
    return nc


def make_in_maps(query, key, value, Wq, bq, Wk, bk, Wv, bv, n_cores=8,
                 mm_dtype="float32r"):
    """Host-side sharding: slice weights Megatron-style, transpose activations."""
    import ml_dtypes

    bft = ml_dtypes.bfloat16
    q = np.asarray(query, dtype=np.float32)
    k = np.asarray(key, dtype=np.float32)
    v = np.asarray(value, dtype=np.float32)
    Wq = np.asarray(Wq, dtype=np.float32)
    Wk = np.asarray(Wk, dtype=np.float32)
    Wv = np.asarray(Wv, dtype=np.float32)
    bq = np.asarray(bq, dtype=np.float32)
    bk = np.asarray(bk, dtype=np.float32)
    D = Wq.shape[0]
    DL = D // (n_cores // q.shape[0])
    scale = 1.0 / np.sqrt(np.float32(DK))
    in_maps = []
    for c in range(n_cores):
        b, g = divmod(c, n_cores // q.shape[0])
        sl = slice(DL * g, DL * (g + 1))
        in_maps.append(
            {
                "xqT": np.ascontiguousarray(q[b].T).astype(bft),
                "xkT": np.ascontiguousarray(k[b].T).astype(bft),
                "xvT": np.ascontiguousarray(v[b].T).astype(bft),
                "wqT": (np.ascontiguousarray(Wq[sl].T) * scale).astype(bft),
                "wkT": np.ascontiguousarray(Wk[sl].T).astype(bft),
                "wvT": np.ascontiguousarray(Wv[sl].T).astype(bft),
                "bq": np.ascontiguousarray((bq[sl] * scale).reshape(DL, 1)),
                "bk": np.ascontiguousarray(bk[sl].reshape(DL, 1)),
            }
        )
    return in_maps


def add_wo_maps(in_maps, Wo, n_cores=8, n_batch=4, mm_dtype="float32r"):
    import ml_dtypes

    Wo = np.asarray(Wo, dtype=np.float32)
    D = Wo.shape[0]
    DL = D // (n_cores // n_batch)
    for c in range(n_cores):
        _, g = divmod(c, n_cores // n_batch)
        sl = slice(DL * g, DL * (g + 1))
        in_maps[c]["woT"] = np.ascontiguousarray(Wo[:, sl].T).astype(ml_dtypes.bfloat16)
    return in_maps


MM_DTYPE = "float32r"


def kernel(query, key, value, Wq, bq, Wk, bk, Wv, bv, Wo, bo):
    if "nc" not in _CACHE:
        _CACHE["nc"] = build_nc(mm_dtype=MM_DTYPE)
    nc = _CACHE["nc"]
    n_cores = 8
    in_maps = make_in_maps(
        query, key, value, Wq, bq, Wk, bk, Wv, bv, n_cores, MM_DTYPE
    )
    add_wo_maps(in_maps, Wo, n_cores, np.asarray(query).shape[0], MM_DTYPE)
    res = run_bass_kernel_spmd(nc, in_maps, list(range(n_cores)))
    ys = [res.results[c]["y"] for c in range(n_cores)]
    bo = np.asarray(bo, dtype=np.float32)
    bv = np.asarray(bv, dtype=np.float32)
    Wo = np.asarray(Wo, dtype=np.float32)
    const = bo + bv @ Wo.T
    out = np.stack([ys[2 * b] + ys[2 * b + 1] for b in range(4)]) + const[None, None, :]
    return out.astype(np.float32)


# revision 69
# speedup vs baseline: 1.0090x; 1.0005x over previous
"""Trainium2 Bass kernel for nn_MultiHeadAttention_37838661877847.

Full-input contract: kernel(**inputs) takes the complete tensors and returns
the complete output. Internally shards across 8 NeuronCores:
  core c -> batch b = c // 2, head-group g = c % 2 (8 heads, 512 dims each).
Each core computes Q/K/V projections for its (batch, head-group) slice
(column-parallel weights), attention for its 8 heads, and a partial output
projection (row-parallel Wo). Host sums core pairs and adds bo + bv @ Wo.T
(the V bias commutes through softmax-weighted averaging, so it is folded
into the output-projection bias on the host).

Engine-level design (per core), built as ONE interleaved instruction stream
so the scalar engine's softmax-exp (the 266us serial floor: 33.5M exps at
1 elem/lane/cycle) overlaps the tensor engine work (281us):

  - Q_T/K_T stored (dl, s) in bf16; scores come out (k, q) per 128-k tile.
  - exp groups of [128, 1024] PSUM (4 score blocks: 2 heads x ... see sc
    layout below) -> ets tiles in bf16.
  - attn@V is FLIPPED: out (q, dk+1) accumulating over k with the exp tile
    as the stationary operand -> 65-row bf16 matmuls, half the PE rows of
    the (dk+1, q) orientation. V is augmented with a ones column per head so
    the softmax denominator Z lands in column 64; normalization is then a
    per-partition reciprocal+scale on DVE.
  - normalized output (q, dl) is transposed back to (dl, q) via PE-transpose
    through spare score-PSUM space, then the output projection streams wo.
  - emission interleaves projections / attn@V / transposes / out-proj between
    score+exp groups so the scalar engine rarely starves.

mm dtypes: x and w_qk in f32r/bf16 keep projections+scores accurate; the
attention path (probs, V, attn-out, Wo) runs in bf16 (PSUM accumulation is
fp32 throughout).
"""

import sys

sys.path.insert(0, "/opt/trn_rl_repo")

from collections import deque
from contextlib import ExitStack

import numpy as np

import concourse.bass as bass  # noqa: F401
import concourse.tile as tile
from concourse import bacc, masks, mybir
from concourse.bass_utils import run_bass_kernel_spmd

P = 128
DK = 64  # head dim

_CACHE = {}


def build_nc(S=2048, D=1024, DL=512, mm_dtype="float32r", n_cores=8,
             repeats=1, phases="ABC"):
    """Build + compile the per-core Bass program (same program on all cores).

    repeats exists only for timing experiments; production uses the default.
    mm_dtype/phases are accepted for test-harness compatibility (the kernel
    uses a fixed mixed f32r/bf16 precision scheme).
    """
    f32 = mybir.dt.float32
    f32r = mybir.dt.float32r
    bf16 = mybir.dt.bfloat16
    Exp = mybir.ActivationFunctionType.Exp

    H = DL // DK          # 8 local heads
    HP = H // 2           # 4 head pairs (one pair per 128-row q/k tile)
    ET = D // P           # 8 contraction tiles for projections
    ST = S // P           # 16 k tiles (and q tiles)
    NDT = DL // P         # 4 dl tiles
    QC = 512              # projection s-chunk
    NQ = S // QC          # 4
    KG = 4                # k-tiles per exp group
    NKG = ST // KG        # 4
    VW = H * (DK + 1)     # 520: v tile width incl. ones columns

    nc = bacc.Bacc("TRN2", target_bir_lowering=False, num_devices=n_cores)

    xqT = nc.dram_tensor("xqT", [D, S], bf16, kind="ExternalInput")
    xkT = nc.dram_tensor("xkT", [D, S], bf16, kind="ExternalInput")
    xvT = nc.dram_tensor("xvT", [D, S], bf16, kind="ExternalInput")
    wqT = nc.dram_tensor("wqT", [D, DL], bf16, kind="ExternalInput")
    wkT = nc.dram_tensor("wkT", [D, DL], bf16, kind="ExternalInput")
    wvT = nc.dram_tensor("wvT", [D, DL], bf16, kind="ExternalInput")
    woT = nc.dram_tensor("woT", [DL, D], bf16, kind="ExternalInput")
    bqd = nc.dram_tensor("bq", [DL, 1], f32, kind="ExternalInput")
    bkd = nc.dram_tensor("bk", [DL, 1], f32, kind="ExternalInput")
    y = nc.dram_tensor("y", [S, D], f32, kind="ExternalOutput")

    def mm(out, lhsT, rhs, start, stop):
        nc.tensor.matmul(out, lhsT=lhsT, rhs=rhs, start=start, stop=stop)

    with tile.TileContext(nc) as tc, ExitStack() as top:
        top.enter_context(
            nc.allow_low_precision(
                reason="attention path in bf16; PSUM accumulation stays fp32"
            )
        )
        persist = top.enter_context(tc.tile_pool(name="persist", bufs=1))
        qt = [persist.tile([P, S], bf16, tag=f"qt{i}", name=f"qt{i}") for i in range(NDT)]
        kt = [persist.tile([P, S], bf16, tag=f"kt{i}", name=f"kt{i}") for i in range(NDT)]
        vt = [persist.tile([P, VW], bf16, tag=f"vt{i}", name=f"vt{i}") for i in range(ST)]
        oaT = [persist.tile([P, S], bf16, tag=f"oaT{i}", name=f"oaT{i}") for i in range(NDT)]
        ident = persist.tile([P, P], f32, tag="ident", name="ident")
        bq_t = persist.tile([P, NDT], f32, tag="bq", name="bq")
        bk_t = persist.tile([P, NDT], f32, tag="bk", name="bk")

        masks.make_identity(nc, ident[:])
        warm = persist.tile([P, P], bf16, tag="warm", name="warm")
        nc.vector.memset(warm[:], 0.0)
        # vt ones-columns are memset inside slot 0 (below) so the head's
        # K/Q projection evacuations reach the DVE queue first.

        # PSUM: scores/exp 2x[128,1024] (4 banks) + attn@V accum 2x[128,260]
        # (2 banks) + generic matmul 2x[128,512] (2 banks) = 8 banks.
        spool = top.enter_context(tc.tile_pool(name="spool", bufs=2, space="PSUM"))
        acpool = top.enter_context(tc.tile_pool(name="acpool", bufs=2, space="PSUM"))
        gpool = top.enter_context(tc.tile_pool(name="gpool", bufs=2, space="PSUM"))

        # weight/x pools for Q (live through all Q chunks); wide layouts:
        # w tiles hold all ET contraction blocks side by side (one DMA each).
        wqp = top.enter_context(tc.tile_pool(name="wqp", bufs=1))
        wq = wqp.tile([P, ET * DL], bf16, tag="wq", name="wq")
        xqp = top.enter_context(tc.tile_pool(name="xqp", bufs=1))

        # long-lived attention pools (opened before any scoped pool so that
        # mid-stream pool closes stay LIFO)
        etsp = top.enter_context(tc.tile_pool(name="etsp", bufs=2))
        oasp = top.enter_context(tc.tile_pool(name="oasp", bufs=4))
        yvp = top.enter_context(tc.tile_pool(name="yvp", bufs=2))
        rcp = top.enter_context(tc.tile_pool(name="rcp", bufs=4))

        for _rep in range(repeats):
            # ---------------- pools for K and Q chunk streams -------------
            vstate = {}
            s3 = ExitStack()
            s2 = ExitStack()
            vxa = s2.enter_context(tc.tile_pool(name="vxa", bufs=1))
            vstate["wv"] = vxa.tile([P, ET * DL], bf16, tag="wv", name="wv")
            vstate["xv0"] = vxa.tile([P, ET * (S // 2)], bf16, tag="xv0",
                                     name="xv0")
            s1 = ExitStack()
            kx = s1.enter_context(tc.tile_pool(name="kx", bufs=2))
            wkp = s1.enter_context(tc.tile_pool(name="wkp", bufs=1))
            wk = wkp.tile([P, ET * DL], bf16, tag="wk", name="wk")

            def load_xk(c, eng=None):
                xkc = kx.tile([P, ET * QC], bf16, tag="xk", name="xk")
                (eng or nc.sync).dma_start(
                    out=xkc[:].rearrange("p (e s) -> p e s", e=ET),
                    in_=xkT[:, c * QC : (c + 1) * QC].rearrange(
                        "(e p) s -> p e s", p=P),
                )
                vstate["xk"] = xkc

            def load_xq(c, eng):
                xqc = xqp.tile([P, ET * QC], bf16, tag="xq", name="xq")
                eng.dma_start(
                    out=xqc[:].rearrange("p (e s) -> p e s", e=ET),
                    in_=xqT[:, c * QC : (c + 1) * QC].rearrange(
                        "(e p) s -> p e s", p=P),
                )
                vstate["xq"] = xqc

            def proj_piece(c, dch, w, xkey, bias, out_tiles):
                """One (chunk, dl-tile) projection: out (dl 128, s 512) + bias."""
                x = vstate[xkey]
                gp = gpool.tile([P, QC], f32, tag="gp", name="gp")
                for e in range(ET):
                    mm(gp[:], w[:, e * DL + dch * P : e * DL + (dch + 1) * P],
                       x[:, e * QC : (e + 1) * QC], e == 0, e == ET - 1)
                nc.vector.tensor_scalar_add(
                    out_tiles[dch][:, c * QC : (c + 1) * QC], gp[:],
                    bias[:, dch : dch + 1]
                )

            # --- V pools: wv + the first s-half of xv preload alongside the
            # K pool (slot 0); the second s-half lands in the space the K pool
            # frees. V projection runs head-half-major so attn@V for heads 0-3
            # unblocks as early as possible.
            SH = S // 2

            def load_wv():
                nc.sync.dma_start(
                    out=vstate["wv"][:].rearrange("p (e d) -> p e d", e=ET),
                    in_=wvT[:].rearrange("(e p) d -> p e d", p=P),
                )

            def load_xv0():
                nc.sync.dma_start(
                    out=vstate["xv0"][:].rearrange("p (e s) -> p e s", e=ET),
                    in_=xvT[:, 0:SH].rearrange("(e p) s -> p e s", p=P),
                )

            def open_vx():
                vxb = s2.enter_context(tc.tile_pool(name="vxb", bufs=1))
                vstate["xv1"] = vxb.tile([P, ET * SH], bf16, tag="xv1", name="xv1")
                nc.sync.dma_start(
                    out=vstate["xv1"][:].rearrange("p (e s) -> p e s", e=ET),
                    in_=xvT[:, SH:S].rearrange("(e p) s -> p e s", p=P),
                )

            def vproj_piece(st, qtr):
                """V projection for (s-tile st, head pair qtr): 2 heads.
                Quarters 0-1 run before attn@V starts, so odd s-tiles borrow
                the idle attn@V accumulator banks for deeper pipelining."""
                Q4 = DL // 4
                if qtr < 2 and st % 2 == 1:
                    gp = acpool.tile([P, 512], f32, tag="ac", name="ac")
                else:
                    gp = gpool.tile([P, QC], f32, tag="gp", name="gp")
                wv = vstate["wv"]
                xv = vstate["xv0"] if st < ST // 2 else vstate["xv1"]
                stl = st % (ST // 2)
                for e in range(ET):
                    mm(gp[:, 0:Q4],
                       xv[:, e * SH + stl * P : e * SH + (stl + 1) * P],
                       wv[:, e * DL + qtr * Q4 : e * DL + (qtr + 1) * Q4],
                       e == 0, e == ET - 1)
                nc.vector.tensor_copy(
                    vt[st][:].rearrange("p (h c) -> p h c", h=H)
                    [:, qtr * 2 : (qtr + 1) * 2, 0:DK],
                    gp[:, 0:Q4].rearrange("p (h c) -> p h c", h=2),
                )

            state = {
                "ets": {},    # (qt_idx, hp, kg) -> tile  (live window)
                "ac": {},     # (qt_idx, hgrp) -> tile
                "oas": {},    # qt_idx -> tile
                "wo": None,
            }

            def sc_use(qi, hp, kg):
                """Scores + exp for (q-tile qi, head pair hp, k-group kg)."""
                sc = spool.tile([P, 2 * KG * P], f32, tag="sc", name="sc")
                for hloc in range(2):
                    h = 2 * hp + hloc
                    r0 = hloc * DK
                    for ktl in range(KG):
                        ki = kg * KG + ktl
                        mm(
                            sc[:, hloc * KG * P + ktl * P : hloc * KG * P + (ktl + 1) * P],
                            kt[hp][r0 : r0 + DK, ki * P : (ki + 1) * P],
                            qt[hp][r0 : r0 + DK, qi * P : (qi + 1) * P],
                            True,
                            True,
                        )
                et = etsp.tile([P, 2 * KG * P], bf16, tag=f"et{hp}_{kg}",
                               name=f"et{hp}_{kg}")
                nc.scalar.activation(et[:], sc[:], Exp)
                state["ets"][(qi, hp, kg)] = et

            def attnv_piece(qi, h, kg):
                """attn@V for (q-tile qi, head h, k-group kg): 4 x 65-row mms."""
                hgrp, hidx = divmod(h, 4)
                key = (qi, hgrp)
                if key not in state["ac"]:
                    # padded to a full 2KB bank; cols 0-259 used (4 heads x 65)
                    state["ac"][key] = acpool.tile([P, 512], f32, tag="ac", name="ac")
                ac = state["ac"][key]
                et = state["ets"][(qi, h // 2, kg)]
                hloc = h % 2
                for ktl in range(KG):
                    ki = kg * KG + ktl
                    mm(
                        ac[:, hidx * (DK + 1) : (hidx + 1) * (DK + 1)],
                        et[:, hloc * KG * P + ktl * P : hloc * KG * P + (ktl + 1) * P],
                        vt[ki][:, h * (DK + 1) : (h + 1) * (DK + 1)],
                        ki == 0,
                        ki == ST - 1,
                    )
                if hloc == 1:
                    del state["ets"][(qi, h // 2, kg)]

            def norm_piece(qi, hgrp):
                """Normalize 4 heads: oa_s[:, hgrp*256:+256] = num * (1/Z)."""
                if qi not in state["oas"]:
                    state["oas"][qi] = oasp.tile([P, DL], f32, tag="oas", name="oas")
                oas = state["oas"][qi]
                ac = state["ac"].pop((qi, hgrp))
                acr = ac[:, 0 : 4 * (DK + 1)].rearrange("p (h c) -> p h c", h=4)
                rc = rcp.tile([P, 4], f32, tag="rc", name="rc")
                nc.vector.reciprocal(rc[:], acr[:, :, DK])
                for hh in range(4):
                    nc.vector.tensor_scalar_mul(
                        oas[:, hgrp * 4 * DK + hh * DK : hgrp * 4 * DK + (hh + 1) * DK],
                        acr[:, hh, 0:DK],
                        rc[:, hh : hh + 1],
                    )

            def transp_qt(qi):
                """Transpose oa_s (q, dl) -> oaT (dl, q) for one q-tile."""
                sc = spool.tile([P, 2 * KG * P], f32, tag="sc", name="sc")
                oas = state["oas"].pop(qi)
                for dlb in range(NDT):
                    nc.tensor.transpose(
                        sc[:, dlb * P : (dlb + 1) * P],
                        oas[:, dlb * P : (dlb + 1) * P],
                        ident[:],
                    )
                for dlb in range(NDT):
                    nc.vector.tensor_copy(
                        oaT[dlb][:, qi * P : (qi + 1) * P], sc[:, dlb * P : (dlb + 1) * P]
                    )

            def load_wo():
                wop = s3.enter_context(tc.tile_pool(name="wop", bufs=1))
                wo = wop.tile([P, NDT * D], bf16, tag="wo", name="wo")
                nc.sync.dma_start(
                    out=wo[:].rearrange("p (i d) -> p i d", i=NDT),
                    in_=woT[:].rearrange("(i p) d -> p i d", p=P),
                )
                state["wo"] = wo

            def c_piece(st, fc):
                """Output projection for (s-tile st, f-chunk fc)."""
                wo = state["wo"]
                gp = gpool.tile([P, QC], f32, tag="gp", name="gp")
                for dl in range(NDT):
                    mm(gp[:], oaT[dl][:, st * P : (st + 1) * P],
                       wo[:, dl * D + fc * QC : dl * D + (fc + 1) * QC],
                       dl == 0, dl == NDT - 1)
                yv = yvp.tile([P, QC], f32, tag="yv", name="yv")
                nc.vector.tensor_copy(yv[:], gp[:])
                nc.sync.dma_start(
                    out=y[st * P : (st + 1) * P, fc * QC : (fc + 1) * QC], in_=yv[:]
                )

            # ------------- interleaved emission ---------------------------
            # One FIFO of side pieces per slot, drained between score+exp
            # groups under a PE-lead budget, force-drained at slot end (and at
            # the MID marker before the hp2/hp3 half). Estimated PE ns/piece.
            EXP_NS, SC_NS = 1090.0, 430.0
            COST = {}

            def piece_cost(p):
                fn = p[0]
                if fn == proj_piece:
                    return 1750.0
                if fn == qproj_rest:
                    return 1350.0
                if fn == vproj_piece:
                    return 450.0
                if fn == attnv_piece:
                    return 160.0
                if fn == transp_qt:
                    return 520.0
                if fn == c_piece:
                    return 900.0
                return 0.0

            def run_piece(p):
                if p[0] == "loadxk":
                    load_xk(p[1])
                elif p[0] == "loadxq":
                    load_xq(p[1], nc.sync)
                elif p[0] == "loadwo":
                    load_wo()
                elif p[0] == "closes1":
                    s1.close()
                elif p[0] == "openvx":
                    open_vx()
                elif p[0] == "memset":
                    nc.vector.memset(vt[p[1]][:], 1.0)
                elif p[0] == "loadwv":
                    load_wv()
                elif p[0] == "loadxv0":
                    load_xv0()
                elif p[0] == "closes2":
                    s2.close()
                elif p[0] == "HPM":
                    pass
                else:
                    p[0](*p[1:])

            def attnv_hp(qi, hp):
                out = []
                for h in (2 * hp, 2 * hp + 1):
                    for kg in range(NKG):
                        out.append((attnv_piece, qi, h, kg))
                return out

            slot_inter = [[] for _ in range(ST)]
            slot_markers = [set() for _ in range(ST)]
            slot_inter[0] += [("memset", i) for i in range(ST)]
            slot_inter[0] += [("loadwv",), ("loadxv0",)]
            slot_inter[0] += [(vproj_piece, st, 0) for st in range(ST // 2)]
            slot_inter[1] += [(vproj_piece, st, 0) for st in range(ST // 2, ST)]
            slot_inter[1] += attnv_hp(0, 0)
            slot_inter[1] += [(vproj_piece, st, 1) for st in range(ST)]
            slot_inter[1] += attnv_hp(0, 1) + [(norm_piece, 0, 0)]
            slot_inter[2] += [(vproj_piece, st, 2) for st in range(ST)]
            slot_inter[2] += attnv_hp(0, 2) + [("HPM", 2)]
            slot_inter[2] += [(vproj_piece, st, 3) for st in range(ST)]
            slot_inter[2] += attnv_hp(0, 3) + [(norm_piece, 0, 1), ("HPM", 3)]
            slot_inter[2] += [("closes2",), ("loadwo",)]
            slot_markers[2] = {2, 3}
            slot_inter[3] += (attnv_hp(1, 0) + [("HPM", 0)]
                              + attnv_hp(1, 1) + [(norm_piece, 1, 0), ("HPM", 1)]
                              + attnv_hp(1, 2) + [("HPM", 2)]
                              + attnv_hp(1, 3) + [(norm_piece, 1, 1), ("HPM", 3)])
            slot_markers[3] = {0, 1, 2, 3}
            slot_inter[3] += [(transp_qt, 0), (transp_qt, 1)]
            for j in range(3, ST):
                slot_inter[j] += (attnv_hp(j - 1, 0) + attnv_hp(j - 1, 1)
                                  + [(norm_piece, j - 1, 0)]
                                  + attnv_hp(j - 1, 2) + attnv_hp(j - 1, 3)
                                  + [(norm_piece, j - 1, 1)])
                if j - 1 >= 2:
                    slot_inter[j].append((transp_qt, j - 1))
            # out-projection: early s-tiles deferred to late slots (the early
            # slots carry the V/K/Q overload), the rest two slots after their
            # transpose.
            for st in range(0, 5):
                slot_inter[11 + st] += [(c_piece, st, 0), (c_piece, st, 1)]
            for st in range(5, 14):
                slot_inter[st + 2] += [(c_piece, st, 0), (c_piece, st, 1)]
            # Q chunk c: DMA early, project each dl-tile just before the
            # first score group of slot 4c that needs it.
            def qproj_mini(c, dch):
                """Q projection for q-tile 4c only (slot 4c's own columns)."""
                x = vstate["xq"]
                gp = gpool.tile([P, QC], f32, tag="gp", name="gp")
                for e in range(ET):
                    mm(gp[:, 0:P],
                       wq[:, e * DL + dch * P : e * DL + (dch + 1) * P],
                       x[:, e * QC : e * QC + P], e == 0, e == ET - 1)
                nc.vector.tensor_scalar_add(
                    qt[dch][:, 4 * c * P : (4 * c + 1) * P], gp[:, 0:P],
                    bq_t[:, dch : dch + 1])

            def qproj_rest(c, dch):
                """Q projection for q-tiles 4c+1..4c+3 (needed next slot)."""
                x = vstate["xq"]
                gp = gpool.tile([P, QC], f32, tag="gp", name="gp")
                for e in range(ET):
                    mm(gp[:, 0 : 3 * P],
                       wq[:, e * DL + dch * P : e * DL + (dch + 1) * P],
                       x[:, e * QC + P : (e + 1) * QC], e == 0, e == ET - 1)
                nc.vector.tensor_scalar_add(
                    qt[dch][:, (4 * c + 1) * P : (4 * c + 4) * P],
                    gp[:, 0 : 3 * P], bq_t[:, dch : dch + 1])

            slot_hp = [[[] for _ in range(HP)] for _ in range(ST)]
            for c in (1, 2, 3):
                slot_inter[4 * c - 2].append(("loadxq", c))
                for dch in range(NDT):
                    slot_hp[4 * c][dch].append((qproj_mini, c, dch))
                slot_inter[4 * c] += [(qproj_rest, c, dch)
                                      for dch in range(NDT)]

            # ---------------- head + slot loop ----------------------------
            nc.scalar.dma_start(
                out=wk[:].rearrange("p (e d) -> p e d", e=ET),
                in_=wkT[:].rearrange("(e p) d -> p e d", p=P))
            load_xk(0)
            load_xq(0, nc.scalar)
            nc.sync.dma_start(
                out=wq[:].rearrange("p (e d) -> p e d", e=ET),
                in_=wqT[:].rearrange("(e p) d -> p e d", p=P))
            nc.sync.dma_start(out=bk_t[:].rearrange("p i -> p i ()"),
                              in_=bkd[:].rearrange("(i p) o -> p i o", p=P))
            nc.sync.dma_start(out=bq_t[:].rearrange("p i -> p i ()"),
                              in_=bqd[:].rearrange("(i p) o -> p i o", p=P))
            # PE clock warm-up while the first loads are in flight: harmless
            # zero matmuls keep the PE busy so it reaches full p-state before
            # the first projection.
            for wu in range(26):
                wsc = spool.tile([P, 2 * KG * P], f32, tag="sc", name="sc")
                for j in range(4):
                    mm(wsc[:, j * P : (j + 1) * P], warm[:], warm[:], True, True)

            budget = [0.0]

            def drain(inter, force=False, to_marker=None):
                while inter:
                    if (not force and to_marker is None
                            and piece_cost(inter[0]) > budget[0]):
                        return
                    p = inter.popleft()
                    run_piece(p)
                    budget[0] -= piece_cost(p)
                    if to_marker is not None and p == ("HPM", to_marker):
                        return

            def do_sc(qi, hp, kg, inter):
                sc_use(qi, hp, kg)
                budget[0] = min(budget[0] + (EXP_NS - SC_NS), 4000.0)
                drain(inter)

            # slot 0: interleave K/Q chunk-0 projections with the first score
            # groups (head pair hp becomes ready as soon as dl-tile hp
            # projects), then kg 1-3 as K chunks 1-3 land.
            inter0 = deque(slot_inter[0])
            for dch in range(NDT):
                proj_piece(0, dch, wk, "xk", bk_t, kt)
                proj_piece(0, dch, wq, "xq", bq_t, qt)
                sc_use(0, dch, 0)
            xk_next = {}
            load_xk(1)
            xk_next[1] = vstate["xk"]
            for kg in range(1, NKG):
                vstate["xk"] = xk_next[kg]
                if kg + 1 < NKG:
                    load_xk(kg + 1)
                    xk_next[kg + 1] = vstate["xk"]
                    vstate["xk"] = xk_next[kg]
                for dch in range(NDT):
                    proj_piece(kg, dch, wk, "xk", bk_t, kt)
                for hp in range(HP):
                    do_sc(0, hp, kg, inter0)
            drain(inter0, force=True)
            s1.close()
            open_vx()

            for qi in range(1, ST):
                inter = deque(slot_inter[qi])
                for hp in range(HP):
                    if hp in slot_markers[qi]:
                        drain(inter, to_marker=hp)
                    for p in slot_hp[qi][hp]:
                        run_piece(p)
                    for kg in range(NKG):
                        if hp == HP - 1 and kg == 0:
                            # flush leftovers while the previous exp groups
                            # still cover the scalar engine
                            drain(inter, force=True)
                        do_sc(qi, hp, kg, inter)
                drain(inter, force=True)

            # ---------------- tail ----------------------------------------
            for piece in (attnv_hp(ST - 1, 0) + attnv_hp(ST - 1, 1)
                          + [(norm_piece, ST - 1, 0)]
                          + attnv_hp(ST - 1, 2) + attnv_hp(ST - 1, 3)
                          + [(norm_piece, ST - 1, 1)]):
                run_piece(piece)
            transp_qt(ST - 1)
            for st in range(ST - 2, ST):
                for fc in range(D // QC):
                    c_piece(st, fc)
            s3.close()

    nc.compile()
    return nc


def make_in_maps(query, key, value, Wq, bq, Wk, bk, Wv, bv, n_cores=8,
                 mm_dtype="float32r"):
    """Host-side sharding: slice weights Megatron-style, transpose activations."""
    import ml_dtypes

    bft = ml_dtypes.bfloat16
    q = np.asarray(query, dtype=np.float32)
    k = np.asarray(key, dtype=np.float32)
    v = np.asarray(value, dtype=np.float32)
    Wq = np.asarray(Wq, dtype=np.float32)
    Wk = np.asarray(Wk, dtype=np.float32)
    Wv = np.asarray(Wv, dtype=np.float32)
    bq = np.asarray(bq, dtype=np.float32)
    bk = np.asarray(bk, dtype=np.float32)
    D = Wq.shape[0]
    DL = D // (n_cores // q.shape[0])
    scale = 1.0 / np.sqrt(np.float32(DK))
    in_maps = []
    for c in range(n_cores):
        b, g = divmod(c, n_cores // q.shape[0])
        sl = slice(DL * g, DL * (g + 1))
        in_maps.append(
            {
                "xqT": np.ascontiguousarray(q[b].T).astype(bft),
                "xkT": np.ascontiguousarray(k[b].T).astype(bft),
                "xvT": np.ascontiguousarray(v[b].T).astype(bft),
                "wqT": (np.ascontiguousarray(Wq[sl].T) * scale).astype(bft),
                "wkT": np.ascontiguousarray(Wk[sl].T).astype(bft),
                "wvT": np.ascontiguousarray(Wv[sl].T).astype(bft),
                "bq": np.ascontiguousarray((bq[sl] * scale).reshape(DL, 1)),
                "bk": np.ascontiguousarray(bk[sl].reshape(DL, 1)),
            }
        )
    return in_maps


def add_wo_maps(in_maps, Wo, n_cores=8, n_batch=4, mm_dtype="float32r"):
    import ml_dtypes

    Wo = np.asarray(Wo, dtype=np.float32)
    D = Wo.shape[0]
    DL = D // (n_cores // n_batch)
    for c in range(n_cores):
        _, g = divmod(c, n_cores // n_batch)
        sl = slice(DL * g, DL * (g + 1))
        in_maps[c]["woT"] = np.ascontiguousarray(Wo[:, sl].T).astype(ml_dtypes.bfloat16)
    return in_maps


MM_DTYPE = "float32r"


def kernel(query, key, value, Wq, bq, Wk, bk, Wv, bv, Wo, bo):
    if "nc" not in _CACHE:
        _CACHE["nc"] = build_nc(mm_dtype=MM_DTYPE)
    nc = _CACHE["nc"]
    n_cores = 8
    in_maps = make_in_maps(
        query, key, value, Wq, bq, Wk, bk, Wv, bv, n_cores, MM_DTYPE
    )
    add_wo_maps(in_maps, Wo, n_cores, np.asarray(query).shape[0], MM_DTYPE)
    res = run_bass_kernel_spmd(nc, in_maps, list(range(n_cores)))
    ys = [res.results[c]["y"] for c in range(n_cores)]
    bo = np.asarray(bo, dtype=np.float32)
    bv = np.asarray(bv, dtype=np.float32)
    Wo = np.asarray(Wo, dtype=np.float32)
    const = bo + bv @ Wo.T
    out = np.stack([ys[2 * b] + ys[2 * b + 1] for b in range(4)]) + const[None, None, :]
    return out.astype(np.float32)


# revision 74
# speedup vs baseline: 1.0111x; 1.0020x over previous
"""Trainium2 Bass kernel for nn_MultiHeadAttention_37838661877847.

Full-input contract: kernel(**inputs) takes the complete tensors and returns
the complete output. Internally shards across 8 NeuronCores:
  core c -> batch b = c // 2, head-group g = c % 2 (8 heads, 512 dims each).
Each core computes Q/K/V projections for its (batch, head-group) slice
(column-parallel weights), attention for its 8 heads, and a partial output
projection (row-parallel Wo). Host sums core pairs and adds bo + bv @ Wo.T
(the V bias commutes through softmax-weighted averaging, so it is folded
into the output-projection bias on the host).

Engine-level design (per core), built as ONE interleaved instruction stream
so the scalar engine's softmax-exp (the 266us serial floor: 33.5M exps at
1 elem/lane/cycle) overlaps the tensor engine work (281us):

  - Q_T/K_T stored (dl, s) in bf16; scores come out (k, q) per 128-k tile.
  - exp groups of [128, 1024] PSUM (4 score blocks: 2 heads x ... see sc
    layout below) -> ets tiles in bf16.
  - attn@V is FLIPPED: out (q, dk+1) accumulating over k with the exp tile
    as the stationary operand -> 65-row bf16 matmuls, half the PE rows of
    the (dk+1, q) orientation. V is augmented with a ones column per head so
    the softmax denominator Z lands in column 64; normalization is then a
    per-partition reciprocal+scale on DVE.
  - normalized output (q, dl) is transposed back to (dl, q) via PE-transpose
    through spare score-PSUM space, then the output projection streams wo.
  - emission interleaves projections / attn@V / transposes / out-proj between
    score+exp groups so the scalar engine rarely starves.

mm dtypes: x and w_qk in f32r/bf16 keep projections+scores accurate; the
attention path (probs, V, attn-out, Wo) runs in bf16 (PSUM accumulation is
fp32 throughout).
"""

import sys

sys.path.insert(0, "/opt/trn_rl_repo")

from collections import deque
from contextlib import ExitStack

import numpy as np

import concourse.bass as bass  # noqa: F401
import concourse.tile as tile
from concourse import bacc, masks, mybir
from concourse.bass_utils import run_bass_kernel_spmd

P = 128
DK = 64  # head dim

_CACHE = {}


def build_nc(S=2048, D=1024, DL=512, mm_dtype="float32r", n_cores=8,
             repeats=1, phases="ABC"):
    """Build + compile the per-core Bass program (same program on all cores).

    repeats exists only for timing experiments; production uses the default.
    mm_dtype/phases are accepted for test-harness compatibility (the kernel
    uses a fixed mixed f32r/bf16 precision scheme).
    """
    f32 = mybir.dt.float32
    f32r = mybir.dt.float32r
    bf16 = mybir.dt.bfloat16
    Exp = mybir.ActivationFunctionType.Exp

    H = DL // DK          # 8 local heads
    HP = H // 2           # 4 head pairs (one pair per 128-row q/k tile)
    ET = D // P           # 8 contraction tiles for projections
    ST = S // P           # 16 k tiles (and q tiles)
    NDT = DL // P         # 4 dl tiles
    QC = 512              # projection s-chunk
    NQ = S // QC          # 4
    KG = 4                # k-tiles per exp group
    NKG = ST // KG        # 4
    VW = H * (DK + 1)     # 520: v tile width incl. ones columns

    nc = bacc.Bacc("TRN2", target_bir_lowering=False, num_devices=n_cores)

    xqT = nc.dram_tensor("xqT", [D, S], bf16, kind="ExternalInput")
    xkT = nc.dram_tensor("xkT", [D, S], bf16, kind="ExternalInput")
    xvT = nc.dram_tensor("xvT", [D, S], bf16, kind="ExternalInput")
    wqT = nc.dram_tensor("wqT", [D, DL], bf16, kind="ExternalInput")
    wkT = nc.dram_tensor("wkT", [D, DL], bf16, kind="ExternalInput")
    wvT = nc.dram_tensor("wvT", [D, DL], bf16, kind="ExternalInput")
    woT = nc.dram_tensor("woT", [DL, D], bf16, kind="ExternalInput")
    bqd = nc.dram_tensor("bq", [DL, 1], f32, kind="ExternalInput")
    bkd = nc.dram_tensor("bk", [DL, 1], f32, kind="ExternalInput")
    y = nc.dram_tensor("y", [S, D], f32, kind="ExternalOutput")

    def mm(out, lhsT, rhs, start, stop):
        nc.tensor.matmul(out, lhsT=lhsT, rhs=rhs, start=start, stop=stop)

    with tile.TileContext(nc) as tc, ExitStack() as top:
        top.enter_context(
            nc.allow_low_precision(
                reason="attention path in bf16; PSUM accumulation stays fp32"
            )
        )
        persist = top.enter_context(tc.tile_pool(name="persist", bufs=1))
        qt = [persist.tile([P, S], bf16, tag=f"qt{i}", name=f"qt{i}") for i in range(NDT)]
        kt = [persist.tile([P, S], bf16, tag=f"kt{i}", name=f"kt{i}") for i in range(NDT)]
        vt = [persist.tile([P, VW], bf16, tag=f"vt{i}", name=f"vt{i}") for i in range(ST)]
        oaT = [persist.tile([P, S], bf16, tag=f"oaT{i}", name=f"oaT{i}") for i in range(NDT)]
        ident = persist.tile([P, P], f32, tag="ident", name="ident")
        bq_t = persist.tile([P, NDT], f32, tag="bq", name="bq")
        bk_t = persist.tile([P, NDT], f32, tag="bk", name="bk")

        masks.make_identity(nc, ident[:])
        warm = persist.tile([P, P], bf16, tag="warm", name="warm")
        nc.vector.memset(warm[:], 0.0)
        # vt ones-columns are memset inside slot 0 (below) so the head's
        # K/Q projection evacuations reach the DVE queue first.

        # PSUM: scores/exp 2x[128,1024] (4 banks) + attn@V accum 2x[128,260]
        # (2 banks) + generic matmul 2x[128,512] (2 banks) = 8 banks.
        spool = top.enter_context(tc.tile_pool(name="spool", bufs=2, space="PSUM"))
        acpool = top.enter_context(tc.tile_pool(name="acpool", bufs=2, space="PSUM"))
        gpool = top.enter_context(tc.tile_pool(name="gpool", bufs=2, space="PSUM"))

        # weight/x pools for Q (live through all Q chunks); wide layouts:
        # w tiles hold all ET contraction blocks side by side (one DMA each).
        wqp = top.enter_context(tc.tile_pool(name="wqp", bufs=1))
        wq = wqp.tile([P, ET * DL], bf16, tag="wq", name="wq")
        xqp = top.enter_context(tc.tile_pool(name="xqp", bufs=1))

        # long-lived attention pools (opened before any scoped pool so that
        # mid-stream pool closes stay LIFO)
        etsp = top.enter_context(tc.tile_pool(name="etsp", bufs=2))
        oasp = top.enter_context(tc.tile_pool(name="oasp", bufs=4))
        yvp = top.enter_context(tc.tile_pool(name="yvp", bufs=2))
        rcp = top.enter_context(tc.tile_pool(name="rcp", bufs=4))

        for _rep in range(repeats):
            # ---------------- pools for K and Q chunk streams -------------
            vstate = {}
            s3 = ExitStack()
            s2 = ExitStack()
            vxa = s2.enter_context(tc.tile_pool(name="vxa", bufs=1))
            vstate["wv"] = vxa.tile([P, ET * DL], bf16, tag="wv", name="wv")
            vstate["xv0"] = vxa.tile([P, ET * (S // 2)], bf16, tag="xv0",
                                     name="xv0")
            s1 = ExitStack()
            kx = s1.enter_context(tc.tile_pool(name="kx", bufs=2))
            wkp = s1.enter_context(tc.tile_pool(name="wkp", bufs=1))
            wk = wkp.tile([P, ET * DL], bf16, tag="wk", name="wk")

            def load_xk(c, eng=None):
                xkc = kx.tile([P, ET * QC], bf16, tag="xk", name="xk")
                (eng or nc.sync).dma_start(
                    out=xkc[:].rearrange("p (e s) -> p e s", e=ET),
                    in_=xkT[:, c * QC : (c + 1) * QC].rearrange(
                        "(e p) s -> p e s", p=P),
                )
                vstate["xk"] = xkc

            def load_xq(c, eng):
                xqc = xqp.tile([P, ET * QC], bf16, tag="xq", name="xq")
                eng.dma_start(
                    out=xqc[:].rearrange("p (e s) -> p e s", e=ET),
                    in_=xqT[:, c * QC : (c + 1) * QC].rearrange(
                        "(e p) s -> p e s", p=P),
                )
                vstate["xq"] = xqc

            def proj_piece(c, dch, w, xkey, bias, out_tiles):
                """One (chunk, dl-tile) projection: out (dl 128, s 512) + bias."""
                x = vstate[xkey]
                gp = gpool.tile([P, QC], f32, tag="gp", name="gp")
                for e in range(ET):
                    mm(gp[:], w[:, e * DL + dch * P : e * DL + (dch + 1) * P],
                       x[:, e * QC : (e + 1) * QC], e == 0, e == ET - 1)
                nc.vector.tensor_scalar_add(
                    out_tiles[dch][:, c * QC : (c + 1) * QC], gp[:],
                    bias[:, dch : dch + 1]
                )

            # --- V pools: wv + the first s-half of xv preload alongside the
            # K pool (slot 0); the second s-half lands in the space the K pool
            # frees. V projection runs head-half-major so attn@V for heads 0-3
            # unblocks as early as possible.
            SH = S // 2

            def load_wv():
                nc.sync.dma_start(
                    out=vstate["wv"][:].rearrange("p (e d) -> p e d", e=ET),
                    in_=wvT[:].rearrange("(e p) d -> p e d", p=P),
                )

            def load_xv0():
                nc.sync.dma_start(
                    out=vstate["xv0"][:].rearrange("p (e s) -> p e s", e=ET),
                    in_=xvT[:, 0:SH].rearrange("(e p) s -> p e s", p=P),
                )

            def open_vx():
                vxb = s2.enter_context(tc.tile_pool(name="vxb", bufs=1))
                vstate["xv1"] = vxb.tile([P, ET * SH], bf16, tag="xv1", name="xv1")
                nc.sync.dma_start(
                    out=vstate["xv1"][:].rearrange("p (e s) -> p e s", e=ET),
                    in_=xvT[:, SH:S].rearrange("(e p) s -> p e s", p=P),
                )

            def vproj_piece(st, qtr):
                """V projection for (s-tile st, head pair qtr): 2 heads.
                Quarters 0-1 run before attn@V starts, so odd s-tiles borrow
                the idle attn@V accumulator banks for deeper pipelining."""
                Q4 = DL // 4
                if qtr < 2 and st % 2 == 1:
                    gp = acpool.tile([P, 512], f32, tag="ac", name="ac")
                else:
                    gp = gpool.tile([P, QC], f32, tag="gp", name="gp")
                wv = vstate["wv"]
                xv = vstate["xv0"] if st < ST // 2 else vstate["xv1"]
                stl = st % (ST // 2)
                for e in range(ET):
                    mm(gp[:, 0:Q4],
                       xv[:, e * SH + stl * P : e * SH + (stl + 1) * P],
                       wv[:, e * DL + qtr * Q4 : e * DL + (qtr + 1) * Q4],
                       e == 0, e == ET - 1)
                nc.vector.tensor_copy(
                    vt[st][:].rearrange("p (h c) -> p h c", h=H)
                    [:, qtr * 2 : (qtr + 1) * 2, 0:DK],
                    gp[:, 0:Q4].rearrange("p (h c) -> p h c", h=2),
                )

            state = {
                "ets": {},    # (qt_idx, hp, kg) -> tile  (live window)
                "ac": {},     # (qt_idx, hgrp) -> tile
                "oas": {},    # qt_idx -> tile
                "wo": None,
            }

            def sc_use(qi, hp, kg):
                """Scores + exp for (q-tile qi, head pair hp, k-group kg)."""
                sc = spool.tile([P, 2 * KG * P], f32, tag="sc", name="sc")
                for hloc in range(2):
                    h = 2 * hp + hloc
                    r0 = hloc * DK
                    for ktl in range(KG):
                        ki = kg * KG + ktl
                        mm(
                            sc[:, hloc * KG * P + ktl * P : hloc * KG * P + (ktl + 1) * P],
                            kt[hp][r0 : r0 + DK, ki * P : (ki + 1) * P],
                            qt[hp][r0 : r0 + DK, qi * P : (qi + 1) * P],
                            True,
                            True,
                        )
                et = etsp.tile([P, 2 * KG * P], bf16, tag=f"et{hp}_{kg}",
                               name=f"et{hp}_{kg}")
                nc.scalar.activation(et[:], sc[:], Exp)
                state["ets"][(qi, hp, kg)] = et

            def attnv_piece(qi, h, kg):
                """attn@V for (q-tile qi, head h, k-group kg): 4 x 65-row mms."""
                hgrp, hidx = divmod(h, 4)
                key = (qi, hgrp)
                if key not in state["ac"]:
                    # padded to a full 2KB bank; cols 0-259 used (4 heads x 65)
                    state["ac"][key] = acpool.tile([P, 512], f32, tag="ac", name="ac")
                ac = state["ac"][key]
                et = state["ets"][(qi, h // 2, kg)]
                hloc = h % 2
                for ktl in range(KG):
                    ki = kg * KG + ktl
                    mm(
                        ac[:, hidx * (DK + 1) : (hidx + 1) * (DK + 1)],
                        et[:, hloc * KG * P + ktl * P : hloc * KG * P + (ktl + 1) * P],
                        vt[ki][:, h * (DK + 1) : (h + 1) * (DK + 1)],
                        ki == 0,
                        ki == ST - 1,
                    )
                if hloc == 1:
                    del state["ets"][(qi, h // 2, kg)]

            def norm_piece(qi, hgrp):
                """Normalize 4 heads: oa_s[:, hgrp*256:+256] = num * (1/Z)."""
                if qi not in state["oas"]:
                    state["oas"][qi] = oasp.tile([P, DL], f32, tag="oas", name="oas")
                oas = state["oas"][qi]
                ac = state["ac"].pop((qi, hgrp))
                acr = ac[:, 0 : 4 * (DK + 1)].rearrange("p (h c) -> p h c", h=4)
                rc = rcp.tile([P, 4], f32, tag="rc", name="rc")
                nc.vector.reciprocal(rc[:], acr[:, :, DK])
                for hh in range(4):
                    nc.vector.tensor_scalar_mul(
                        oas[:, hgrp * 4 * DK + hh * DK : hgrp * 4 * DK + (hh + 1) * DK],
                        acr[:, hh, 0:DK],
                        rc[:, hh : hh + 1],
                    )

            def transp_qt(qi, half=None):
                """Transpose oa_s (q, dl) -> oaT (dl, q) for one q-tile.
                half=0 covers dl-tiles 0-1 (ready after the hgrp-0 norm),
                half=1 covers 2-3; None does both."""
                rng = (range(NDT) if half is None
                       else range(2 * half, 2 * half + 2))
                sc = spool.tile([P, 2 * KG * P], f32, tag="sc", name="sc")
                oas = state["oas"][qi]
                for dlb in rng:
                    nc.tensor.transpose(
                        sc[:, dlb * P : (dlb + 1) * P],
                        oas[:, dlb * P : (dlb + 1) * P],
                        ident[:],
                    )
                for dlb in rng:
                    nc.vector.tensor_copy(
                        oaT[dlb][:, qi * P : (qi + 1) * P], sc[:, dlb * P : (dlb + 1) * P]
                    )
                if half != 0:
                    del state["oas"][qi]

            def load_wo():
                wop = s3.enter_context(tc.tile_pool(name="wop", bufs=1))
                wo = wop.tile([P, NDT * D], bf16, tag="wo", name="wo")
                nc.sync.dma_start(
                    out=wo[:].rearrange("p (i d) -> p i d", i=NDT),
                    in_=woT[:].rearrange("(i p) d -> p i d", p=P),
                )
                state["wo"] = wo

            def c_piece(st, fc):
                """Output projection for (s-tile st, f-chunk fc)."""
                wo = state["wo"]
                gp = gpool.tile([P, QC], f32, tag="gp", name="gp")
                for dl in range(NDT):
                    mm(gp[:], oaT[dl][:, st * P : (st + 1) * P],
                       wo[:, dl * D + fc * QC : dl * D + (fc + 1) * QC],
                       dl == 0, dl == NDT - 1)
                yv = yvp.tile([P, QC], f32, tag="yv", name="yv")
                nc.vector.tensor_copy(yv[:], gp[:])
                nc.sync.dma_start(
                    out=y[st * P : (st + 1) * P, fc * QC : (fc + 1) * QC], in_=yv[:]
                )

            # ------------- interleaved emission ---------------------------
            # One FIFO of side pieces per slot, drained between score+exp
            # groups under a PE-lead budget, force-drained at slot end (and at
            # the MID marker before the hp2/hp3 half). Estimated PE ns/piece.
            EXP_NS, SC_NS = 1090.0, 430.0
            COST = {}

            def piece_cost(p):
                fn = p[0]
                if fn == proj_piece:
                    return 1750.0
                if fn == qproj_rest:
                    return 1350.0
                if fn == vproj_piece:
                    return 450.0
                if fn == attnv_piece:
                    return 160.0
                if fn == transp_qt:
                    return 520.0
                if fn == c_piece:
                    return 900.0
                return 0.0

            def run_piece(p):
                if p[0] == "loadxk":
                    load_xk(p[1])
                elif p[0] == "loadxq":
                    load_xq(p[1], nc.sync)
                elif p[0] == "loadwo":
                    load_wo()
                elif p[0] == "closes1":
                    s1.close()
                elif p[0] == "openvx":
                    open_vx()
                elif p[0] == "memset":
                    nc.vector.memset(vt[p[1]][:], 1.0)
                elif p[0] == "loadwv":
                    load_wv()
                elif p[0] == "loadxv0":
                    load_xv0()
                elif p[0] == "closes2":
                    s2.close()
                elif p[0] == "HPM":
                    pass
                else:
                    p[0](*p[1:])

            def attnv_hp(qi, hp):
                out = []
                for h in (2 * hp, 2 * hp + 1):
                    for kg in range(NKG):
                        out.append((attnv_piece, qi, h, kg))
                return out

            slot_inter = [[] for _ in range(ST)]
            slot_markers = [set() for _ in range(ST)]
            slot_inter[0] += [("memset", i) for i in range(ST)]
            slot_inter[0] += [("loadwv",), ("loadxv0",)]
            slot_inter[0] += [(vproj_piece, st, 0) for st in range(ST // 2)]
            slot_inter[1] += [(vproj_piece, st, 0) for st in range(ST // 2, ST)]
            slot_inter[1] += attnv_hp(0, 0)
            slot_inter[1] += [(vproj_piece, st, 1) for st in range(ST)]
            slot_inter[1] += attnv_hp(0, 1) + [(norm_piece, 0, 0)]
            slot_inter[2] += [(vproj_piece, st, 2) for st in range(ST)]
            slot_inter[2] += attnv_hp(0, 2) + [("HPM", 2)]
            slot_inter[2] += [(vproj_piece, st, 3) for st in range(ST)]
            slot_inter[2] += attnv_hp(0, 3) + [(norm_piece, 0, 1), ("HPM", 3)]
            slot_inter[2] += [("closes2",), ("loadwo",)]
            slot_markers[2] = {2, 3}
            slot_inter[3] += (attnv_hp(1, 0) + [("HPM", 0)]
                              + attnv_hp(1, 1) + [(norm_piece, 1, 0), ("HPM", 1)]
                              + attnv_hp(1, 2) + [("HPM", 2)]
                              + attnv_hp(1, 3) + [(norm_piece, 1, 1), ("HPM", 3)])
            slot_markers[3] = {0, 1, 2, 3}
            slot_inter[3] += [(transp_qt, 0), (transp_qt, 1)]
            for j in range(3, ST):
                slot_inter[j] += (attnv_hp(j - 1, 0) + attnv_hp(j - 1, 1)
                                  + [(norm_piece, j - 1, 0)]
                                  + attnv_hp(j - 1, 2) + attnv_hp(j - 1, 3)
                                  + [(norm_piece, j - 1, 1)])
                if j - 1 >= 2:
                    slot_inter[j].append((transp_qt, j - 1))
            # out-projection: early s-tiles deferred to late slots (the early
            # slots carry the V/K/Q overload), the rest two slots after their
            # transpose.
            for st in range(0, 5):
                slot_inter[11 + st] += [(c_piece, st, 0), (c_piece, st, 1)]
            for st in range(5, 14):
                slot_inter[st + 2] += [(c_piece, st, 0), (c_piece, st, 1)]
            # Q chunk c: DMA early, project each dl-tile just before the
            # first score group of slot 4c that needs it.
            def qproj_mini(c, dch):
                """Q projection for q-tile 4c only (slot 4c's own columns)."""
                x = vstate["xq"]
                gp = gpool.tile([P, QC], f32, tag="gp", name="gp")
                for e in range(ET):
                    mm(gp[:, 0:P],
                       wq[:, e * DL + dch * P : e * DL + (dch + 1) * P],
                       x[:, e * QC : e * QC + P], e == 0, e == ET - 1)
                nc.vector.tensor_scalar_add(
                    qt[dch][:, 4 * c * P : (4 * c + 1) * P], gp[:, 0:P],
                    bq_t[:, dch : dch + 1])

            def qproj_rest(c, dch):
                """Q projection for q-tiles 4c+1..4c+3 (needed next slot)."""
                x = vstate["xq"]
                gp = gpool.tile([P, QC], f32, tag="gp", name="gp")
                for e in range(ET):
                    mm(gp[:, 0 : 3 * P],
                       wq[:, e * DL + dch * P : e * DL + (dch + 1) * P],
                       x[:, e * QC + P : (e + 1) * QC], e == 0, e == ET - 1)
                nc.vector.tensor_scalar_add(
                    qt[dch][:, (4 * c + 1) * P : (4 * c + 4) * P],
                    gp[:, 0 : 3 * P], bq_t[:, dch : dch + 1])

            slot_hp = [[[] for _ in range(HP)] for _ in range(ST)]
            for c in (1, 2, 3):
                slot_inter[4 * c - 2].append(("loadxq", c))
                for dch in range(NDT):
                    slot_hp[4 * c][dch].append((qproj_mini, c, dch))
                slot_inter[4 * c] += [(qproj_rest, c, dch)
                                      for dch in range(NDT)]

            # ---------------- head + slot loop ----------------------------
            nc.scalar.dma_start(
                out=wk[:].rearrange("p (e d) -> p e d", e=ET),
                in_=wkT[:].rearrange("(e p) d -> p e d", p=P))
            load_xk(0)
            load_xq(0, nc.scalar)
            nc.sync.dma_start(
                out=wq[:].rearrange("p (e d) -> p e d", e=ET),
                in_=wqT[:].rearrange("(e p) d -> p e d", p=P))
            nc.sync.dma_start(out=bk_t[:].rearrange("p i -> p i ()"),
                              in_=bkd[:].rearrange("(i p) o -> p i o", p=P))
            nc.sync.dma_start(out=bq_t[:].rearrange("p i -> p i ()"),
                              in_=bqd[:].rearrange("(i p) o -> p i o", p=P))
            # PE clock warm-up while the first loads are in flight: harmless
            # zero matmuls keep the PE busy so it reaches full p-state before
            # the first projection.
            for wu in range(26):
                wsc = spool.tile([P, 2 * KG * P], f32, tag="sc", name="sc")
                for j in range(4):
                    mm(wsc[:, j * P : (j + 1) * P], warm[:], warm[:], True, True)

            budget = [0.0]

            def drain(inter, force=False, to_marker=None):
                while inter:
                    if (not force and to_marker is None
                            and piece_cost(inter[0]) > budget[0]):
                        return
                    p = inter.popleft()
                    run_piece(p)
                    budget[0] -= piece_cost(p)
                    if to_marker is not None and p == ("HPM", to_marker):
                        return

            def do_sc(qi, hp, kg, inter):
                sc_use(qi, hp, kg)
                budget[0] = min(budget[0] + (EXP_NS - SC_NS), 4000.0)
                drain(inter)

            # slot 0: interleave K/Q chunk-0 projections with the first score
            # groups (head pair hp becomes ready as soon as dl-tile hp
            # projects), then kg 1-3 as K chunks 1-3 land.
            inter0 = deque(slot_inter[0])
            for dch in range(NDT):
                proj_piece(0, dch, wk, "xk", bk_t, kt)
                proj_piece(0, dch, wq, "xq", bq_t, qt)
                sc_use(0, dch, 0)
            xk_next = {}
            load_xk(1)
            xk_next[1] = vstate["xk"]
            for kg in range(1, NKG):
                vstate["xk"] = xk_next[kg]
                if kg + 1 < NKG:
                    load_xk(kg + 1)
                    xk_next[kg + 1] = vstate["xk"]
                    vstate["xk"] = xk_next[kg]
                for dch in range(NDT):
                    proj_piece(kg, dch, wk, "xk", bk_t, kt)
                for hp in range(HP):
                    do_sc(0, hp, kg, inter0)
            drain(inter0, force=True)
            s1.close()
            open_vx()

            for qi in range(1, ST):
                inter = deque(slot_inter[qi])
                for hp in range(HP):
                    if hp in slot_markers[qi]:
                        drain(inter, to_marker=hp)
                    for p in slot_hp[qi][hp]:
                        run_piece(p)
                    for kg in range(NKG):
                        if hp == HP - 1 and kg == 0:
                            # flush leftovers while the previous exp groups
                            # still cover the scalar engine
                            drain(inter, force=True)
                        do_sc(qi, hp, kg, inter)
                drain(inter, force=True)

            # ---------------- tail ----------------------------------------
            for piece in (attnv_hp(ST - 1, 0) + attnv_hp(ST - 1, 1)
                          + [(norm_piece, ST - 1, 0)]):
                run_piece(piece)
            transp_qt(ST - 1, half=0)
            for piece in (attnv_hp(ST - 1, 2) + attnv_hp(ST - 1, 3)
                          + [(norm_piece, ST - 1, 1)]):
                run_piece(piece)
            transp_qt(ST - 1, half=1)
            for st in range(ST - 2, ST):
                for fc in range(D // QC):
                    c_piece(st, fc)
            s3.close()

    nc.compile()
    return nc


def make_in_maps(query, key, value, Wq, bq, Wk, bk, Wv, bv, n_cores=8,
                 mm_dtype="float32r"):
    """Host-side sharding: slice weights Megatron-style, transpose activations."""
    import ml_dtypes

    bft = ml_dtypes.bfloat16
    q = np.asarray(query, dtype=np.float32)
    k = np.asarray(key, dtype=np.float32)
    v = np.asarray(value, dtype=np.float32)
    Wq = np.asarray(Wq, dtype=np.float32)
    Wk = np.asarray(Wk, dtype=np.float32)
    Wv = np.asarray(Wv, dtype=np.float32)
    bq = np.asarray(bq, dtype=np.float32)
    bk = np.asarray(bk, dtype=np.float32)
    D = Wq.shape[0]
    DL = D // (n_cores // q.shape[0])
    scale = 1.0 / np.sqrt(np.float32(DK))
    in_maps = []
    for c in range(n_cores):
        b, g = divmod(c, n_cores // q.shape[0])
        sl = slice(DL * g, DL * (g + 1))
        in_maps.append(
            {
                "xqT": np.ascontiguousarray(q[b].T).astype(bft),
                "xkT": np.ascontiguousarray(k[b].T).astype(bft),
                "xvT": np.ascontiguousarray(v[b].T).astype(bft),
                "wqT": (np.ascontiguousarray(Wq[sl].T) * scale).astype(bft),
                "wkT": np.ascontiguousarray(Wk[sl].T).astype(bft),
                "wvT": np.ascontiguousarray(Wv[sl].T).astype(bft),
                "bq": np.ascontiguousarray((bq[sl] * scale).reshape(DL, 1)),
                "bk": np.ascontiguousarray(bk[sl].reshape(DL, 1)),
            }
        )
    return in_maps


def add_wo_maps(in_maps, Wo, n_cores=8, n_batch=4, mm_dtype="float32r"):
    import ml_dtypes

    Wo = np.asarray(Wo, dtype=np.float32)
    D = Wo.shape[0]
    DL = D // (n_cores // n_batch)
    for c in range(n_cores):
        _, g = divmod(c, n_cores // n_batch)
        sl = slice(DL * g, DL * (g + 1))
        in_maps[c]["woT"] = np.ascontiguousarray(Wo[:, sl].T).astype(ml_dtypes.bfloat16)
    return in_maps


MM_DTYPE = "float32r"


def kernel(query, key, value, Wq, bq, Wk, bk, Wv, bv, Wo, bo):
    if "nc" not in _CACHE:
        _CACHE["nc"] = build_nc(mm_dtype=MM_DTYPE)
    nc = _CACHE["nc"]
    n_cores = 8
    in_maps = make_in_maps(
        query, key, value, Wq, bq, Wk, bk, Wv, bv, n_cores, MM_DTYPE
    )
    add_wo_maps(in_maps, Wo, n_cores, np.asarray(query).shape[0], MM_DTYPE)
    res = run_bass_kernel_spmd(nc, in_maps, list(range(n_cores)))
    ys = [res.results[c]["y"] for c in range(n_cores)]
    bo = np.asarray(bo, dtype=np.float32)
    bv = np.asarray(bv, dtype=np.float32)
    Wo = np.asarray(Wo, dtype=np.float32)
    const = bo + bv @ Wo.T
    out = np.stack([ys[2 * b] + ys[2 * b + 1] for b in range(4)]) + const[None, None, :]
    return out.astype(np.float32)


# revision 83
# speedup vs baseline: 1.0427x; 1.0313x over previous
"""Trainium2 Bass kernel for nn_MultiHeadAttention_37838661877847.

Full-input contract: kernel(**inputs) takes the complete tensors and returns
the complete output. Internally shards across 8 NeuronCores:
  core c -> batch b = c // 2, head-group g = c % 2 (8 heads, 512 dims each).
Each core computes Q/K/V projections for its (batch, head-group) slice
(column-parallel weights), attention for its 8 heads, and a partial output
projection (row-parallel Wo). Host sums core pairs and adds bo + bv @ Wo.T
(the V bias commutes through softmax-weighted averaging, so it is folded
into the output-projection bias on the host).

Engine-level design (per core), built as ONE interleaved instruction stream
so the scalar engine's softmax-exp (the 266us serial floor: 33.5M exps at
1 elem/lane/cycle) overlaps the tensor engine work (281us):

  - Q_T/K_T stored (dl, s) in bf16; scores come out (k, q) per 128-k tile.
  - exp groups of [128, 1024] PSUM (4 score blocks: 2 heads x ... see sc
    layout below) -> ets tiles in bf16.
  - attn@V is FLIPPED: out (q, dk+1) accumulating over k with the exp tile
    as the stationary operand -> 65-row bf16 matmuls, half the PE rows of
    the (dk+1, q) orientation. V is augmented with a ones column per head so
    the softmax denominator Z lands in column 64; normalization is then a
    per-partition reciprocal+scale on DVE.
  - normalized output (q, dl) is transposed back to (dl, q) via PE-transpose
    through spare score-PSUM space, then the output projection streams wo.
  - emission interleaves projections / attn@V / transposes / out-proj between
    score+exp groups so the scalar engine rarely starves.

mm dtypes: x and w_qk in f32r/bf16 keep projections+scores accurate; the
attention path (probs, V, attn-out, Wo) runs in bf16 (PSUM accumulation is
fp32 throughout).
"""

import sys

sys.path.insert(0, "/opt/trn_rl_repo")

from collections import deque
from contextlib import ExitStack

import numpy as np

import concourse.bass as bass  # noqa: F401
import concourse.tile as tile
from concourse import bacc, masks, mybir
from concourse.bass_utils import run_bass_kernel_spmd

P = 128
DK = 64  # head dim

_CACHE = {}


def build_nc(S=2048, D=1024, DL=512, mm_dtype="float32r", n_cores=8,
             repeats=1, phases="ABC"):
    """Build + compile the per-core Bass program (same program on all cores).

    repeats exists only for timing experiments; production uses the default.
    mm_dtype/phases are accepted for test-harness compatibility (the kernel
    uses a fixed mixed f32r/bf16 precision scheme).
    """
    f32 = mybir.dt.float32
    f32r = mybir.dt.float32r
    bf16 = mybir.dt.bfloat16
    Exp = mybir.ActivationFunctionType.Exp

    H = DL // DK          # 8 local heads
    HP = H // 2           # 4 head pairs (one pair per 128-row q/k tile)
    ET = D // P           # 8 contraction tiles for projections
    ST = S // P           # 16 k tiles (and q tiles)
    NDT = DL // P         # 4 dl tiles
    QC = 512              # projection s-chunk
    NQ = S // QC          # 4
    KG = 4                # k-tiles per exp group
    NKG = ST // KG        # 4
    VW = H * (DK + 1)     # 520: v tile width incl. ones columns

    nc = bacc.Bacc("TRN2", target_bir_lowering=False, num_devices=n_cores)

    xqT = nc.dram_tensor("xqT", [D, S], bf16, kind="ExternalInput")
    xkT = nc.dram_tensor("xkT", [D, S], bf16, kind="ExternalInput")
    xvT = nc.dram_tensor("xvT", [D, S], bf16, kind="ExternalInput")
    wqT = nc.dram_tensor("wqT", [D, DL], bf16, kind="ExternalInput")
    wkT = nc.dram_tensor("wkT", [D, DL], bf16, kind="ExternalInput")
    wvT = nc.dram_tensor("wvT", [D, DL], bf16, kind="ExternalInput")
    woT = nc.dram_tensor("woT", [DL, D], bf16, kind="ExternalInput")
    bqd = nc.dram_tensor("bq", [DL, 1], f32, kind="ExternalInput")
    bkd = nc.dram_tensor("bk", [DL, 1], f32, kind="ExternalInput")
    y = nc.dram_tensor("y", [S, D], f32, kind="ExternalOutput")

    def mm(out, lhsT, rhs, start, stop):
        nc.tensor.matmul(out, lhsT=lhsT, rhs=rhs, start=start, stop=stop)

    with tile.TileContext(nc) as tc, ExitStack() as top:
        top.enter_context(
            nc.allow_low_precision(
                reason="attention path in bf16; PSUM accumulation stays fp32"
            )
        )
        persist = top.enter_context(tc.tile_pool(name="persist", bufs=1))
        qt = [persist.tile([P, S], bf16, tag=f"qt{i}", name=f"qt{i}") for i in range(NDT)]
        kt = [persist.tile([P, S], bf16, tag=f"kt{i}", name=f"kt{i}") for i in range(NDT)]
        vt = [persist.tile([P, VW], bf16, tag=f"vt{i}", name=f"vt{i}") for i in range(ST)]
        oaT = [persist.tile([P, S], bf16, tag=f"oaT{i}", name=f"oaT{i}") for i in range(NDT)]
        ident = persist.tile([P, P], f32, tag="ident", name="ident")
        bq_t = persist.tile([P, NDT], f32, tag="bq", name="bq")
        bk_t = persist.tile([P, NDT], f32, tag="bk", name="bk")

        masks.make_identity(nc, ident[:])
        warm = persist.tile([P, P], bf16, tag="warm", name="warm")
        nc.vector.memset(warm[:], 0.0)
        # vt ones-columns are memset inside slot 0 (below) so the head's
        # K/Q projection evacuations reach the DVE queue first.

        # PSUM: scores/exp 2x[128,1024] (4 banks) + attn@V accum 2x[128,260]
        # (2 banks) + generic matmul 2x[128,512] (2 banks) = 8 banks.
        spool = top.enter_context(tc.tile_pool(name="spool", bufs=2, space="PSUM"))
        acpool = top.enter_context(tc.tile_pool(name="acpool", bufs=2, space="PSUM"))
        gpool = top.enter_context(tc.tile_pool(name="gpool", bufs=2, space="PSUM"))

        # weight/x pools for Q (live through all Q chunks); wide layouts:
        # w tiles hold all ET contraction blocks side by side (one DMA each).
        wqp = top.enter_context(tc.tile_pool(name="wqp", bufs=1))
        wq = wqp.tile([P, ET * DL], bf16, tag="wq", name="wq")
        xqp = top.enter_context(tc.tile_pool(name="xqp", bufs=1))

        # long-lived attention pools (opened before any scoped pool so that
        # mid-stream pool closes stay LIFO)
        etsp = top.enter_context(tc.tile_pool(name="etsp", bufs=2))
        oasp = top.enter_context(tc.tile_pool(name="oasp", bufs=4))
        yvp = top.enter_context(tc.tile_pool(name="yvp", bufs=2))
        rcp = top.enter_context(tc.tile_pool(name="rcp", bufs=4))

        for _rep in range(repeats):
            # ---------------- pools for K and Q chunk streams -------------
            vstate = {}
            s3 = ExitStack()
            s2 = ExitStack()
            vxa = s2.enter_context(tc.tile_pool(name="vxa", bufs=1))
            vstate["wv"] = vxa.tile([P, ET * DL], bf16, tag="wv", name="wv")
            vstate["xv0"] = vxa.tile([P, ET * (S // 2)], bf16, tag="xv0",
                                     name="xv0")
            s1 = ExitStack()
            kx = s1.enter_context(tc.tile_pool(name="kx", bufs=2))
            wkp = s1.enter_context(tc.tile_pool(name="wkp", bufs=1))
            wk = wkp.tile([P, ET * DL], bf16, tag="wk", name="wk")

            def load_xk(c, eng=None):
                xkc = kx.tile([P, ET * QC], bf16, tag="xk", name="xk")
                (eng or nc.sync).dma_start(
                    out=xkc[:].rearrange("p (e s) -> p e s", e=ET),
                    in_=xkT[:, c * QC : (c + 1) * QC].rearrange(
                        "(e p) s -> p e s", p=P),
                )
                vstate["xk"] = xkc

            def load_xq(c, eng):
                xqc = xqp.tile([P, ET * QC], bf16, tag="xq", name="xq")
                eng.dma_start(
                    out=xqc[:].rearrange("p (e s) -> p e s", e=ET),
                    in_=xqT[:, c * QC : (c + 1) * QC].rearrange(
                        "(e p) s -> p e s", p=P),
                )
                vstate["xq"] = xqc

            def proj_piece(c, dch, w, xkey, bias, out_tiles):
                """One (chunk, dl-tile) projection: out (dl 128, s 512) + bias."""
                x = vstate[xkey]
                gp = gpool.tile([P, QC], f32, tag="gp", name="gp")
                for e in range(ET):
                    mm(gp[:], w[:, e * DL + dch * P : e * DL + (dch + 1) * P],
                       x[:, e * QC : (e + 1) * QC], e == 0, e == ET - 1)
                nc.vector.tensor_scalar_add(
                    out_tiles[dch][:, c * QC : (c + 1) * QC], gp[:],
                    bias[:, dch : dch + 1]
                )

            # --- V pools: wv + the first s-half of xv preload alongside the
            # K pool (slot 0); the second s-half lands in the space the K pool
            # frees. V projection runs head-half-major so attn@V for heads 0-3
            # unblocks as early as possible.
            SH = S // 2

            def load_wv():
                nc.sync.dma_start(
                    out=vstate["wv"][:].rearrange("p (e d) -> p e d", e=ET),
                    in_=wvT[:].rearrange("(e p) d -> p e d", p=P),
                )

            def load_xv0():
                nc.sync.dma_start(
                    out=vstate["xv0"][:].rearrange("p (e s) -> p e s", e=ET),
                    in_=xvT[:, 0:SH].rearrange("(e p) s -> p e s", p=P),
                )

            def open_vx():
                vxb = s2.enter_context(tc.tile_pool(name="vxb", bufs=1))
                vstate["xv1"] = vxb.tile([P, ET * SH], bf16, tag="xv1", name="xv1")
                nc.sync.dma_start(
                    out=vstate["xv1"][:].rearrange("p (e s) -> p e s", e=ET),
                    in_=xvT[:, SH:S].rearrange("(e p) s -> p e s", p=P),
                )

            def vproj_piece(st, qtr):
                """V projection for (s-tile st, head pair qtr): 2 heads.
                Quarters 0-1 run before attn@V starts, so odd s-tiles borrow
                the idle attn@V accumulator banks for deeper pipelining."""
                Q4 = DL // 4
                if qtr < 2 and st % 2 == 1:
                    gp = acpool.tile([P, 512], f32, tag="ac", name="ac")
                else:
                    gp = gpool.tile([P, QC], f32, tag="gp", name="gp")
                wv = vstate["wv"]
                xv = vstate["xv0"] if st < ST // 2 else vstate["xv1"]
                stl = st % (ST // 2)
                for e in range(ET):
                    mm(gp[:, 0:Q4],
                       xv[:, e * SH + stl * P : e * SH + (stl + 1) * P],
                       wv[:, e * DL + qtr * Q4 : e * DL + (qtr + 1) * Q4],
                       e == 0, e == ET - 1)
                nc.vector.tensor_copy(
                    vt[st][:].rearrange("p (h c) -> p h c", h=H)
                    [:, qtr * 2 : (qtr + 1) * 2, 0:DK],
                    gp[:, 0:Q4].rearrange("p (h c) -> p h c", h=2),
                )

            state = {
                "ets": {},    # (qt_idx, hp, kg) -> tile  (live window)
                "ac": {},     # (qt_idx, hgrp) -> tile
                "oas": {},    # qt_idx -> tile
                "wo": None,
            }

            def sc_use(qi, hp, kg):
                """Scores + exp for (q-tile qi, head pair hp, k-group kg)."""
                sc = spool.tile([P, 2 * KG * P], f32, tag="sc", name="sc")
                for hloc in range(2):
                    h = 2 * hp + hloc
                    r0 = hloc * DK
                    for ktl in range(KG):
                        ki = kg * KG + ktl
                        mm(
                            sc[:, hloc * KG * P + ktl * P : hloc * KG * P + (ktl + 1) * P],
                            kt[hp][r0 : r0 + DK, ki * P : (ki + 1) * P],
                            qt[hp][r0 : r0 + DK, qi * P : (qi + 1) * P],
                            True,
                            True,
                        )
                et = etsp.tile([P, 2 * KG * P], bf16, tag=f"et{hp}_{kg}",
                               name=f"et{hp}_{kg}")
                nc.scalar.activation(et[:], sc[:], Exp)
                state["ets"][(qi, hp, kg)] = et

            def attnv_piece(qi, h, kg):
                """attn@V for (q-tile qi, head h, k-group kg): 4 x 65-row mms."""
                hgrp, hidx = divmod(h, 4)
                key = (qi, hgrp)
                if key not in state["ac"]:
                    # padded to a full 2KB bank; cols 0-259 used (4 heads x 65)
                    state["ac"][key] = acpool.tile([P, 512], f32, tag="ac", name="ac")
                ac = state["ac"][key]
                et = state["ets"][(qi, h // 2, kg)]
                hloc = h % 2
                for ktl in range(KG):
                    ki = kg * KG + ktl
                    mm(
                        ac[:, hidx * (DK + 1) : (hidx + 1) * (DK + 1)],
                        et[:, hloc * KG * P + ktl * P : hloc * KG * P + (ktl + 1) * P],
                        vt[ki][:, h * (DK + 1) : (h + 1) * (DK + 1)],
                        ki == 0,
                        ki == ST - 1,
                    )
                if hloc == 1:
                    del state["ets"][(qi, h // 2, kg)]

            def norm_piece(qi, hgrp):
                """Normalize 4 heads: oa_s[:, hgrp*256:+256] = num * (1/Z)."""
                if qi not in state["oas"]:
                    state["oas"][qi] = oasp.tile([P, DL], f32, tag="oas", name="oas")
                oas = state["oas"][qi]
                ac = state["ac"].pop((qi, hgrp))
                acr = ac[:, 0 : 4 * (DK + 1)].rearrange("p (h c) -> p h c", h=4)
                rc = rcp.tile([P, 4], f32, tag="rc", name="rc")
                nc.vector.reciprocal(rc[:], acr[:, :, DK])
                for hh in range(4):
                    nc.vector.tensor_scalar_mul(
                        oas[:, hgrp * 4 * DK + hh * DK : hgrp * 4 * DK + (hh + 1) * DK],
                        acr[:, hh, 0:DK],
                        rc[:, hh : hh + 1],
                    )

            def transp_qt(qi, half=None):
                """Transpose oa_s (q, dl) -> oaT (dl, q) for one q-tile.
                half=0 covers dl-tiles 0-1 (ready after the hgrp-0 norm),
                half=1 covers 2-3; None does both."""
                rng = (range(NDT) if half is None
                       else range(2 * half, 2 * half + 2))
                sc = acpool.tile([P, 512], f32, tag="ac", name="ac")
                oas = state["oas"][qi]
                for dlb in rng:
                    nc.tensor.transpose(
                        sc[:, dlb * P : (dlb + 1) * P],
                        oas[:, dlb * P : (dlb + 1) * P],
                        ident[:],
                    )
                for dlb in rng:
                    nc.vector.tensor_copy(
                        oaT[dlb][:, qi * P : (qi + 1) * P], sc[:, dlb * P : (dlb + 1) * P]
                    )
                if half != 0:
                    del state["oas"][qi]

            def load_wo():
                wop = s3.enter_context(tc.tile_pool(name="wop", bufs=1))
                wo = wop.tile([P, NDT * D], bf16, tag="wo", name="wo")
                nc.sync.dma_start(
                    out=wo[:].rearrange("p (i d) -> p i d", i=NDT),
                    in_=woT[:].rearrange("(i p) d -> p i d", p=P),
                )
                state["wo"] = wo

            def c_piece(st, fc):
                """Output projection for (s-tile st, f-chunk fc)."""
                wo = state["wo"]
                gp = gpool.tile([P, QC], f32, tag="gp", name="gp")
                for dl in range(NDT):
                    mm(gp[:], oaT[dl][:, st * P : (st + 1) * P],
                       wo[:, dl * D + fc * QC : dl * D + (fc + 1) * QC],
                       dl == 0, dl == NDT - 1)
                yv = yvp.tile([P, QC], f32, tag="yv", name="yv")
                nc.vector.tensor_copy(yv[:], gp[:])
                nc.sync.dma_start(
                    out=y[st * P : (st + 1) * P, fc * QC : (fc + 1) * QC], in_=yv[:]
                )

            # ------------- interleaved emission ---------------------------
            # One FIFO of side pieces per slot, drained between score+exp
            # groups under a PE-lead budget, force-drained at slot end (and at
            # the MID marker before the hp2/hp3 half). Estimated PE ns/piece.
            EXP_NS, SC_NS = 1090.0, 430.0
            COST = {}

            def piece_cost(p):
                fn = p[0]
                if fn == proj_piece:
                    return 1750.0
                if fn == qproj_rest:
                    return 1350.0
                if fn == vproj_piece:
                    return 450.0
                if fn == attnv_piece:
                    return 160.0
                if fn == transp_qt:
                    return 520.0
                if fn == c_piece:
                    return 900.0
                return 0.0

            def run_piece(p):
                if p[0] == "loadxk":
                    load_xk(p[1])
                elif p[0] == "loadxq":
                    load_xq(p[1], nc.sync)
                elif p[0] == "loadwo":
                    load_wo()
                elif p[0] == "closes1":
                    s1.close()
                elif p[0] == "openvx":
                    open_vx()
                elif p[0] == "memset":
                    nc.vector.memset(vt[p[1]][:], 1.0)
                elif p[0] == "loadwv":
                    load_wv()
                elif p[0] == "loadxv0":
                    load_xv0()
                elif p[0] == "closes2":
                    s2.close()
                elif p[0] == "HPM":
                    pass
                else:
                    p[0](*p[1:])

            def attnv_hp(qi, hp):
                out = []
                for h in (2 * hp, 2 * hp + 1):
                    for kg in range(NKG):
                        out.append((attnv_piece, qi, h, kg))
                return out

            slot_inter = [[] for _ in range(ST)]
            slot_markers = [set() for _ in range(ST)]
            slot_inter[0] += [("memset", i) for i in range(ST)]
            slot_inter[0] += [("loadwv",), ("loadxv0",)]
            slot_inter[0] += [(vproj_piece, st, 0) for st in range(ST // 2)]
            slot_inter[1] += [(vproj_piece, st, 0) for st in range(ST // 2, ST)]
            slot_inter[1] += attnv_hp(0, 0)
            slot_inter[1] += [(vproj_piece, st, 1) for st in range(ST)]
            slot_inter[1] += attnv_hp(0, 1) + [(norm_piece, 0, 0)]
            slot_inter[2] += [(vproj_piece, st, 2) for st in range(ST)]
            slot_inter[2] += attnv_hp(0, 2) + [("HPM", 2)]
            slot_inter[2] += [(vproj_piece, st, 3) for st in range(ST)]
            slot_inter[2] += attnv_hp(0, 3) + [(norm_piece, 0, 1), ("HPM", 3)]
            slot_inter[2] += [("closes2",), ("loadwo",)]
            slot_markers[2] = {2, 3}
            slot_inter[3] += (attnv_hp(1, 0) + [("HPM", 0)]
                              + attnv_hp(1, 1) + [(norm_piece, 1, 0), ("HPM", 1)]
                              + attnv_hp(1, 2) + [("HPM", 2)]
                              + attnv_hp(1, 3) + [(norm_piece, 1, 1), ("HPM", 3)])
            slot_markers[3] = {0, 1, 2, 3}
            slot_inter[3] += [(transp_qt, 0), (transp_qt, 1)]
            for j in range(3, ST):
                slot_inter[j] += (attnv_hp(j - 1, 0) + attnv_hp(j - 1, 1)
                                  + [(norm_piece, j - 1, 0)]
                                  + attnv_hp(j - 1, 2) + attnv_hp(j - 1, 3)
                                  + [(norm_piece, j - 1, 1)])
                if j - 1 >= 2:
                    slot_inter[j].append((transp_qt, j - 1))
            # out-projection: early s-tiles deferred to late slots (the early
            # slots carry the V/K/Q overload), the rest two slots after their
            # transpose.
            for st in range(0, 5):
                slot_inter[11 + st] += [(c_piece, st, 0), (c_piece, st, 1)]
            for st in range(5, 14):
                slot_inter[st + 2] += [(c_piece, st, 0), (c_piece, st, 1)]
            # Q chunk c: DMA early, project each dl-tile just before the
            # first score group of slot 4c that needs it.
            def qproj_mini(c, dch):
                """Q projection for q-tile 4c only (slot 4c's own columns)."""
                x = vstate["xq"]
                gp = gpool.tile([P, QC], f32, tag="gp", name="gp")
                for e in range(ET):
                    mm(gp[:, 0:P],
                       wq[:, e * DL + dch * P : e * DL + (dch + 1) * P],
                       x[:, e * QC : e * QC + P], e == 0, e == ET - 1)
                nc.vector.tensor_scalar_add(
                    qt[dch][:, 4 * c * P : (4 * c + 1) * P], gp[:, 0:P],
                    bq_t[:, dch : dch + 1])

            def qproj_rest(c, dch):
                """Q projection for q-tiles 4c+1..4c+3 (needed next slot)."""
                x = vstate["xq"]
                gp = gpool.tile([P, QC], f32, tag="gp", name="gp")
                for e in range(ET):
                    mm(gp[:, 0 : 3 * P],
                       wq[:, e * DL + dch * P : e * DL + (dch + 1) * P],
                       x[:, e * QC + P : (e + 1) * QC], e == 0, e == ET - 1)
                nc.vector.tensor_scalar_add(
                    qt[dch][:, (4 * c + 1) * P : (4 * c + 4) * P],
                    gp[:, 0 : 3 * P], bq_t[:, dch : dch + 1])

            slot_hp = [[[] for _ in range(HP)] for _ in range(ST)]
            for c in (1, 2, 3):
                slot_inter[4 * c - 2].append(("loadxq", c))
                for dch in range(NDT):
                    slot_hp[4 * c][dch].append((qproj_mini, c, dch))
                slot_inter[4 * c] += [(qproj_rest, c, dch)
                                      for dch in range(NDT)]

            # ---------------- head + slot loop ----------------------------
            nc.scalar.dma_start(
                out=wk[:].rearrange("p (e d) -> p e d", e=ET),
                in_=wkT[:].rearrange("(e p) d -> p e d", p=P))
            load_xk(0)
            load_xq(0, nc.scalar)
            nc.sync.dma_start(
                out=wq[:].rearrange("p (e d) -> p e d", e=ET),
                in_=wqT[:].rearrange("(e p) d -> p e d", p=P))
            nc.sync.dma_start(out=bk_t[:].rearrange("p i -> p i ()"),
                              in_=bkd[:].rearrange("(i p) o -> p i o", p=P))
            nc.sync.dma_start(out=bq_t[:].rearrange("p i -> p i ()"),
                              in_=bqd[:].rearrange("(i p) o -> p i o", p=P))
            # PE clock warm-up while the first loads are in flight: harmless
            # zero matmuls keep the PE busy so it reaches full p-state before
            # the first projection.
            for wu in range(26):
                wsc = spool.tile([P, 2 * KG * P], f32, tag="sc", name="sc")
                for j in range(4):
                    mm(wsc[:, j * P : (j + 1) * P], warm[:], warm[:], True, True)

            budget = [0.0]

            def drain(inter, force=False, to_marker=None):
                while inter:
                    if (not force and to_marker is None
                            and piece_cost(inter[0]) > budget[0]):
                        return
                    p = inter.popleft()
                    run_piece(p)
                    budget[0] -= piece_cost(p)
                    if to_marker is not None and p == ("HPM", to_marker):
                        return

            def do_sc(qi, hp, kg, inter):
                sc_use(qi, hp, kg)
                budget[0] = min(budget[0] + (EXP_NS - SC_NS), 4000.0)
                drain(inter)

            # slot 0: interleave K/Q chunk-0 projections with the first score
            # groups (head pair hp becomes ready as soon as dl-tile hp
            # projects), then kg 1-3 as K chunks 1-3 land.
            inter0 = deque(slot_inter[0])
            for dch in range(NDT):
                proj_piece(0, dch, wk, "xk", bk_t, kt)
                proj_piece(0, dch, wq, "xq", bq_t, qt)
                sc_use(0, dch, 0)
            xk_next = {}
            load_xk(1)
            xk_next[1] = vstate["xk"]
            for kg in range(1, NKG):
                vstate["xk"] = xk_next[kg]
                if kg + 1 < NKG:
                    load_xk(kg + 1)
                    xk_next[kg + 1] = vstate["xk"]
                    vstate["xk"] = xk_next[kg]
                for dch in range(NDT):
                    proj_piece(kg, dch, wk, "xk", bk_t, kt)
                for hp in range(HP):
                    do_sc(0, hp, kg, inter0)
            drain(inter0, force=True)
            s1.close()
            open_vx()

            for qi in range(1, ST):
                inter = deque(slot_inter[qi])
                for hp in range(HP):
                    if hp in slot_markers[qi]:
                        drain(inter, to_marker=hp)
                    for p in slot_hp[qi][hp]:
                        run_piece(p)
                    for kg in range(NKG):
                        if hp == HP - 1 and kg == 0:
                            # flush leftovers while the previous exp groups
                            # still cover the scalar engine
                            drain(inter, force=True)
                        do_sc(qi, hp, kg, inter)
                drain(inter, force=True)

            # ---------------- tail ----------------------------------------
            for piece in (attnv_hp(ST - 1, 0) + attnv_hp(ST - 1, 1)
                          + [(norm_piece, ST - 1, 0)]):
                run_piece(piece)
            transp_qt(ST - 1, half=0)
            for piece in (attnv_hp(ST - 1, 2) + attnv_hp(ST - 1, 3)
                          + [(norm_piece, ST - 1, 1)]):
                run_piece(piece)
            transp_qt(ST - 1, half=1)
            for st in range(ST - 2, ST):
                for fc in range(D // QC):
                    c_piece(st, fc)
            s3.close()

    nc.compile()
    return nc


def make_in_maps(query, key, value, Wq, bq, Wk, bk, Wv, bv, n_cores=8,
                 mm_dtype="float32r"):
    """Host-side sharding: slice weights Megatron-style, transpose activations."""
    import ml_dtypes

    bft = ml_dtypes.bfloat16
    q = np.asarray(query, dtype=np.float32)
    k = np.asarray(key, dtype=np.float32)
    v = np.asarray(value, dtype=np.float32)
    Wq = np.asarray(Wq, dtype=np.float32)
    Wk = np.asarray(Wk, dtype=np.float32)
    Wv = np.asarray(Wv, dtype=np.float32)
    bq = np.asarray(bq, dtype=np.float32)
    bk = np.asarray(bk, dtype=np.float32)
    D = Wq.shape[0]
    DL = D // (n_cores // q.shape[0])
    scale = 1.0 / np.sqrt(np.float32(DK))
    in_maps = []
    for c in range(n_cores):
        b, g = divmod(c, n_cores // q.shape[0])
        sl = slice(DL * g, DL * (g + 1))
        in_maps.append(
            {
                "xqT": np.ascontiguousarray(q[b].T).astype(bft),
                "xkT": np.ascontiguousarray(k[b].T).astype(bft),
                "xvT": np.ascontiguousarray(v[b].T).astype(bft),
                "wqT": (np.ascontiguousarray(Wq[sl].T) * scale).astype(bft),
                "wkT": np.ascontiguousarray(Wk[sl].T).astype(bft),
                "wvT": np.ascontiguousarray(Wv[sl].T).astype(bft),
                "bq": np.ascontiguousarray((bq[sl] * scale).reshape(DL, 1)),
                "bk": np.ascontiguousarray(bk[sl].reshape(DL, 1)),
            }
        )
    return in_maps


def add_wo_maps(in_maps, Wo, n_cores=8, n_batch=4, mm_dtype="float32r"):
    import ml_dtypes

    Wo = np.asarray(Wo, dtype=np.float32)
    D = Wo.shape[0]
    DL = D // (n_cores // n_batch)
    for c in range(n_cores):
        _, g = divmod(c, n_cores // n_batch)
        sl = slice(DL * g, DL * (g + 1))
        in_maps[c]["woT"] = np.ascontiguousarray(Wo[:, sl].T).astype(ml_dtypes.bfloat16)
    return in_maps


MM_DTYPE = "float32r"


def kernel(query, key, value, Wq, bq, Wk, bk, Wv, bv, Wo, bo):
    if "nc" not in _CACHE:
        _CACHE["nc"] = build_nc(mm_dtype=MM_DTYPE)
    nc = _CACHE["nc"]
    n_cores = 8
    in_maps = make_in_maps(
        query, key, value, Wq, bq, Wk, bk, Wv, bv, n_cores, MM_DTYPE
    )
    add_wo_maps(in_maps, Wo, n_cores, np.asarray(query).shape[0], MM_DTYPE)
    res = run_bass_kernel_spmd(nc, in_maps, list(range(n_cores)))
    ys = [res.results[c]["y"] for c in range(n_cores)]
    bo = np.asarray(bo, dtype=np.float32)
    bv = np.asarray(bv, dtype=np.float32)
    Wo = np.asarray(Wo, dtype=np.float32)
    const = bo + bv @ Wo.T
    out = np.stack([ys[2 * b] + ys[2 * b + 1] for b in range(4)]) + const[None, None, :]
    return out.astype(np.float32)


# revision 84
# speedup vs baseline: 1.0432x; 1.0005x over previous
"""Trainium2 Bass kernel for nn_MultiHeadAttention_37838661877847.

Full-input contract: kernel(**inputs) takes the complete tensors and returns
the complete output. Internally shards across 8 NeuronCores:
  core c -> batch b = c // 2, head-group g = c % 2 (8 heads, 512 dims each).
Each core computes Q/K/V projections for its (batch, head-group) slice
(column-parallel weights), attention for its 8 heads, and a partial output
projection (row-parallel Wo). Host sums core pairs and adds bo + bv @ Wo.T
(the V bias commutes through softmax-weighted averaging, so it is folded
into the output-projection bias on the host).

Engine-level design (per core), built as ONE interleaved instruction stream
so the scalar engine's softmax-exp (the 266us serial floor: 33.5M exps at
1 elem/lane/cycle) overlaps the tensor engine work (281us):

  - Q_T/K_T stored (dl, s) in bf16; scores come out (k, q) per 128-k tile.
  - exp groups of [128, 1024] PSUM (4 score blocks: 2 heads x ... see sc
    layout below) -> ets tiles in bf16.
  - attn@V is FLIPPED: out (q, dk+1) accumulating over k with the exp tile
    as the stationary operand -> 65-row bf16 matmuls, half the PE rows of
    the (dk+1, q) orientation. V is augmented with a ones column per head so
    the softmax denominator Z lands in column 64; normalization is then a
    per-partition reciprocal+scale on DVE.
  - normalized output (q, dl) is transposed back to (dl, q) via PE-transpose
    through spare score-PSUM space, then the output projection streams wo.
  - emission interleaves projections / attn@V / transposes / out-proj between
    score+exp groups so the scalar engine rarely starves.

mm dtypes: x and w_qk in f32r/bf16 keep projections+scores accurate; the
attention path (probs, V, attn-out, Wo) runs in bf16 (PSUM accumulation is
fp32 throughout).
"""

import sys

sys.path.insert(0, "/opt/trn_rl_repo")

from collections import deque
from contextlib import ExitStack

import numpy as np

import concourse.bass as bass  # noqa: F401
import concourse.tile as tile
from concourse import bacc, masks, mybir
from concourse.bass_utils import run_bass_kernel_spmd

P = 128
DK = 64  # head dim

_CACHE = {}


def build_nc(S=2048, D=1024, DL=512, mm_dtype="float32r", n_cores=8,
             repeats=1, phases="ABC"):
    """Build + compile the per-core Bass program (same program on all cores).

    repeats exists only for timing experiments; production uses the default.
    mm_dtype/phases are accepted for test-harness compatibility (the kernel
    uses a fixed mixed f32r/bf16 precision scheme).
    """
    f32 = mybir.dt.float32
    f32r = mybir.dt.float32r
    bf16 = mybir.dt.bfloat16
    Exp = mybir.ActivationFunctionType.Exp

    H = DL // DK          # 8 local heads
    HP = H // 2           # 4 head pairs (one pair per 128-row q/k tile)
    ET = D // P           # 8 contraction tiles for projections
    ST = S // P           # 16 k tiles (and q tiles)
    NDT = DL // P         # 4 dl tiles
    QC = 512              # projection s-chunk
    NQ = S // QC          # 4
    KG = 4                # k-tiles per exp group
    NKG = ST // KG        # 4
    VW = H * (DK + 1)     # 520: v tile width incl. ones columns

    nc = bacc.Bacc("TRN2", target_bir_lowering=False, num_devices=n_cores)

    xqT = nc.dram_tensor("xqT", [D, S], bf16, kind="ExternalInput")
    xkT = nc.dram_tensor("xkT", [D, S], bf16, kind="ExternalInput")
    xvT = nc.dram_tensor("xvT", [D, S], bf16, kind="ExternalInput")
    wqT = nc.dram_tensor("wqT", [D, DL], bf16, kind="ExternalInput")
    wkT = nc.dram_tensor("wkT", [D, DL], bf16, kind="ExternalInput")
    wvT = nc.dram_tensor("wvT", [D, DL], bf16, kind="ExternalInput")
    woT = nc.dram_tensor("woT", [DL, D], bf16, kind="ExternalInput")
    bqd = nc.dram_tensor("bq", [DL, 1], f32, kind="ExternalInput")
    bkd = nc.dram_tensor("bk", [DL, 1], f32, kind="ExternalInput")
    y = nc.dram_tensor("y", [S, D], f32, kind="ExternalOutput")

    def mm(out, lhsT, rhs, start, stop):
        nc.tensor.matmul(out, lhsT=lhsT, rhs=rhs, start=start, stop=stop)

    with tile.TileContext(nc) as tc, ExitStack() as top:
        top.enter_context(
            nc.allow_low_precision(
                reason="attention path in bf16; PSUM accumulation stays fp32"
            )
        )
        persist = top.enter_context(tc.tile_pool(name="persist", bufs=1))
        qt = [persist.tile([P, S], bf16, tag=f"qt{i}", name=f"qt{i}") for i in range(NDT)]
        kt = [persist.tile([P, S], bf16, tag=f"kt{i}", name=f"kt{i}") for i in range(NDT)]
        vt = [persist.tile([P, VW], bf16, tag=f"vt{i}", name=f"vt{i}") for i in range(ST)]
        oaT = [persist.tile([P, S], bf16, tag=f"oaT{i}", name=f"oaT{i}") for i in range(NDT)]
        ident = persist.tile([P, P], f32, tag="ident", name="ident")
        bq_t = persist.tile([P, NDT], f32, tag="bq", name="bq")
        bk_t = persist.tile([P, NDT], f32, tag="bk", name="bk")

        masks.make_identity(nc, ident[:])
        warm = persist.tile([P, P], bf16, tag="warm", name="warm")
        nc.vector.memset(warm[:], 0.0)
        # vt ones-columns are memset inside slot 0 (below) so the head's
        # K/Q projection evacuations reach the DVE queue first.

        # PSUM: scores/exp 2x[128,1024] (4 banks) + attn@V accum 2x[128,260]
        # (2 banks) + generic matmul 2x[128,512] (2 banks) = 8 banks.
        spool = top.enter_context(tc.tile_pool(name="spool", bufs=2, space="PSUM"))
        acpool = top.enter_context(tc.tile_pool(name="acpool", bufs=2, space="PSUM"))
        gpool = top.enter_context(tc.tile_pool(name="gpool", bufs=2, space="PSUM"))

        # weight/x pools for Q (live through all Q chunks); wide layouts:
        # w tiles hold all ET contraction blocks side by side (one DMA each).
        wqp = top.enter_context(tc.tile_pool(name="wqp", bufs=1))
        wq = wqp.tile([P, ET * DL], bf16, tag="wq", name="wq")
        xqp = top.enter_context(tc.tile_pool(name="xqp", bufs=1))

        # long-lived attention pools (opened before any scoped pool so that
        # mid-stream pool closes stay LIFO)
        etsp = top.enter_context(tc.tile_pool(name="etsp", bufs=2))
        oasp = top.enter_context(tc.tile_pool(name="oasp", bufs=4))
        yvp = top.enter_context(tc.tile_pool(name="yvp", bufs=2))
        rcp = top.enter_context(tc.tile_pool(name="rcp", bufs=4))

        for _rep in range(repeats):
            # ---------------- pools for K and Q chunk streams -------------
            vstate = {}
            s3 = ExitStack()
            s2 = ExitStack()
            vxa = s2.enter_context(tc.tile_pool(name="vxa", bufs=1))
            vstate["wv"] = vxa.tile([P, ET * DL], bf16, tag="wv", name="wv")
            vstate["xv0"] = vxa.tile([P, ET * (S // 2)], bf16, tag="xv0",
                                     name="xv0")
            s1 = ExitStack()
            kx = s1.enter_context(tc.tile_pool(name="kx", bufs=2))
            wkp = s1.enter_context(tc.tile_pool(name="wkp", bufs=1))
            wk = wkp.tile([P, ET * DL], bf16, tag="wk", name="wk")

            def load_xk(c, eng=None):
                xkc = kx.tile([P, ET * QC], bf16, tag="xk", name="xk")
                (eng or nc.sync).dma_start(
                    out=xkc[:].rearrange("p (e s) -> p e s", e=ET),
                    in_=xkT[:, c * QC : (c + 1) * QC].rearrange(
                        "(e p) s -> p e s", p=P),
                )
                vstate["xk"] = xkc

            def load_xq(c, eng):
                xqc = xqp.tile([P, ET * QC], bf16, tag="xq", name="xq")
                eng.dma_start(
                    out=xqc[:].rearrange("p (e s) -> p e s", e=ET),
                    in_=xqT[:, c * QC : (c + 1) * QC].rearrange(
                        "(e p) s -> p e s", p=P),
                )
                vstate["xq"] = xqc

            def proj_piece(c, dch, w, xkey, bias, out_tiles):
                """One (chunk, dl-tile) projection: out (dl 128, s 512) + bias."""
                x = vstate[xkey]
                gp = gpool.tile([P, QC], f32, tag="gp", name="gp")
                for e in range(ET):
                    mm(gp[:], w[:, e * DL + dch * P : e * DL + (dch + 1) * P],
                       x[:, e * QC : (e + 1) * QC], e == 0, e == ET - 1)
                nc.vector.tensor_scalar_add(
                    out_tiles[dch][:, c * QC : (c + 1) * QC], gp[:],
                    bias[:, dch : dch + 1]
                )

            # --- V pools: wv + the first s-half of xv preload alongside the
            # K pool (slot 0); the second s-half lands in the space the K pool
            # frees. V projection runs head-half-major so attn@V for heads 0-3
            # unblocks as early as possible.
            SH = S // 2

            def load_wv():
                nc.sync.dma_start(
                    out=vstate["wv"][:].rearrange("p (e d) -> p e d", e=ET),
                    in_=wvT[:].rearrange("(e p) d -> p e d", p=P),
                )

            def load_xv0():
                nc.sync.dma_start(
                    out=vstate["xv0"][:].rearrange("p (e s) -> p e s", e=ET),
                    in_=xvT[:, 0:SH].rearrange("(e p) s -> p e s", p=P),
                )

            def open_vx():
                vxb = s2.enter_context(tc.tile_pool(name="vxb", bufs=1))
                vstate["xv1"] = vxb.tile([P, ET * SH], bf16, tag="xv1", name="xv1")
                nc.sync.dma_start(
                    out=vstate["xv1"][:].rearrange("p (e s) -> p e s", e=ET),
                    in_=xvT[:, SH:S].rearrange("(e p) s -> p e s", p=P),
                )

            def vproj_piece(st, qtr):
                """V projection for (s-tile st, head pair qtr): 2 heads.
                Quarters 0-1 run before attn@V starts, so odd s-tiles borrow
                the idle attn@V accumulator banks for deeper pipelining."""
                Q4 = DL // 4
                if qtr < 2 and st % 2 == 1:
                    gp = acpool.tile([P, 512], f32, tag="ac", name="ac")
                else:
                    gp = gpool.tile([P, QC], f32, tag="gp", name="gp")
                wv = vstate["wv"]
                xv = vstate["xv0"] if st < ST // 2 else vstate["xv1"]
                stl = st % (ST // 2)
                for e in range(ET):
                    mm(gp[:, 0:Q4],
                       xv[:, e * SH + stl * P : e * SH + (stl + 1) * P],
                       wv[:, e * DL + qtr * Q4 : e * DL + (qtr + 1) * Q4],
                       e == 0, e == ET - 1)
                nc.vector.tensor_copy(
                    vt[st][:].rearrange("p (h c) -> p h c", h=H)
                    [:, qtr * 2 : (qtr + 1) * 2, 0:DK],
                    gp[:, 0:Q4].rearrange("p (h c) -> p h c", h=2),
                )

            state = {
                "ets": {},    # (qt_idx, hp, kg) -> tile  (live window)
                "ac": {},     # (qt_idx, hgrp) -> tile
                "oas": {},    # qt_idx -> tile
                "wo": None,
            }

            def sc_use(qi, hp, kg):
                """Scores + exp for (q-tile qi, head pair hp, k-group kg)."""
                sc = spool.tile([P, 2 * KG * P], f32, tag="sc", name="sc")
                for hloc in range(2):
                    h = 2 * hp + hloc
                    r0 = hloc * DK
                    for ktl in range(KG):
                        ki = kg * KG + ktl
                        mm(
                            sc[:, hloc * KG * P + ktl * P : hloc * KG * P + (ktl + 1) * P],
                            kt[hp][r0 : r0 + DK, ki * P : (ki + 1) * P],
                            qt[hp][r0 : r0 + DK, qi * P : (qi + 1) * P],
                            True,
                            True,
                        )
                et = etsp.tile([P, 2 * KG * P], bf16, tag=f"et{hp}_{kg}",
                               name=f"et{hp}_{kg}")
                nc.scalar.activation(et[:], sc[:], Exp)
                state["ets"][(qi, hp, kg)] = et

            def attnv_piece(qi, h, kg):
                """attn@V for (q-tile qi, head h, k-group kg): 4 x 65-row mms."""
                hgrp, hidx = divmod(h, 4)
                key = (qi, hgrp)
                if key not in state["ac"]:
                    # padded to a full 2KB bank; cols 0-259 used (4 heads x 65)
                    state["ac"][key] = acpool.tile([P, 512], f32, tag="ac", name="ac")
                ac = state["ac"][key]
                et = state["ets"][(qi, h // 2, kg)]
                hloc = h % 2
                for ktl in range(KG):
                    ki = kg * KG + ktl
                    mm(
                        ac[:, hidx * (DK + 1) : (hidx + 1) * (DK + 1)],
                        et[:, hloc * KG * P + ktl * P : hloc * KG * P + (ktl + 1) * P],
                        vt[ki][:, h * (DK + 1) : (h + 1) * (DK + 1)],
                        ki == 0,
                        ki == ST - 1,
                    )
                if hloc == 1:
                    del state["ets"][(qi, h // 2, kg)]

            def norm_piece(qi, hgrp):
                """Normalize 4 heads: oa_s[:, hgrp*256:+256] = num * (1/Z)."""
                if qi not in state["oas"]:
                    state["oas"][qi] = oasp.tile([P, DL], f32, tag="oas", name="oas")
                oas = state["oas"][qi]
                ac = state["ac"].pop((qi, hgrp))
                acr = ac[:, 0 : 4 * (DK + 1)].rearrange("p (h c) -> p h c", h=4)
                rc = rcp.tile([P, 4], f32, tag="rc", name="rc")
                nc.vector.reciprocal(rc[:], acr[:, :, DK])
                for hh in range(4):
                    nc.vector.tensor_scalar_mul(
                        oas[:, hgrp * 4 * DK + hh * DK : hgrp * 4 * DK + (hh + 1) * DK],
                        acr[:, hh, 0:DK],
                        rc[:, hh : hh + 1],
                    )

            def transp_qt(qi, half=None):
                """Transpose oa_s (q, dl) -> oaT (dl, q) for one q-tile.
                half=0 covers dl-tiles 0-1 (ready after the hgrp-0 norm),
                half=1 covers 2-3; None does both."""
                rng = (range(NDT) if half is None
                       else range(2 * half, 2 * half + 2))
                sc = acpool.tile([P, 512], f32, tag="ac", name="ac")
                oas = state["oas"][qi]
                for dlb in rng:
                    nc.tensor.transpose(
                        sc[:, dlb * P : (dlb + 1) * P],
                        oas[:, dlb * P : (dlb + 1) * P],
                        ident[:],
                    )
                for dlb in rng:
                    nc.vector.tensor_copy(
                        oaT[dlb][:, qi * P : (qi + 1) * P], sc[:, dlb * P : (dlb + 1) * P]
                    )
                if half != 0:
                    del state["oas"][qi]

            def load_wo():
                wop = s3.enter_context(tc.tile_pool(name="wop", bufs=1))
                wo = wop.tile([P, NDT * D], bf16, tag="wo", name="wo")
                nc.sync.dma_start(
                    out=wo[:].rearrange("p (i d) -> p i d", i=NDT),
                    in_=woT[:].rearrange("(i p) d -> p i d", p=P),
                )
                state["wo"] = wo

            def c_piece(st, fc):
                """Output projection for (s-tile st, f-chunk fc)."""
                wo = state["wo"]
                gp = gpool.tile([P, QC], f32, tag="gp", name="gp")
                for dl in range(NDT):
                    mm(gp[:], oaT[dl][:, st * P : (st + 1) * P],
                       wo[:, dl * D + fc * QC : dl * D + (fc + 1) * QC],
                       dl == 0, dl == NDT - 1)
                yv = yvp.tile([P, QC], f32, tag="yv", name="yv")
                nc.vector.tensor_copy(yv[:], gp[:])
                nc.sync.dma_start(
                    out=y[st * P : (st + 1) * P, fc * QC : (fc + 1) * QC], in_=yv[:]
                )

            # ------------- interleaved emission ---------------------------
            # One FIFO of side pieces per slot, drained between score+exp
            # groups under a PE-lead budget, force-drained at slot end (and at
            # the MID marker before the hp2/hp3 half). Estimated PE ns/piece.
            EXP_NS, SC_NS = 1090.0, 430.0
            COST = {}

            def piece_cost(p):
                fn = p[0]
                if fn == proj_piece:
                    return 1750.0
                if fn == qproj_rest:
                    return 1350.0
                if fn == vproj_piece:
                    return 450.0
                if fn == attnv_piece:
                    return 160.0
                if fn == transp_qt:
                    return 520.0
                if fn == c_piece:
                    return 900.0
                return 0.0

            def run_piece(p):
                if p[0] == "loadxk":
                    load_xk(p[1])
                elif p[0] == "loadxq":
                    load_xq(p[1], nc.sync)
                elif p[0] == "loadwo":
                    load_wo()
                elif p[0] == "closes1":
                    s1.close()
                elif p[0] == "openvx":
                    open_vx()
                elif p[0] == "memset":
                    nc.vector.memset(vt[p[1]][:], 1.0)
                elif p[0] == "loadwv":
                    load_wv()
                elif p[0] == "loadxv0":
                    load_xv0()
                elif p[0] == "closes2":
                    s2.close()
                elif p[0] == "HPM":
                    pass
                else:
                    p[0](*p[1:])

            def attnv_hp(qi, hp):
                out = []
                for h in (2 * hp, 2 * hp + 1):
                    for kg in range(NKG):
                        out.append((attnv_piece, qi, h, kg))
                return out

            slot_inter = [[] for _ in range(ST)]
            slot_markers = [set() for _ in range(ST)]
            slot_inter[0] += [("memset", i) for i in range(ST)]
            slot_inter[0] += [("loadwv",), ("loadxv0",)]
            slot_inter[0] += [(vproj_piece, st, 0) for st in range(ST // 2)]
            slot_inter[1] += [(vproj_piece, st, 0) for st in range(ST // 2, ST)]
            slot_inter[1] += attnv_hp(0, 0)
            slot_inter[1] += [(vproj_piece, st, 1) for st in range(ST)]
            slot_inter[1] += attnv_hp(0, 1) + [(norm_piece, 0, 0)]
            slot_inter[2] += [(vproj_piece, st, 2) for st in range(ST)]
            slot_inter[2] += attnv_hp(0, 2) + [("HPM", 2)]
            slot_inter[2] += [(vproj_piece, st, 3) for st in range(ST)]
            slot_inter[2] += attnv_hp(0, 3) + [(norm_piece, 0, 1), ("HPM", 3)]
            slot_inter[2] += [("closes2",), ("loadwo",)]
            slot_markers[2] = {2, 3}
            slot_inter[3] += (attnv_hp(1, 0) + [("HPM", 0)]
                              + attnv_hp(1, 1) + [(norm_piece, 1, 0), ("HPM", 1)]
                              + attnv_hp(1, 2) + [("HPM", 2)]
                              + attnv_hp(1, 3) + [(norm_piece, 1, 1), ("HPM", 3)])
            slot_markers[3] = {0, 1, 2, 3}
            slot_inter[3] += [(transp_qt, 0), (transp_qt, 1)]
            for j in range(3, ST):
                slot_inter[j] += (attnv_hp(j - 1, 0) + attnv_hp(j - 1, 1)
                                  + [(norm_piece, j - 1, 0)]
                                  + attnv_hp(j - 1, 2) + attnv_hp(j - 1, 3)
                                  + [(norm_piece, j - 1, 1)])
                if j - 1 >= 2:
                    slot_inter[j].append((transp_qt, j - 1))
            # out-projection: early s-tiles deferred to late slots (the early
            # slots carry the V/K/Q overload), the rest two slots after their
            # transpose.
            for st in range(0, 5):
                slot_inter[11 + st] += [(c_piece, st, 0), (c_piece, st, 1)]
            for st in range(5, 14):
                slot_inter[st + 2] += [(c_piece, st, 0), (c_piece, st, 1)]
            # Q chunk c: DMA early, project each dl-tile just before the
            # first score group of slot 4c that needs it.
            def qproj_mini(c, dch):
                """Q projection for q-tile 4c only (slot 4c's own columns)."""
                x = vstate["xq"]
                gp = gpool.tile([P, QC], f32, tag="gp", name="gp")
                for e in range(ET):
                    mm(gp[:, 0:P],
                       wq[:, e * DL + dch * P : e * DL + (dch + 1) * P],
                       x[:, e * QC : e * QC + P], e == 0, e == ET - 1)
                nc.vector.tensor_scalar_add(
                    qt[dch][:, 4 * c * P : (4 * c + 1) * P], gp[:, 0:P],
                    bq_t[:, dch : dch + 1])

            def qproj_rest(c, dch):
                """Q projection for q-tiles 4c+1..4c+3 (needed next slot)."""
                x = vstate["xq"]
                gp = gpool.tile([P, QC], f32, tag="gp", name="gp")
                for e in range(ET):
                    mm(gp[:, 0 : 3 * P],
                       wq[:, e * DL + dch * P : e * DL + (dch + 1) * P],
                       x[:, e * QC + P : (e + 1) * QC], e == 0, e == ET - 1)
                nc.vector.tensor_scalar_add(
                    qt[dch][:, (4 * c + 1) * P : (4 * c + 4) * P],
                    gp[:, 0 : 3 * P], bq_t[:, dch : dch + 1])

            slot_hp = [[[] for _ in range(HP)] for _ in range(ST)]
            for c in (1, 2, 3):
                slot_inter[4 * c - 2].append(("loadxq", c))
                for dch in range(NDT):
                    slot_hp[4 * c][dch].append((qproj_mini, c, dch))
                slot_inter[4 * c] += [(qproj_rest, c, dch)
                                      for dch in range(NDT)]

            # ---------------- head + slot loop ----------------------------
            nc.scalar.dma_start(
                out=wk[:].rearrange("p (e d) -> p e d", e=ET),
                in_=wkT[:].rearrange("(e p) d -> p e d", p=P))
            load_xk(0)
            load_xq(0, nc.scalar)
            nc.sync.dma_start(
                out=wq[:].rearrange("p (e d) -> p e d", e=ET),
                in_=wqT[:].rearrange("(e p) d -> p e d", p=P))
            nc.sync.dma_start(out=bk_t[:].rearrange("p i -> p i ()"),
                              in_=bkd[:].rearrange("(i p) o -> p i o", p=P))
            nc.sync.dma_start(out=bq_t[:].rearrange("p i -> p i ()"),
                              in_=bqd[:].rearrange("(i p) o -> p i o", p=P))
            # PE clock warm-up while the first loads are in flight: harmless
            # zero matmuls keep the PE busy so it reaches full p-state before
            # the first projection.
            for wu in range(26):
                wsc = spool.tile([P, 2 * KG * P], f32, tag="sc", name="sc")
                for j in range(4):
                    mm(wsc[:, j * P : (j + 1) * P], warm[:], warm[:], True, True)

            budget = [0.0]

            def drain(inter, force=False, to_marker=None):
                while inter:
                    if (not force and to_marker is None
                            and piece_cost(inter[0]) > budget[0]):
                        return
                    p = inter.popleft()
                    run_piece(p)
                    budget[0] -= piece_cost(p)
                    if to_marker is not None and p == ("HPM", to_marker):
                        return

            def do_sc(qi, hp, kg, inter):
                sc_use(qi, hp, kg)
                budget[0] = min(budget[0] + (EXP_NS - SC_NS), 3200.0)
                drain(inter)

            # slot 0: interleave K/Q chunk-0 projections with the first score
            # groups (head pair hp becomes ready as soon as dl-tile hp
            # projects), then kg 1-3 as K chunks 1-3 land.
            inter0 = deque(slot_inter[0])
            for dch in range(NDT):
                proj_piece(0, dch, wk, "xk", bk_t, kt)
                proj_piece(0, dch, wq, "xq", bq_t, qt)
                sc_use(0, dch, 0)
            xk_next = {}
            load_xk(1)
            xk_next[1] = vstate["xk"]
            for kg in range(1, NKG):
                vstate["xk"] = xk_next[kg]
                if kg + 1 < NKG:
                    load_xk(kg + 1)
                    xk_next[kg + 1] = vstate["xk"]
                    vstate["xk"] = xk_next[kg]
                for dch in range(NDT):
                    proj_piece(kg, dch, wk, "xk", bk_t, kt)
                for hp in range(HP):
                    do_sc(0, hp, kg, inter0)
            drain(inter0, force=True)
            s1.close()
            open_vx()

            for qi in range(1, ST):
                inter = deque(slot_inter[qi])
                for hp in range(HP):
                    if hp in slot_markers[qi]:
                        drain(inter, to_marker=hp)
                    for p in slot_hp[qi][hp]:
                        run_piece(p)
                    for kg in range(NKG):
                        if hp == HP - 1 and kg == 0:
                            # flush leftovers while the previous exp groups
                            # still cover the scalar engine
                            drain(inter, force=True)
                        do_sc(qi, hp, kg, inter)
                drain(inter, force=True)

            # ---------------- tail ----------------------------------------
            for piece in (attnv_hp(ST - 1, 0) + attnv_hp(ST - 1, 1)
                          + [(norm_piece, ST - 1, 0)]):
                run_piece(piece)
            transp_qt(ST - 1, half=0)
            for piece in (attnv_hp(ST - 1, 2) + attnv_hp(ST - 1, 3)
                          + [(norm_piece, ST - 1, 1)]):
                run_piece(piece)
            transp_qt(ST - 1, half=1)
            for st in range(ST - 2, ST):
                for fc in range(D // QC):
                    c_piece(st, fc)
            s3.close()

    nc.compile()
    return nc


def make_in_maps(query, key, value, Wq, bq, Wk, bk, Wv, bv, n_cores=8,
                 mm_dtype="float32r"):
    """Host-side sharding: slice weights Megatron-style, transpose activations."""
    import ml_dtypes

    bft = ml_dtypes.bfloat16
    q = np.asarray(query, dtype=np.float32)
    k = np.asarray(key, dtype=np.float32)
    v = np.asarray(value, dtype=np.float32)
    Wq = np.asarray(Wq, dtype=np.float32)
    Wk = np.asarray(Wk, dtype=np.float32)
    Wv = np.asarray(Wv, dtype=np.float32)
    bq = np.asarray(bq, dtype=np.float32)
    bk = np.asarray(bk, dtype=np.float32)
    D = Wq.shape[0]
    DL = D // (n_cores // q.shape[0])
    scale = 1.0 / np.sqrt(np.float32(DK))
    in_maps = []
    for c in range(n_cores):
        b, g = divmod(c, n_cores // q.shape[0])
        sl = slice(DL * g, DL * (g + 1))
        in_maps.append(
            {
                "xqT": np.ascontiguousarray(q[b].T).astype(bft),
                "xkT": np.ascontiguousarray(k[b].T).astype(bft),
                "xvT": np.ascontiguousarray(v[b].T).astype(bft),
                "wqT": (np.ascontiguousarray(Wq[sl].T) * scale).astype(bft),
                "wkT": np.ascontiguousarray(Wk[sl].T).astype(bft),
                "wvT": np.ascontiguousarray(Wv[sl].T).astype(bft),
                "bq": np.ascontiguousarray((bq[sl] * scale).reshape(DL, 1)),
                "bk": np.ascontiguousarray(bk[sl].reshape(DL, 1)),
            }
        )
    return in_maps


def add_wo_maps(in_maps, Wo, n_cores=8, n_batch=4, mm_dtype="float32r"):
    import ml_dtypes

    Wo = np.asarray(Wo, dtype=np.float32)
    D = Wo.shape[0]
    DL = D // (n_cores // n_batch)
    for c in range(n_cores):
        _, g = divmod(c, n_cores // n_batch)
        sl = slice(DL * g, DL * (g + 1))
        in_maps[c]["woT"] = np.ascontiguousarray(Wo[:, sl].T).astype(ml_dtypes.bfloat16)
    return in_maps


MM_DTYPE = "float32r"


def kernel(query, key, value, Wq, bq, Wk, bk, Wv, bv, Wo, bo):
    if "nc" not in _CACHE:
        _CACHE["nc"] = build_nc(mm_dtype=MM_DTYPE)
    nc = _CACHE["nc"]
    n_cores = 8
    in_maps = make_in_maps(
        query, key, value, Wq, bq, Wk, bk, Wv, bv, n_cores, MM_DTYPE
    )
    add_wo_maps(in_maps, Wo, n_cores, np.asarray(query).shape[0], MM_DTYPE)
    res = run_bass_kernel_spmd(nc, in_maps, list(range(n_cores)))
    ys = [res.results[c]["y"] for c in range(n_cores)]
    bo = np.asarray(bo, dtype=np.float32)
    bv = np.asarray(bv, dtype=np.float32)
    Wo = np.asarray(Wo, dtype=np.float32)
    const = bo + bv @ Wo.T
    out = np.stack([ys[2 * b] + ys[2 * b + 1] for b in range(4)]) + const[None, None, :]
    return out.astype(np.float32)


# revision 85
# speedup vs baseline: 1.0436x; 1.0003x over previous
"""Trainium2 Bass kernel for nn_MultiHeadAttention_37838661877847.

Full-input contract: kernel(**inputs) takes the complete tensors and returns
the complete output. Internally shards across 8 NeuronCores:
  core c -> batch b = c // 2, head-group g = c % 2 (8 heads, 512 dims each).
Each core computes Q/K/V projections for its (batch, head-group) slice
(column-parallel weights), attention for its 8 heads, and a partial output
projection (row-parallel Wo). Host sums core pairs and adds bo + bv @ Wo.T
(the V bias commutes through softmax-weighted averaging, so it is folded
into the output-projection bias on the host).

Engine-level design (per core), built as ONE interleaved instruction stream
so the scalar engine's softmax-exp (the 266us serial floor: 33.5M exps at
1 elem/lane/cycle) overlaps the tensor engine work (281us):

  - Q_T/K_T stored (dl, s) in bf16; scores come out (k, q) per 128-k tile.
  - exp groups of [128, 1024] PSUM (4 score blocks: 2 heads x ... see sc
    layout below) -> ets tiles in bf16.
  - attn@V is FLIPPED: out (q, dk+1) accumulating over k with the exp tile
    as the stationary operand -> 65-row bf16 matmuls, half the PE rows of
    the (dk+1, q) orientation. V is augmented with a ones column per head so
    the softmax denominator Z lands in column 64; normalization is then a
    per-partition reciprocal+scale on DVE.
  - normalized output (q, dl) is transposed back to (dl, q) via PE-transpose
    through spare score-PSUM space, then the output projection streams wo.
  - emission interleaves projections / attn@V / transposes / out-proj between
    score+exp groups so the scalar engine rarely starves.

mm dtypes: x and w_qk in f32r/bf16 keep projections+scores accurate; the
attention path (probs, V, attn-out, Wo) runs in bf16 (PSUM accumulation is
fp32 throughout).
"""

import sys

sys.path.insert(0, "/opt/trn_rl_repo")

from collections import deque
from contextlib import ExitStack

import numpy as np

import concourse.bass as bass  # noqa: F401
import concourse.tile as tile
from concourse import bacc, masks, mybir
from concourse.bass_utils import run_bass_kernel_spmd

P = 128
DK = 64  # head dim

_CACHE = {}


def build_nc(S=2048, D=1024, DL=512, mm_dtype="float32r", n_cores=8,
             repeats=1, phases="ABC"):
    """Build + compile the per-core Bass program (same program on all cores).

    repeats exists only for timing experiments; production uses the default.
    mm_dtype/phases are accepted for test-harness compatibility (the kernel
    uses a fixed mixed f32r/bf16 precision scheme).
    """
    f32 = mybir.dt.float32
    f32r = mybir.dt.float32r
    bf16 = mybir.dt.bfloat16
    Exp = mybir.ActivationFunctionType.Exp

    H = DL // DK          # 8 local heads
    HP = H // 2           # 4 head pairs (one pair per 128-row q/k tile)
    ET = D // P           # 8 contraction tiles for projections
    ST = S // P           # 16 k tiles (and q tiles)
    NDT = DL // P         # 4 dl tiles
    QC = 512              # projection s-chunk
    NQ = S // QC          # 4
    KG = 4                # k-tiles per exp group
    NKG = ST // KG        # 4
    VW = H * (DK + 1)     # 520: v tile width incl. ones columns

    nc = bacc.Bacc("TRN2", target_bir_lowering=False, num_devices=n_cores)

    xqT = nc.dram_tensor("xqT", [D, S], bf16, kind="ExternalInput")
    xkT = nc.dram_tensor("xkT", [D, S], bf16, kind="ExternalInput")
    xvT = nc.dram_tensor("xvT", [D, S], bf16, kind="ExternalInput")
    wqT = nc.dram_tensor("wqT", [D, DL], bf16, kind="ExternalInput")
    wkT = nc.dram_tensor("wkT", [D, DL], bf16, kind="ExternalInput")
    wvT = nc.dram_tensor("wvT", [D, DL], bf16, kind="ExternalInput")
    woT = nc.dram_tensor("woT", [DL, D], bf16, kind="ExternalInput")
    bqd = nc.dram_tensor("bq", [DL, 1], f32, kind="ExternalInput")
    bkd = nc.dram_tensor("bk", [DL, 1], f32, kind="ExternalInput")
    y = nc.dram_tensor("y", [S, D], f32, kind="ExternalOutput")

    def mm(out, lhsT, rhs, start, stop):
        nc.tensor.matmul(out, lhsT=lhsT, rhs=rhs, start=start, stop=stop)

    with tile.TileContext(nc) as tc, ExitStack() as top:
        top.enter_context(
            nc.allow_low_precision(
                reason="attention path in bf16; PSUM accumulation stays fp32"
            )
        )
        persist = top.enter_context(tc.tile_pool(name="persist", bufs=1))
        qt = [persist.tile([P, S], bf16, tag=f"qt{i}", name=f"qt{i}") for i in range(NDT)]
        kt = [persist.tile([P, S], bf16, tag=f"kt{i}", name=f"kt{i}") for i in range(NDT)]
        vt = [persist.tile([P, VW], bf16, tag=f"vt{i}", name=f"vt{i}") for i in range(ST)]
        oaT = [persist.tile([P, S], bf16, tag=f"oaT{i}", name=f"oaT{i}") for i in range(NDT)]
        ident = persist.tile([P, P], f32, tag="ident", name="ident")
        bq_t = persist.tile([P, NDT], f32, tag="bq", name="bq")
        bk_t = persist.tile([P, NDT], f32, tag="bk", name="bk")

        masks.make_identity(nc, ident[:])
        warm = persist.tile([P, P], bf16, tag="warm", name="warm")
        nc.vector.memset(warm[:], 0.0)
        # vt ones-columns are memset inside slot 0 (below) so the head's
        # K/Q projection evacuations reach the DVE queue first.

        # PSUM: scores/exp 2x[128,1024] (4 banks) + attn@V accum 2x[128,260]
        # (2 banks) + generic matmul 2x[128,512] (2 banks) = 8 banks.
        spool = top.enter_context(tc.tile_pool(name="spool", bufs=2, space="PSUM"))
        acpool = top.enter_context(tc.tile_pool(name="acpool", bufs=2, space="PSUM"))
        gpool = top.enter_context(tc.tile_pool(name="gpool", bufs=2, space="PSUM"))

        # weight/x pools for Q (live through all Q chunks); wide layouts:
        # w tiles hold all ET contraction blocks side by side (one DMA each).
        wqp = top.enter_context(tc.tile_pool(name="wqp", bufs=1))
        wq = wqp.tile([P, ET * DL], bf16, tag="wq", name="wq")
        xqp = top.enter_context(tc.tile_pool(name="xqp", bufs=1))

        # long-lived attention pools (opened before any scoped pool so that
        # mid-stream pool closes stay LIFO)
        etsp = top.enter_context(tc.tile_pool(name="etsp", bufs=2))
        oasp = top.enter_context(tc.tile_pool(name="oasp", bufs=4))
        yvp = top.enter_context(tc.tile_pool(name="yvp", bufs=2))
        rcp = top.enter_context(tc.tile_pool(name="rcp", bufs=4))

        for _rep in range(repeats):
            # ---------------- pools for K and Q chunk streams -------------
            vstate = {}
            s3 = ExitStack()
            s2 = ExitStack()
            vxa = s2.enter_context(tc.tile_pool(name="vxa", bufs=1))
            vstate["wv"] = vxa.tile([P, ET * DL], bf16, tag="wv", name="wv")
            vstate["xv0"] = vxa.tile([P, ET * (S // 2)], bf16, tag="xv0",
                                     name="xv0")
            s1 = ExitStack()
            kx = s1.enter_context(tc.tile_pool(name="kx", bufs=2))
            wkp = s1.enter_context(tc.tile_pool(name="wkp", bufs=1))
            wk = wkp.tile([P, ET * DL], bf16, tag="wk", name="wk")

            def load_xk(c, eng=None):
                xkc = kx.tile([P, ET * QC], bf16, tag="xk", name="xk")
                (eng or nc.sync).dma_start(
                    out=xkc[:].rearrange("p (e s) -> p e s", e=ET),
                    in_=xkT[:, c * QC : (c + 1) * QC].rearrange(
                        "(e p) s -> p e s", p=P),
                )
                vstate["xk"] = xkc

            def load_xq(c, eng):
                xqc = xqp.tile([P, ET * QC], bf16, tag="xq", name="xq")
                eng.dma_start(
                    out=xqc[:].rearrange("p (e s) -> p e s", e=ET),
                    in_=xqT[:, c * QC : (c + 1) * QC].rearrange(
                        "(e p) s -> p e s", p=P),
                )
                vstate["xq"] = xqc

            def proj_piece(c, dch, w, xkey, bias, out_tiles):
                """One (chunk, dl-tile) projection: out (dl 128, s 512) + bias."""
                x = vstate[xkey]
                gp = gpool.tile([P, QC], f32, tag="gp", name="gp")
                for e in range(ET):
                    mm(gp[:], w[:, e * DL + dch * P : e * DL + (dch + 1) * P],
                       x[:, e * QC : (e + 1) * QC], e == 0, e == ET - 1)
                nc.vector.tensor_scalar_add(
                    out_tiles[dch][:, c * QC : (c + 1) * QC], gp[:],
                    bias[:, dch : dch + 1]
                )

            # --- V pools: wv + the first s-half of xv preload alongside the
            # K pool (slot 0); the second s-half lands in the space the K pool
            # frees. V projection runs head-half-major so attn@V for heads 0-3
            # unblocks as early as possible.
            SH = S // 2

            def load_wv():
                nc.sync.dma_start(
                    out=vstate["wv"][:].rearrange("p (e d) -> p e d", e=ET),
                    in_=wvT[:].rearrange("(e p) d -> p e d", p=P),
                )

            def load_xv0():
                nc.sync.dma_start(
                    out=vstate["xv0"][:].rearrange("p (e s) -> p e s", e=ET),
                    in_=xvT[:, 0:SH].rearrange("(e p) s -> p e s", p=P),
                )

            def open_vx():
                vxb = s2.enter_context(tc.tile_pool(name="vxb", bufs=1))
                vstate["xv1"] = vxb.tile([P, ET * SH], bf16, tag="xv1", name="xv1")
                nc.sync.dma_start(
                    out=vstate["xv1"][:].rearrange("p (e s) -> p e s", e=ET),
                    in_=xvT[:, SH:S].rearrange("(e p) s -> p e s", p=P),
                )

            def vproj_piece(st, qtr):
                """V projection for (s-tile st, head pair qtr): 2 heads.
                Quarters 0-1 run before attn@V starts, so odd s-tiles borrow
                the idle attn@V accumulator banks for deeper pipelining."""
                Q4 = DL // 4
                if qtr < 2 and st % 2 == 1:
                    gp = acpool.tile([P, 512], f32, tag="ac", name="ac")
                else:
                    gp = gpool.tile([P, QC], f32, tag="gp", name="gp")
                wv = vstate["wv"]
                xv = vstate["xv0"] if st < ST // 2 else vstate["xv1"]
                stl = st % (ST // 2)
                for e in range(ET):
                    mm(gp[:, 0:Q4],
                       xv[:, e * SH + stl * P : e * SH + (stl + 1) * P],
                       wv[:, e * DL + qtr * Q4 : e * DL + (qtr + 1) * Q4],
                       e == 0, e == ET - 1)
                nc.vector.tensor_copy(
                    vt[st][:].rearrange("p (h c) -> p h c", h=H)
                    [:, qtr * 2 : (qtr + 1) * 2, 0:DK],
                    gp[:, 0:Q4].rearrange("p (h c) -> p h c", h=2),
                )

            state = {
                "ets": {},    # (qt_idx, hp, kg) -> tile  (live window)
                "ac": {},     # (qt_idx, hgrp) -> tile
                "oas": {},    # qt_idx -> tile
                "wo": None,
            }

            def sc_use(qi, hp, kg):
                """Scores + exp for (q-tile qi, head pair hp, k-group kg)."""
                sc = spool.tile([P, 2 * KG * P], f32, tag="sc", name="sc")
                for hloc in range(2):
                    h = 2 * hp + hloc
                    r0 = hloc * DK
                    for ktl in range(KG):
                        ki = kg * KG + ktl
                        mm(
                            sc[:, hloc * KG * P + ktl * P : hloc * KG * P + (ktl + 1) * P],
                            kt[hp][r0 : r0 + DK, ki * P : (ki + 1) * P],
                            qt[hp][r0 : r0 + DK, qi * P : (qi + 1) * P],
                            True,
                            True,
                        )
                et = etsp.tile([P, 2 * KG * P], bf16, tag=f"et{hp}_{kg}",
                               name=f"et{hp}_{kg}")
                nc.scalar.activation(et[:], sc[:], Exp)
                state["ets"][(qi, hp, kg)] = et

            def attnv_piece(qi, h, kg):
                """attn@V for (q-tile qi, head h, k-group kg): 4 x 65-row mms."""
                hgrp, hidx = divmod(h, 4)
                key = (qi, hgrp)
                if key not in state["ac"]:
                    # padded to a full 2KB bank; cols 0-259 used (4 heads x 65)
                    state["ac"][key] = acpool.tile([P, 512], f32, tag="ac", name="ac")
                ac = state["ac"][key]
                et = state["ets"][(qi, h // 2, kg)]
                hloc = h % 2
                for ktl in range(KG):
                    ki = kg * KG + ktl
                    mm(
                        ac[:, hidx * (DK + 1) : (hidx + 1) * (DK + 1)],
                        et[:, hloc * KG * P + ktl * P : hloc * KG * P + (ktl + 1) * P],
                        vt[ki][:, h * (DK + 1) : (h + 1) * (DK + 1)],
                        ki == 0,
                        ki == ST - 1,
                    )
                if hloc == 1:
                    del state["ets"][(qi, h // 2, kg)]

            def norm_piece(qi, hgrp):
                """Normalize 4 heads: oa_s[:, hgrp*256:+256] = num * (1/Z)."""
                if qi not in state["oas"]:
                    state["oas"][qi] = oasp.tile([P, DL], f32, tag="oas", name="oas")
                oas = state["oas"][qi]
                ac = state["ac"].pop((qi, hgrp))
                acr = ac[:, 0 : 4 * (DK + 1)].rearrange("p (h c) -> p h c", h=4)
                rc = rcp.tile([P, 4], f32, tag="rc", name="rc")
                nc.vector.reciprocal(rc[:], acr[:, :, DK])
                for hh in range(4):
                    nc.vector.tensor_scalar_mul(
                        oas[:, hgrp * 4 * DK + hh * DK : hgrp * 4 * DK + (hh + 1) * DK],
                        acr[:, hh, 0:DK],
                        rc[:, hh : hh + 1],
                    )

            def transp_qt(qi, half=None):
                """Transpose oa_s (q, dl) -> oaT (dl, q) for one q-tile.
                half=0 covers dl-tiles 0-1 (ready after the hgrp-0 norm),
                half=1 covers 2-3; None does both."""
                rng = (range(NDT) if half is None
                       else range(2 * half, 2 * half + 2))
                sc = acpool.tile([P, 512], f32, tag="ac", name="ac")
                oas = state["oas"][qi]
                for dlb in rng:
                    nc.tensor.transpose(
                        sc[:, dlb * P : (dlb + 1) * P],
                        oas[:, dlb * P : (dlb + 1) * P],
                        ident[:],
                    )
                for dlb in rng:
                    nc.vector.tensor_copy(
                        oaT[dlb][:, qi * P : (qi + 1) * P], sc[:, dlb * P : (dlb + 1) * P]
                    )
                if half != 0:
                    del state["oas"][qi]

            def load_wo():
                wop = s3.enter_context(tc.tile_pool(name="wop", bufs=1))
                wo = wop.tile([P, NDT * D], bf16, tag="wo", name="wo")
                nc.sync.dma_start(
                    out=wo[:].rearrange("p (i d) -> p i d", i=NDT),
                    in_=woT[:].rearrange("(i p) d -> p i d", p=P),
                )
                state["wo"] = wo

            def c_piece(st, fc):
                """Output projection for (s-tile st, f-chunk fc)."""
                wo = state["wo"]
                gp = gpool.tile([P, QC], f32, tag="gp", name="gp")
                for dl in range(NDT):
                    mm(gp[:], oaT[dl][:, st * P : (st + 1) * P],
                       wo[:, dl * D + fc * QC : dl * D + (fc + 1) * QC],
                       dl == 0, dl == NDT - 1)
                yv = yvp.tile([P, QC], f32, tag="yv", name="yv")
                nc.vector.tensor_copy(yv[:], gp[:])
                nc.sync.dma_start(
                    out=y[st * P : (st + 1) * P, fc * QC : (fc + 1) * QC], in_=yv[:]
                )

            # ------------- interleaved emission ---------------------------
            # One FIFO of side pieces per slot, drained between score+exp
            # groups under a PE-lead budget, force-drained at slot end (and at
            # the MID marker before the hp2/hp3 half). Estimated PE ns/piece.
            EXP_NS, SC_NS = 1090.0, 430.0
            COST = {}

            def piece_cost(p):
                fn = p[0]
                if fn == proj_piece:
                    return 1750.0
                if fn == qproj_rest:
                    return 1350.0
                if fn == vproj_piece:
                    return 450.0
                if fn == attnv_piece:
                    return 160.0
                if fn == transp_qt:
                    return 520.0
                if fn == c_piece:
                    return 900.0
                return 0.0

            def run_piece(p):
                if p[0] == "loadxk":
                    load_xk(p[1])
                elif p[0] == "loadxq":
                    load_xq(p[1], nc.sync)
                elif p[0] == "loadwo":
                    load_wo()
                elif p[0] == "closes1":
                    s1.close()
                elif p[0] == "openvx":
                    open_vx()
                elif p[0] == "memset":
                    nc.vector.memset(vt[p[1]][:], 1.0)
                elif p[0] == "loadwv":
                    load_wv()
                elif p[0] == "loadxv0":
                    load_xv0()
                elif p[0] == "closes2":
                    s2.close()
                elif p[0] == "HPM":
                    pass
                else:
                    p[0](*p[1:])

            def attnv_hp(qi, hp):
                out = []
                for h in (2 * hp, 2 * hp + 1):
                    for kg in range(NKG):
                        out.append((attnv_piece, qi, h, kg))
                return out

            slot_inter = [[] for _ in range(ST)]
            slot_markers = [set() for _ in range(ST)]
            slot_inter[0] += [("memset", i) for i in range(ST)]
            slot_inter[0] += [("loadwv",), ("loadxv0",)]
            slot_inter[0] += [(vproj_piece, st, 0) for st in range(ST // 2)]
            slot_inter[1] += [(vproj_piece, st, 0) for st in range(ST // 2, ST)]
            slot_inter[1] += attnv_hp(0, 0)
            slot_inter[1] += [(vproj_piece, st, 1) for st in range(ST)]
            slot_inter[1] += attnv_hp(0, 1) + [(norm_piece, 0, 0)]
            slot_inter[2] += [(vproj_piece, st, 2) for st in range(ST)]
            slot_inter[2] += attnv_hp(0, 2) + [("HPM", 2)]
            slot_inter[2] += [(vproj_piece, st, 3) for st in range(ST)]
            slot_inter[2] += attnv_hp(0, 3) + [(norm_piece, 0, 1), ("HPM", 3)]
            slot_inter[2] += [("closes2",), ("loadwo",)]
            slot_markers[2] = {2, 3}
            slot_inter[3] += (attnv_hp(1, 0) + [("HPM", 0)]
                              + attnv_hp(1, 1) + [(norm_piece, 1, 0), ("HPM", 1)]
                              + attnv_hp(1, 2) + [("HPM", 2)]
                              + attnv_hp(1, 3) + [(norm_piece, 1, 1), ("HPM", 3)])
            slot_markers[3] = {0, 1, 2, 3}
            slot_inter[3] += [(transp_qt, 0), (transp_qt, 1)]
            for j in range(3, ST):
                slot_inter[j] += (attnv_hp(j - 1, 0) + attnv_hp(j - 1, 1)
                                  + [(norm_piece, j - 1, 0)]
                                  + attnv_hp(j - 1, 2) + attnv_hp(j - 1, 3)
                                  + [(norm_piece, j - 1, 1)])
                if j - 1 >= 2:
                    slot_inter[j].append((transp_qt, j - 1))
            # out-projection: early s-tiles deferred to late slots (the early
            # slots carry the V/K/Q overload), the rest two slots after their
            # transpose.
            for st in range(0, 5):
                slot_inter[11 + st] += [(c_piece, st, 0), (c_piece, st, 1)]
            for st in range(5, 14):
                slot_inter[st + 2] += [(c_piece, st, 0), (c_piece, st, 1)]
            # Q chunk c: DMA early, project each dl-tile just before the
            # first score group of slot 4c that needs it.
            def qproj_mini(c, dch):
                """Q projection for q-tile 4c only (slot 4c's own columns)."""
                x = vstate["xq"]
                gp = gpool.tile([P, QC], f32, tag="gp", name="gp")
                for e in range(ET):
                    mm(gp[:, 0:P],
                       wq[:, e * DL + dch * P : e * DL + (dch + 1) * P],
                       x[:, e * QC : e * QC + P], e == 0, e == ET - 1)
                nc.vector.tensor_scalar_add(
                    qt[dch][:, 4 * c * P : (4 * c + 1) * P], gp[:, 0:P],
                    bq_t[:, dch : dch + 1])

            def qproj_rest(c, dch):
                """Q projection for q-tiles 4c+1..4c+3 (needed next slot)."""
                x = vstate["xq"]
                gp = gpool.tile([P, QC], f32, tag="gp", name="gp")
                for e in range(ET):
                    mm(gp[:, 0 : 3 * P],
                       wq[:, e * DL + dch * P : e * DL + (dch + 1) * P],
                       x[:, e * QC + P : (e + 1) * QC], e == 0, e == ET - 1)
                nc.vector.tensor_scalar_add(
                    qt[dch][:, (4 * c + 1) * P : (4 * c + 4) * P],
                    gp[:, 0 : 3 * P], bq_t[:, dch : dch + 1])

            slot_hp = [[[] for _ in range(HP)] for _ in range(ST)]
            for c in (1, 2, 3):
                slot_inter[4 * c - 2].append(("loadxq", c))
                for dch in range(NDT):
                    slot_hp[4 * c][dch].append((qproj_mini, c, dch))
                slot_inter[4 * c] += [(qproj_rest, c, dch)
                                      for dch in range(NDT)]

            # ---------------- head + slot loop ----------------------------
            nc.scalar.dma_start(
                out=wk[:].rearrange("p (e d) -> p e d", e=ET),
                in_=wkT[:].rearrange("(e p) d -> p e d", p=P))
            load_xk(0)
            load_xq(0, nc.scalar)
            nc.sync.dma_start(
                out=wq[:].rearrange("p (e d) -> p e d", e=ET),
                in_=wqT[:].rearrange("(e p) d -> p e d", p=P))
            nc.sync.dma_start(out=bk_t[:].rearrange("p i -> p i ()"),
                              in_=bkd[:].rearrange("(i p) o -> p i o", p=P))
            nc.sync.dma_start(out=bq_t[:].rearrange("p i -> p i ()"),
                              in_=bqd[:].rearrange("(i p) o -> p i o", p=P))
            # PE clock warm-up while the first loads are in flight: harmless
            # zero matmuls keep the PE busy so it reaches full p-state before
            # the first projection.
            for wu in range(26):
                wsc = spool.tile([P, 2 * KG * P], f32, tag="sc", name="sc")
                for j in range(4):
                    mm(wsc[:, j * P : (j + 1) * P], warm[:], warm[:], True, True)

            budget = [0.0]

            def drain(inter, force=False, to_marker=None):
                while inter:
                    if (not force and to_marker is None
                            and piece_cost(inter[0]) > budget[0]):
                        return
                    p = inter.popleft()
                    run_piece(p)
                    budget[0] -= piece_cost(p)
                    if to_marker is not None and p == ("HPM", to_marker):
                        return

            def do_sc(qi, hp, kg, inter):
                sc_use(qi, hp, kg)
                budget[0] = min(budget[0] + (EXP_NS - SC_NS), 3200.0)
                drain(inter)

            # slot 0: interleave K/Q chunk-0 projections with the first score
            # groups (head pair hp becomes ready as soon as dl-tile hp
            # projects), then kg 1-3 as K chunks 1-3 land.
            inter0 = deque(slot_inter[0])
            for dch in range(NDT):
                proj_piece(0, dch, wk, "xk", bk_t, kt)
                proj_piece(0, dch, wq, "xq", bq_t, qt)
                sc_use(0, dch, 0)
            xk_next = {}
            load_xk(1)
            xk_next[1] = vstate["xk"]
            for kg in range(1, NKG):
                vstate["xk"] = xk_next[kg]
                if kg + 1 < NKG:
                    load_xk(kg + 1)
                    xk_next[kg + 1] = vstate["xk"]
                    vstate["xk"] = xk_next[kg]
                for dch in range(NDT):
                    proj_piece(kg, dch, wk, "xk", bk_t, kt)
                for hp in range(HP):
                    do_sc(0, hp, kg, inter0)
            drain(inter0, force=True)
            s1.close()
            open_vx()

            for qi in range(1, ST):
                inter = deque(slot_inter[qi])
                for hp in range(HP):
                    if hp in slot_markers[qi]:
                        drain(inter, to_marker=hp)
                    for p in slot_hp[qi][hp]:
                        run_piece(p)
                    for kg in range(NKG):
                        if hp == HP - 1 and kg == 2:
                            # flush leftovers while the previous exp groups
                            # still cover the scalar engine
                            drain(inter, force=True)
                        do_sc(qi, hp, kg, inter)
                drain(inter, force=True)

            # ---------------- tail ----------------------------------------
            for piece in (attnv_hp(ST - 1, 0) + attnv_hp(ST - 1, 1)
                          + [(norm_piece, ST - 1, 0)]):
                run_piece(piece)
            transp_qt(ST - 1, half=0)
            for piece in (attnv_hp(ST - 1, 2) + attnv_hp(ST - 1, 3)
                          + [(norm_piece, ST - 1, 1)]):
                run_piece(piece)
            transp_qt(ST - 1, half=1)
            for st in range(ST - 2, ST):
                for fc in range(D // QC):
                    c_piece(st, fc)
            s3.close()

    nc.compile()
    return nc


def make_in_maps(query, key, value, Wq, bq, Wk, bk, Wv, bv, n_cores=8,
                 mm_dtype="float32r"):
    """Host-side sharding: slice weights Megatron-style, transpose activations."""
    import ml_dtypes

    bft = ml_dtypes.bfloat16
    q = np.asarray(query, dtype=np.float32)
    k = np.asarray(key, dtype=np.float32)
    v = np.asarray(value, dtype=np.float32)
    Wq = np.asarray(Wq, dtype=np.float32)
    Wk = np.asarray(Wk, dtype=np.float32)
    Wv = np.asarray(Wv, dtype=np.float32)
    bq = np.asarray(bq, dtype=np.float32)
    bk = np.asarray(bk, dtype=np.float32)
    D = Wq.shape[0]
    DL = D // (n_cores // q.shape[0])
    scale = 1.0 / np.sqrt(np.float32(DK))
    in_maps = []
    for c in range(n_cores):
        b, g = divmod(c, n_cores // q.shape[0])
        sl = slice(DL * g, DL * (g + 1))
        in_maps.append(
            {
                "xqT": np.ascontiguousarray(q[b].T).astype(bft),
                "xkT": np.ascontiguousarray(k[b].T).astype(bft),
                "xvT": np.ascontiguousarray(v[b].T).astype(bft),
                "wqT": (np.ascontiguousarray(Wq[sl].T) * scale).astype(bft),
                "wkT": np.ascontiguousarray(Wk[sl].T).astype(bft),
                "wvT": np.ascontiguousarray(Wv[sl].T).astype(bft),
                "bq": np.ascontiguousarray((bq[sl] * scale).reshape(DL, 1)),
                "bk": np.ascontiguousarray(bk[sl].reshape(DL, 1)),
            }
        )
    return in_maps


def add_wo_maps(in_maps, Wo, n_cores=8, n_batch=4, mm_dtype="float32r"):
    import ml_dtypes

    Wo = np.asarray(Wo, dtype=np.float32)
    D = Wo.shape[0]
    DL = D // (n_cores // n_batch)
    for c in range(n_cores):
        _, g = divmod(c, n_cores // n_batch)
        sl = slice(DL * g, DL * (g + 1))
        in_maps[c]["woT"] = np.ascontiguousarray(Wo[:, sl].T).astype(ml_dtypes.bfloat16)
    return in_maps


MM_DTYPE = "float32r"


def kernel(query, key, value, Wq, bq, Wk, bk, Wv, bv, Wo, bo):
    if "nc" not in _CACHE:
        _CACHE["nc"] = build_nc(mm_dtype=MM_DTYPE)
    nc = _CACHE["nc"]
    n_cores = 8
    in_maps = make_in_maps(
        query, key, value, Wq, bq, Wk, bk, Wv, bv, n_cores, MM_DTYPE
    )
    add_wo_maps(in_maps, Wo, n_cores, np.asarray(query).shape[0], MM_DTYPE)
    res = run_bass_kernel_spmd(nc, in_maps, list(range(n_cores)))
    ys = [res.results[c]["y"] for c in range(n_cores)]
    bo = np.asarray(bo, dtype=np.float32)
    bv = np.asarray(bv, dtype=np.float32)
    Wo = np.asarray(Wo, dtype=np.float32)
    const = bo + bv @ Wo.T
    out = np.stack([ys[2 * b] + ys[2 * b + 1] for b in range(4)]) + const[None, None, :]
    return out.astype(np.float32)
